# revision 1
# baseline (speedup 1.0000x reference)
"""Trainium2 distributed kernel for a dense transformer block (8 NeuronCores).

Sharding: tokens are data-parallel for LN/QKV/proj/MLP (512 tokens/core,
causal-balanced pairing: core i owns batch0 chunk i and batch1 chunk 7-i),
attention is head-parallel (2 heads/core) via an AllToAll exchange of
Q/K/V, plus a second AllToAll to bring attention outputs back to token
sharding.  All matmuls run in bf16 (f32 accumulation in PSUM); LayerNorm
statistics are computed with ones-vector matmuls so every activation
stays in transposed [d, token] layout on chip.
"""

import sys

sys.path.insert(0, "/opt/trn_rl_repo")

import numpy as np
import ml_dtypes

NCORES = 8
D = 1024
H = 16
DH = 64
HL = H // NCORES  # heads per core = 2
B = 2
S = 2048
T = 512  # tokens per core
CH = 256  # token chunk (half of T)
DFF = 4096
P = 128
QR, KR, VR = 128, 128, 130  # slot row counts: qT, kT, packed-v regions
SLOT = QR + KR + VR  # 386
EPS = 1e-5

_CACHE = {}
TRACE = False


def _emit_block(nc, tc, env, rep, x_tiles, collectives, write_out):
    """Emit one transformer block; returns the 8 output [128,T] f32 tiles."""
    from concourse import bass, mybir

    f32 = mybir.dt.float32
    bf16 = mybir.dt.bfloat16
    Alu = mybir.AluOpType
    AFT = mybir.ActivationFunctionType

    (xT, wT, wpT, wuT, wdT, out) = env["params"]
    (a1i, a1o, a2i, a2o, a1i_flat, a1o_flat) = env["bounce"]
    if not collectives:
        a1o, a1o_flat, a2o = a1i, a1i_flat, a2i
    c = env["consts"]
    pools = env["pools"]
    vec, recp = pools["vec"], pools["recp"]
    rg = [list(range(NCORES))]

    def layer_norm_T(x_tiles, g_tile, b_tile, out_pool, pfx):
        with tc.tile_pool(name=f"ln_ps{pfx}", bufs=2, space="PSUM") as psp, tc.tile_pool(
            name=f"ln_tmp{pfx}", bufs=3
        ) as tmp_p:
            ps_sum = psp.tile([1, T], f32, name="ps_sum", tag="ps_sum")
            ps_sq = psp.tile([1, T], f32, name="ps_sq", tag="ps_sq")
            for dk in range(8):
                xb = tmp_p.tile([P, T], bf16, name="xb", tag="xb")
                nc.scalar.activation(xb[:], x_tiles[dk][:], AFT.Copy)
                nc.tensor.matmul(
                    ps_sum[:], c["ones"][:], xb[:], start=(dk == 0), stop=(dk == 7)
                )
                sq = tmp_p.tile([P, T], bf16, name="sq", tag="sq")
                nc.scalar.activation(sq[:], x_tiles[dk][:], AFT.Square)
                nc.tensor.matmul(
                    ps_sq[:], c["ones"][:], sq[:], start=(dk == 0), stop=(dk == 7)
                )
            mu = vec.tile([1, T], f32, name="mu", tag="lnvec")
            nc.vector.tensor_scalar(mu[:], ps_sum[:], 1.0 / D, None, Alu.mult)
            msq = vec.tile([1, T], f32, name="msq", tag="lnvec")
            nc.vector.tensor_scalar(msq[:], ps_sq[:], 1.0 / D, None, Alu.mult)
            mu2 = vec.tile([1, T], f32, name="mu2", tag="lnvec")
            nc.vector.tensor_tensor(mu2[:], mu[:], mu[:], Alu.mult)
            var = vec.tile([1, T], f32, name="var", tag="lnvec")
            nc.vector.tensor_tensor(var[:], msq[:], mu2[:], Alu.subtract)
            nc.vector.tensor_scalar(var[:], var[:], EPS, None, Alu.add)
            std = vec.tile([1, T], f32, name="std", tag="lnvec")
            nc.scalar.activation(std[:], var[:], AFT.Sqrt)
            rstd = vec.tile([1, T], f32, name="rstd", tag="lnvec")
            nc.vector.reciprocal(rstd[:], std[:])
            mur = vec.tile([1, T], f32, name="mur", tag="lnvec")
            nc.vector.tensor_tensor(mur[:], mu[:], rstd[:], Alu.mult)
            rstd_c = vec.tile([1, T], bf16, name="rstd_c", tag="lnvec")
            nc.vector.tensor_copy(rstd_c[:], rstd[:])
            mur_c = vec.tile([1, T], bf16, name="mur_c", tag="lnvec")
            nc.vector.tensor_copy(mur_c[:], mur[:])
            rstd_b = psp.tile([P, T], f32, name="rstd_b", tag="rstd_b")
            nc.tensor.matmul(rstd_b[:], c["ones_row"][:], rstd_c[:], start=True, stop=True)
            mur_b = psp.tile([P, T], f32, name="mur_b", tag="mur_b")
            nc.tensor.matmul(mur_b[:], c["ones_row"][:], mur_c[:], start=True, stop=True)
            outs = []
            for dk in range(8):
                t1 = tmp_p.tile([P, T], f32, name="lnt1", tag="lnt1")
                nc.vector.tensor_tensor(t1[:], x_tiles[dk][:], rstd_b[:], Alu.mult)
                nc.vector.tensor_tensor(t1[:], t1[:], mur_b[:], Alu.subtract)
                o = out_pool.tile([P, T], bf16, name="ln_out", tag="ln_out")
                nc.scalar.activation(
                    o[:], t1[:], AFT.Identity,
                    bias=b_tile[:, dk : dk + 1], scale=g_tile[:, dk : dk + 1],
                )
                outs.append(o)
            return outs

    # ---------------- LN1 ----------------
    h_tiles = layer_norm_T(x_tiles, c["g1"], c["be1"], pools["ht"], f"a{rep}")

    # ---------------- QKV ----------------
    with tc.tile_pool(name=f"wqk{rep}", bufs=16) as wqk_p, tc.tile_pool(
        name=f"qkv_ps{rep}", bufs=4, space="PSUM"
    ) as qkv_ps:
        for blk in range(4):
            wts = []
            for dk in range(8):
                wt = wqk_p.tile([P, 512], bf16, name="wqk", tag="wqk")
                nc.sync.dma_start(
                    wt[:], wT[dk * P : (dk + 1) * P, blk * 512 : (blk + 1) * 512]
                )
                wts.append(wt)
            for jl in range(4):
                jt = blk * 4 + jl  # 0..15 (0-7 q, 8-15 k)
                ps = qkv_ps.tile([P, T], f32, name="qk_ps", tag="qk_ps")
                for dk in range(8):
                    nc.tensor.matmul(
                        ps[:], wts[dk][:, jl * P : (jl + 1) * P], h_tiles[dk][:],
                        start=(dk == 0), stop=(dk == 7),
                    )
                stg = pools["stg"].tile([P, T], bf16, name="stg", tag="stg")
                nc.scalar.activation(
                    stg[:], ps[:], AFT.Identity, bias=c["bqk"][:, jt : jt + 1]
                )
                r0 = jt * SLOT if jt < 8 else (jt - 8) * SLOT + QR
                nc.sync.dma_start(a1i[r0 : r0 + P, :], stg[:])

    with tc.tile_pool(name=f"wv{rep}", bufs=8) as wv_p, tc.tile_pool(
        name=f"vst{rep}", bufs=8
    ) as vst_p, tc.tile_pool(name=f"v_ps{rep}", bufs=3, space="PSUM") as v_ps:
        for jc in range(2):
            wvt = []
            for dk in range(8):
                wt = wv_p.tile([P, 512], bf16, name="wv", tag="wv")
                nc.sync.dma_start(
                    wt[:],
                    wT[dk * P : (dk + 1) * P, 2048 + jc * 512 : 2048 + (jc + 1) * 512],
                )
                wvt.append(wt)
            for tt in range(4):
                ps = v_ps.tile([P, 512], f32, name="v_ps", tag="v_ps")
                for dk in range(8):
                    nc.tensor.matmul(
                        ps[:], h_tiles[dk][:, tt * P : (tt + 1) * P], wvt[dk][:],
                        start=(dk == 0), stop=(dk == 7),
                    )
                for sl in range(4):
                    slot = jc * 4 + sl
                    vt = vst_p.tile([P, VR], bf16, name="vst", tag="vst")
                    for lh in range(HL):
                        cc = slot * P + lh * DH - jc * 512
                        nc.vector.tensor_tensor(
                            vt[:, lh * 65 : lh * 65 + DH],
                            ps[:, cc : cc + DH],
                            c["bv"][:, slot * P + lh * DH : slot * P + lh * DH + DH],
                            Alu.add,
                        )
                        nc.vector.memset(vt[:, lh * 65 + DH : lh * 65 + DH + 1], 1.0)
                    off = (slot * SLOT + QR + KR) * T + (tt * P) * VR
                    dst = a1i_flat[off : off + P * VR].rearrange("(p c) -> p c", c=VR)
                    nc.sync.dma_start(dst, vt[:])

    # ---------------- AllToAll #1 ----------------
    if collectives:
        nc.gpsimd.collective_compute(
            "AllToAll", mybir.AluOpType.bypass, replica_groups=rg,
            ins=[a1i.ap().opt()], outs=[a1o.ap().opt()],
        )

    # ---------------- attention ----------------
    a2_stage = [
        pools["a2stg"].tile([P, T], bf16, name=f"a2stg{j}", tag="a2stg")
        for j in range(8)
    ]
    with tc.tile_pool(name=f"kv{rep}", bufs=20) as kv_p, tc.tile_pool(
        name=f"qe{rep}", bufs=6
    ) as qe_p, tc.tile_pool(name=f"s_ps{rep}", bufs=4, space="PSUM") as s_ps, tc.tile_pool(
        name=f"o_ps{rep}", bufs=2, space="PSUM"
    ) as o_ps:
        for lh in range(HL):
            for b in range(B):
                k_ts, v_ts = [], []
                for kc in range(8):
                    slot = kc if b == 0 else 7 - kc
                    col0 = 0 if b == 0 else CH
                    kt = kv_p.tile([DH, CH], bf16, name="kt", tag="kt")
                    nc.sync.dma_start(
                        kt[:],
                        a1o[
                            slot * SLOT + QR + lh * DH : slot * SLOT + QR + (lh + 1) * DH,
                            col0 : col0 + CH,
                        ],
                    )
                    k_ts.append(kt)
                    for sub in range(2):
                        vt = kv_p.tile([P, 65], bf16, name="vt", tag="vt")
                        off = (
                            (slot * SLOT + QR + KR) * T + (col0 + sub * P) * VR + lh * 65
                        )
                        vsrc = bass.AP(a1o, off, [[VR, P], [1, 65]])
                        nc.sync.dma_start(vt[:], vsrc)
                        v_ts.append(vt)
                for pr in range(4):  # query-chunk pairs (2pr, 2pr+1)
                    q0, q1 = 2 * pr, 2 * pr + 1
                    s0 = q0 if b == 0 else 7 - q0
                    s1 = q1 if b == 0 else 7 - q1
                    qcol0 = 0 if b == 0 else CH
                    qt = qe_p.tile([DH, 2 * CH], bf16, name="qt", tag="qt")
                    nc.sync.dma_start(
                        qt[:, 0:CH],
                        a1o[s0 * SLOT + lh * DH : s0 * SLOT + (lh + 1) * DH,
                            qcol0 : qcol0 + CH],
                    )
                    nc.sync.dma_start(
                        qt[:, CH : 2 * CH],
                        a1o[s1 * SLOT + lh * DH : s1 * SLOT + (lh + 1) * DH,
                            qcol0 : qcol0 + CH],
                    )
                    po = o_ps.tile([65, 2 * CH], f32, name="o_ps", tag="o_ps")
                    n_mm = 2 * (q1 + 1)
                    mi = 0
                    for kc in range(q1 + 1):
                        for sub in range(2):
                            ps = s_ps.tile([P, 2 * CH], f32, name="s_ps", tag="s_ps")
                            nc.tensor.matmul(
                                ps[:], k_ts[kc][:, sub * P : (sub + 1) * P], qt[:],
                                start=True, stop=True,
                            )
                            E = qe_p.tile([P, 2 * CH], bf16, name="E", tag="E")
                            nc.scalar.activation(E[:], ps[:], AFT.Exp, scale=0.125)
                            if kc == q0:  # diagonal for q0, below-diag for q1
                                nc.vector.tensor_tensor(
                                    E[:], E[:], c["tri_lo"][sub][:], Alu.mult
                                )
                            elif kc == q1:  # above-diag for q0, diagonal for q1
                                nc.vector.tensor_tensor(
                                    E[:], E[:], c["tri_hi"][sub][:], Alu.mult
                                )
                            nc.tensor.matmul(
                                po[:], v_ts[kc * 2 + sub][:], E[:],
                                start=(mi == 0), stop=(mi == n_mm - 1),
                            )
                            mi += 1
                    rec = recp.tile([1, 2 * CH], bf16, name="rec", tag="rec")
                    with nc.allow_low_precision(reason="softmax denom bcast"):
                        nc.vector.reciprocal(rec[:], po[64:65, :])
                    rec_ps = s_ps.tile(
                        [DH, 2 * CH], f32, name="rec_ps", tag="rec_ps", bufs=2
                    )
                    nc.tensor.matmul(
                        rec_ps[:], c["ones_row"][:, 0:DH], rec[:], start=True, stop=True
                    )
                    rec_b = recp.tile([DH, 2 * CH], f32, name="rec_b", tag="rec_b")
                    nc.vector.tensor_copy(rec_b[:], rec_ps[:])
                    for half, sq in ((0, s0), (1, s1)):
                        nc.vector.tensor_tensor(
                            a2_stage[sq][lh * DH : (lh + 1) * DH, qcol0 : qcol0 + CH],
                            po[0:DH, half * CH : (half + 1) * CH],
                            rec_b[:, half * CH : (half + 1) * CH],
                            Alu.mult,
                        )
    for j in range(8):
        nc.sync.dma_start(a2i[j * P : (j + 1) * P, :], a2_stage[j][:])

    # ---------------- AllToAll #2 ----------------
    if collectives:
        nc.gpsimd.collective_compute(
            "AllToAll", mybir.AluOpType.bypass, replica_groups=rg,
            ins=[a2i.ap().opt()], outs=[a2o.ap().opt()],
        )

    # ---------------- proj + residual1 ----------------
    o_tiles = []
    for dk in range(8):
        ot = pools["ot"].tile([P, T], bf16, name="ot", tag="ot")
        nc.sync.dma_start(ot[:], a2o[dk * P : (dk + 1) * P, :])
        o_tiles.append(ot)
    x1_tiles = []
    with tc.tile_pool(name=f"wp{rep}", bufs=16) as wp_p, tc.tile_pool(
        name=f"p_ps{rep}", bufs=3, space="PSUM"
    ) as p_ps:
        wpt = {}
        for dk in range(8):
            for db in range(2):
                wt = wp_p.tile([P, 512], bf16, name="wp", tag="wp")
                nc.sync.dma_start(
                    wt[:], wpT[dk * P : (dk + 1) * P, db * 512 : (db + 1) * 512]
                )
                wpt[(dk, db)] = wt
        for do in range(8):
            ps = p_ps.tile([P, T], f32, name="p_ps", tag="p_ps")
            for dk in range(8):
                nc.tensor.matmul(
                    ps[:],
                    wpt[(dk, do // 4)][:, (do % 4) * P : (do % 4 + 1) * P],
                    o_tiles[dk][:],
                    start=(dk == 0), stop=(dk == 7),
                )
            x1 = pools["x1"].tile([P, T], f32, name="x1", tag="x1")
            nc.vector.tensor_scalar(x1[:], ps[:], c["bp"][:, do : do + 1], None, Alu.add)
            nc.vector.tensor_tensor(x1[:], x1[:], x_tiles[do][:], Alu.add)
            x1_tiles.append(x1)

    # ---------------- LN2 ----------------
    h2_tiles = layer_norm_T(x1_tiles, c["g2"], c["be2"], pools["h2"], f"b{rep}")

    # ---------------- MLP up + gelu ----------------
    gu_tiles = []
    with tc.tile_pool(name=f"wu{rep}", bufs=16) as wu_p, tc.tile_pool(
        name=f"u_ps{rep}", bufs=3, space="PSUM"
    ) as u_ps:
        for jb in range(8):
            wut = []
            for dk in range(8):
                wt = wu_p.tile([P, 512], bf16, name="wu", tag="wu")
                nc.sync.dma_start(
                    wt[:], wuT[dk * P : (dk + 1) * P, jb * 512 : (jb + 1) * 512]
                )
                wut.append(wt)
            for jl in range(4):
                j = jb * 4 + jl
                ps = u_ps.tile([P, T], f32, name="u_ps", tag="u_ps")
                for dk in range(8):
                    nc.tensor.matmul(
                        ps[:], wut[dk][:, jl * P : (jl + 1) * P], h2_tiles[dk][:],
                        start=(dk == 0), stop=(dk == 7),
                    )
                gu = pools["gu"].tile([P, T], bf16, name="gu", tag="gu")
                nc.scalar.activation(
                    gu[:], ps[:], AFT.Gelu_apprx_tanh, bias=c["bu"][:, j : j + 1]
                )
                gu_tiles.append(gu)

    # ---------------- MLP down + residual2 ----------------
    out_tiles = []
    with tc.tile_pool(name=f"wd{rep}", bufs=34) as wd_p, tc.tile_pool(
        name=f"d_ps{rep}", bufs=3, space="PSUM"
    ) as d_ps:
        for db in range(2):
            wdt = []
            for j in range(32):
                wt = wd_p.tile([P, 512], bf16, name="wd", tag="wd")
                nc.sync.dma_start(
                    wt[:], wdT[j * P : (j + 1) * P, db * 512 : (db + 1) * 512]
                )
                wdt.append(wt)
            for dol in range(4):
                do = db * 4 + dol
                ps = d_ps.tile([P, T], f32, name="d_ps", tag="d_ps")
                for j in range(32):
                    nc.tensor.matmul(
                        ps[:], wdt[j][:, dol * P : (dol + 1) * P], gu_tiles[j][:],
                        start=(j == 0), stop=(j == 31),
                    )
                o = pools["outp"].tile([P, T], f32, name="out_t", tag="out_t")
                nc.vector.tensor_scalar(
                    o[:], ps[:], c["bd"][:, do : do + 1], None, Alu.add
                )
                nc.vector.tensor_tensor(o[:], o[:], x1_tiles[do][:], Alu.add)
                if write_out:
                    nc.sync.dma_start(out[do * P : (do + 1) * P, :], o[:])
                out_tiles.append(o)
    return out_tiles


def _build(nreps=1, collectives=True):
    from contextlib import ExitStack
    from concourse import bass, mybir, tile, bacc

    f32 = mybir.dt.float32
    bf16 = mybir.dt.bfloat16

    nc = bacc.Bacc("TRN2", target_bir_lowering=False, num_devices=NCORES)

    xT = nc.declare_dram_parameter("xT", [D, T], f32, isOutput=False)
    wT = nc.declare_dram_parameter("wT", [D, 3 * D], bf16, isOutput=False)
    wpT = nc.declare_dram_parameter("wpT", [D, D], bf16, isOutput=False)
    wuT = nc.declare_dram_parameter("wuT", [D, DFF], bf16, isOutput=False)
    wdT = nc.declare_dram_parameter("wdT", [DFF, D], bf16, isOutput=False)
    bqk = nc.declare_dram_parameter("bqk", [P, 16], f32, isOutput=False)
    bv = nc.declare_dram_parameter("bv", [P, D], f32, isOutput=False)
    bp = nc.declare_dram_parameter("bp", [P, 8], f32, isOutput=False)
    bu = nc.declare_dram_parameter("bu", [P, 32], f32, isOutput=False)
    bd = nc.declare_dram_parameter("bd", [P, 8], f32, isOutput=False)
    g1 = nc.declare_dram_parameter("g1", [P, 8], f32, isOutput=False)
    be1 = nc.declare_dram_parameter("be1", [P, 8], f32, isOutput=False)
    g2 = nc.declare_dram_parameter("g2", [P, 8], f32, isOutput=False)
    be2 = nc.declare_dram_parameter("be2", [P, 8], f32, isOutput=False)
    tri = nc.declare_dram_parameter("tri", [CH, CH], bf16, isOutput=False)
    tri2 = nc.declare_dram_parameter("tri2", [CH, 4 * CH], bf16, isOutput=False)
    out = nc.declare_dram_parameter("out", [D, T], f32, isOutput=True)

    a1i = nc.dram_tensor("a2a1_in", [NCORES * SLOT, T], bf16)
    a1o = nc.dram_tensor("a2a1_out", [NCORES * SLOT, T], bf16)
    a2i = nc.dram_tensor("a2a2_in", [NCORES * QR, T], bf16)
    a2o = nc.dram_tensor("a2a2_out", [NCORES * QR, T], bf16)
    a1i_flat = a1i.ap().rearrange("a b -> (a b)")
    a1o_flat = a1o.ap().rearrange("a b -> (a b)")

    with tile.TileContext(nc) as tc, ExitStack() as top:
        const = top.enter_context(tc.tile_pool(name="const", bufs=1))
        ones = const.tile([P, 1], bf16)
        nc.vector.memset(ones[:], 1.0)
        ones_row = const.tile([1, P], bf16)
        nc.vector.memset(ones_row[:], 1.0)
        tri_t = [const.tile([P, CH], bf16, name=f"tri{s}", tag=f"tri{s}") for s in range(2)]
        for s in range(2):
            nc.sync.dma_start(tri_t[s][:], tri[s * P : (s + 1) * P, :])
        tri2_t = [const.tile([P, 4 * CH], bf16, name=f"tri2{s}", tag=f"tri2{s}") for s in range(2)]
        for s in range(2):
            nc.sync.dma_start(tri2_t[s][:], tri2[s * P : (s + 1) * P, :])

        def ctile(name, param, shape):
            t = const.tile(shape, f32, name=name, tag=name)
            nc.sync.dma_start(t[:], param[:, :])
            return t

        consts = {
            "ones": ones, "ones_row": ones_row, "tri": tri_t,
            "tri_lo": [tri2_t[s][:, 0 : 2 * CH] for s in range(2)],
            "tri_hi": [tri2_t[s][:, 2 * CH : 4 * CH] for s in range(2)],
            "bqk": ctile("bqk_t", bqk, [P, 16]),
            "bv": ctile("bv_t", bv, [P, D]),
            "bp": ctile("bp_t", bp, [P, 8]),
            "bu": ctile("bu_t", bu, [P, 32]),
            "bd": ctile("bd_t", bd, [P, 8]),
            "g1": ctile("g1_t", g1, [P, 8]),
            "be1": ctile("be1_t", be1, [P, 8]),
            "g2": ctile("g2_t", g2, [P, 8]),
            "be2": ctile("be2_t", be2, [P, 8]),
        }

        pools = {
            "vec": top.enter_context(tc.tile_pool(name="vec", bufs=6)),
            "recp": top.enter_context(tc.tile_pool(name="recp", bufs=4)),
            "xt": top.enter_context(tc.tile_pool(name="xt", bufs=8)),
            "ht": top.enter_context(tc.tile_pool(name="ht", bufs=8)),
            "stg": top.enter_context(tc.tile_pool(name="stg", bufs=8)),
            "a2stg": top.enter_context(tc.tile_pool(name="a2stg", bufs=8)),
            "ot": top.enter_context(tc.tile_pool(name="ot", bufs=8)),
            "x1": top.enter_context(tc.tile_pool(name="x1", bufs=8)),
            "h2": top.enter_context(tc.tile_pool(name="h2", bufs=8)),
            "gu": top.enter_context(tc.tile_pool(name="gu", bufs=32)),
            "outp": top.enter_context(tc.tile_pool(name="outp", bufs=8)),
        }

        env = {
            "params": (xT, wT, wpT, wuT, wdT, out),
            "bounce": (a1i, a1o, a2i, a2o, a1i_flat, a1o_flat),
            "consts": consts,
            "pools": pools,
        }

        x_tiles = []
        for dk in range(8):
            xt = pools["xt"].tile([P, T], f32, name="xt", tag="xt")
            nc.sync.dma_start(xt[:], xT[dk * P : (dk + 1) * P, :])
            x_tiles.append(xt)

        cur = x_tiles
        for rep in range(nreps):
            cur = _emit_block(
                nc, tc, env, rep, cur, collectives, write_out=(rep == nreps - 1)
            )

    nc.finalize()
    return nc


def _get_nc():
    if "nc" not in _CACHE:
        _CACHE["nc"] = _build()
    return _CACHE["nc"]


def _make_in_maps(inputs):
    x = np.asarray(inputs["x"], np.float32)
    ln1_g = np.asarray(inputs["ln1_g"], np.float32)
    ln1_b = np.asarray(inputs["ln1_b"], np.float32)
    W_attn = np.asarray(inputs["W_attn"], np.float32)
    b_attn = np.asarray(inputs["b_attn"], np.float32)
    W_proj = np.asarray(inputs["W_proj"], np.float32)
    b_proj = np.asarray(inputs["b_proj"], np.float32)
    ln2_g = np.asarray(inputs["ln2_g"], np.float32)
    ln2_b = np.asarray(inputs["ln2_b"], np.float32)
    W_up = np.asarray(inputs["W_up"], np.float32)
    b_up = np.asarray(inputs["b_up"], np.float32)
    W_down = np.asarray(inputs["W_down"], np.float32)
    b_down = np.asarray(inputs["b_down"], np.float32)

    bf = ml_dtypes.bfloat16
    wT = np.ascontiguousarray(W_attn.T).astype(bf)
    wpT = np.ascontiguousarray(W_proj.T).astype(bf)
    wuT = np.ascontiguousarray(W_up.T).astype(bf)
    wdT = np.ascontiguousarray(W_down.T).astype(bf)

    def cols(v):  # [N] -> [128, N//128]: col j = v[j*128:(j+1)*128]
        return np.ascontiguousarray(v.reshape(-1, P).T).astype(np.float32)

    tri = np.tril(np.ones((CH, CH), np.float32)).T.astype(bf)  # tri[a,b] = a<=b
    tri = np.ascontiguousarray(tri)

    ones_m = np.ones((CH, CH), np.float32)
    zeros_m = np.zeros((CH, CH), np.float32)
    tri_f = np.tril(np.ones((CH, CH), np.float32)).T
    tri2 = np.ascontiguousarray(
        np.concatenate([tri_f, ones_m, zeros_m, tri_f], axis=1)
    ).astype(bf)

    common = dict(
        wT=wT, wpT=wpT, wuT=wuT, wdT=wdT, tri2=tri2,
        bqk=cols(b_attn[: 2 * D]),
        bv=np.ascontiguousarray(np.broadcast_to(b_attn[2 * D :].reshape(1, D), (P, D))),
        bp=cols(b_proj), bu=cols(b_up), bd=cols(b_down),
        g1=cols(ln1_g), be1=cols(ln1_b), g2=cols(ln2_g), be2=cols(ln2_b),
        tri=tri,
    )

    in_maps = []
    for i in range(NCORES):
        c0 = x[0, i * CH : (i + 1) * CH]  # [256, 1024]
        c1 = x[1, (7 - i) * CH : (8 - i) * CH]
        xTi = np.ascontiguousarray(np.concatenate([c0, c1], 0).T)  # [1024, 512]
        in_maps.append(dict(common, xT=xTi))
    return in_maps


def make_in_maps(inputs):
    return _make_in_maps(inputs)


def kernel(**inputs):
    in_maps = _make_in_maps(inputs)

    from concourse import bass_utils

    nc = _get_nc()
    res = bass_utils.run_bass_kernel_spmd(
        nc, in_maps, core_ids=list(range(NCORES)), trace=TRACE
    )
    _CACHE["last_res"] = res
    y = np.empty((B, S, D), np.float32)
    for i in range(NCORES):
        o = np.asarray(res.results[i]["out"], np.float32)  # [1024, 512]
        y[0, i * CH : (i + 1) * CH] = o[:, :CH].T
        y[1, (7 - i) * CH : (8 - i) * CH] = o[:, CH:].T
    return y



# revision 34
# speedup vs baseline: 178.0172x; 178.0172x over previous
"""Trainium2 distributed kernel for a dense transformer block (8 NeuronCores).

Sharding: tokens are data-parallel for LN/QKV/proj/MLP (512 tokens/core,
causal-balanced pairing: core i owns batch0 chunk i and batch1 chunk 7-i),
attention is head-parallel (2 heads/core) via an AllToAll exchange of
Q/K/V, plus a second AllToAll to bring attention outputs back to token
sharding.  All matmuls run in bf16 (f32 accumulation in PSUM); LayerNorm
statistics are computed with ones-vector matmuls so every activation
stays in transposed [d, token] layout on chip.

DMA strategy: weight/activation transfers are batched into wide tiles
(multi-block access patterns) to minimize HWDGE descriptor-queue
serialization; attention Q/K/V live in single wide SBUF tiles sliced
per-head/chunk, with batch1 slot order reversed so both batches index
chunks in ascending global order.
"""

import sys

sys.path.insert(0, "/opt/trn_rl_repo")

import numpy as np
import ml_dtypes

NCORES = 8
D = 1024
H = 16
DH = 64
HL = H // NCORES  # heads per core = 2
B = 2
S = 2048
T = 512  # tokens per core
CH = 256  # token chunk (half of T)
DFF = 4096
P = 128
QR, KR, VR = 128, 128, 130  # slot row counts: qT, kT, packed-v regions
SLOT = QR + KR + VR  # 386
EPS = 1e-5

_CACHE = {}
TRACE = False


def _emit_block(nc, tc, env, rep, x_tiles, x_all, collectives, write_out):
    """Emit one transformer block; returns the 8 output [128,T] f32 tiles."""
    from contextlib import ExitStack
    from concourse import bass, mybir

    f32 = mybir.dt.float32
    bf16 = mybir.dt.bfloat16
    f8 = mybir.dt.float8e4
    Alu = mybir.AluOpType
    AFT = mybir.ActivationFunctionType

    (xT, wT, wpT, wuT, wdT, out) = env["params"]
    (a1i, a1o, a2i, a2o) = env["bounce"]
    if not collectives:
        a1o, a2o = a1i, a2i
    c = env["consts"]
    pools = env["pools"]
    vec, recp = pools["vec"], pools["recp"]
    rg = [list(range(NCORES))]

    def ap3(t, part0, nprt, off, dims):
        """Custom free-dim AP on tile t at partition slice [part0, part0+nprt)."""
        base = t[:]
        pstride = base.ap[0][0]
        return bass.AP(base.tensor, base.offset + part0 * pstride + off,
                      [[pstride, nprt]] + dims)

    def ln_stats_half(x_tiles, psp, tmp_p, h, ps_sum, ps_sq):
        c0, c1 = h * CH, (h + 1) * CH
        for dk in range(8):
            nc.tensor.matmul(
                ps_sum[:, c0:c1], c["ones"][:], x_tiles[dk][:, c0:c1],
                start=(dk == 0), stop=(dk == 7),
            )
            sq = tmp_p.tile([P, CH], bf16, name="sq", tag="sq")
            nc.scalar.activation(sq[:], x_tiles[dk][:, c0:c1], AFT.Square)
            nc.tensor.matmul(
                ps_sq[:, c0:c1], c["ones"][:], sq[:],
                start=(dk == 0), stop=(dk == 7),
            )

    def ln_finish(x_tiles, g_tile, b_tile, out_pool, psp, tmp_p, ps_sum, ps_sq):
        ch = vec.tile([1, 8 * T], f32, name="lnchain", tag="lnchain")
        chb = vec.tile([1, 2 * T], bf16, name="lnchainb", tag="lnchainb")
        mu, msq, mu2, var, std, rstd, mur = (
            ch[0:1, i * T : (i + 1) * T] for i in range(7))
        rstd_c, mur_c = chb[0:1, 0:T], chb[0:1, T : 2 * T]
        nc.vector.tensor_scalar(mu, ps_sum[:], 1.0 / D, None, Alu.mult)
        nc.vector.tensor_scalar(msq, ps_sq[:], 1.0 / D, None, Alu.mult)
        nc.vector.tensor_tensor(mu2, mu, mu, Alu.mult)
        nc.vector.tensor_tensor(var, msq, mu2, Alu.subtract)
        nc.vector.tensor_scalar(var, var, EPS, None, Alu.add)
        nc.scalar.activation(std, var, AFT.Sqrt)
        nc.vector.reciprocal(rstd, std)
        nc.vector.tensor_tensor(mur, mu, rstd, Alu.mult)
        nc.vector.tensor_copy(rstd_c, rstd)
        nc.vector.tensor_copy(mur_c, mur)
        rstd_b = psp.tile([P, T], f32, name="rstd_b", tag="rstd_b")
        nc.tensor.matmul(rstd_b[:], c["ones_row"][:], rstd_c, start=True, stop=True)
        mur_b = psp.tile([P, T], f32, name="mur_b", tag="mur_b")
        nc.tensor.matmul(mur_b[:], c["ones_row"][:], mur_c, start=True, stop=True)
        rstd_bb = tmp_p.tile([P, T], bf16, name="rstd_bb", tag="rstd_bb")
        nc.scalar.activation(rstd_bb[:], rstd_b[:], AFT.Copy)
        mur_bb = tmp_p.tile([P, T], bf16, name="mur_bb", tag="mur_bb")
        nc.scalar.activation(mur_bb[:], mur_b[:], AFT.Copy)
        outs = []
        for dk in range(8):
            t1 = tmp_p.tile([P, T], bf16, name="lnt1", tag="lnt1")
            nc.vector.tensor_tensor(t1[:], x_tiles[dk][:], rstd_bb[:], Alu.mult)
            nc.vector.tensor_tensor(t1[:], t1[:], mur_bb[:], Alu.subtract)
            o = out_pool.tile([P, T], bf16, name="ln_out", tag="ln_out")
            nc.scalar.activation(
                o[:], t1[:], AFT.Identity,
                bias=b_tile[:, dk : dk + 1], scale=g_tile[:, dk : dk + 1],
            )
            outs.append(o)
        return outs

    if x_tiles[0].dtype != bf16:
        conv = []
        for dk in range(8):
            xc = pools["ht"].tile([P, T], bf16, name="xc", tag="xc")
            nc.scalar.activation(xc[:], x_tiles[dk][:], AFT.Copy)
            conv.append(xc)
        x_tiles = conv

    # ---------------- LN1 ----------------
    ln1_stack = ExitStack()
    psp1 = ln1_stack.enter_context(
        tc.tile_pool(name=f"ln_psa{rep}", bufs=1, space="PSUM"))
    tmp1 = ln1_stack.enter_context(tc.tile_pool(name=f"ln_tmpa{rep}", bufs=3))
    ps_sum1 = psp1.tile([1, T], f32, name="ps_sum", tag="ps_sum")
    ps_sq1 = psp1.tile([1, T], f32, name="ps_sq", tag="ps_sq")
    for h in range(2):
        ln_stats_half(x_tiles, psp1, tmp1, h, ps_sum1, ps_sq1)
    h_tiles = ln_finish(x_tiles, c["g1"], c["be1"], pools["ht"], psp1, tmp1,
                        ps_sum1, ps_sq1)
    ln1_stack.close()

    # ---------------- QKV (split by batch half for collective overlap) ------
    # Round A covers wT cols 0:1536 (q slots 0-7, k slots 0-3); round B covers
    # 1536:3072 (k slots 4-7 and all of v).  For each half h (b0 tokens =
    # cols 0:256, b1 = 256:512) all staging lands in a1i rows [h*8*SLOT, ...),
    # then the half's AllToAll fires while the other half computes.
    HT = NCORES * SLOT  # rows per half in a1i/a1o

    def qk_block(h, jts, wtiles, col0):
        """Emit psums + staged write for 4 consecutive q/k outputs, half h."""
        stg = pools["stg"].tile([P, 4 * CH], f8, name="stg", tag="stg")
        for i, jt in enumerate(jts):
            ps = env["qkps"].tile([P, CH], f32, name="qk_ps", tag="qk_ps")
            for dk in range(8):
                nc.tensor.matmul(
                    ps[:],
                    wtiles[dk][:, jt * P - col0 : (jt + 1) * P - col0],
                    h_tiles[dk][:, h * CH : (h + 1) * CH],
                    start=(dk == 0), stop=(dk == 7),
                )
            nc.scalar.activation(
                stg[:, i * CH : (i + 1) * CH], ps[:], AFT.Identity,
                bias=c["bqk"][:, jt : jt + 1],
            )
        jt0 = jts[0]
        base = (h * HT + (jt0 * SLOT if jt0 < 8 else (jt0 - 8) * SLOT + QR)) * CH
        dst = bass.AP(a1i, base, [[CH, P], [SLOT * CH, 4], [1, CH]])
        src = stg[:].rearrange("p (j t) -> p j t", t=CH)
        nc.sync.dma_start(dst, src)

    def emit_v(h, wB, vst_p, v_ps):
        # v: psum [tokens, vdims]; stage 4 slots (x2 heads+ones) per DMA
        for jc in range(2):
            for tt in range(2):
                ps = v_ps.tile([P, 512], f32, name="v_ps", tag="v_ps")
                for dk in range(8):
                    nc.tensor.matmul(
                        ps[:],
                        h_tiles[dk][:, h * CH + tt * P : h * CH + (tt + 1) * P],
                        wB[dk][:, 512 + jc * 512 : 1024 + jc * 512],
                        start=(dk == 0), stop=(dk == 7),
                    )
                vt = vst_p.tile([P, 4 * VR], f8, name="vst", tag="vst")
                for sl in range(4):
                    slot = jc * 4 + sl
                    for lh in range(HL):
                        nc.vector.tensor_tensor(
                            vt[:, sl * VR + lh * 65 : sl * VR + lh * 65 + DH],
                            ps[:, sl * P + lh * DH : sl * P + lh * DH + DH],
                            c["bv"][:, slot * P + lh * DH : slot * P + lh * DH + DH],
                            Alu.add,
                        )
                        nc.vector.memset(
                            vt[:, sl * VR + lh * 65 + DH : sl * VR + lh * 65 + DH + 1],
                            1.0,
                        )
                base = (h * HT + (jc * 4) * SLOT + QR + KR) * CH + tt * P * VR
                dst = bass.AP(a1i, base, [[VR, P], [SLOT * CH, 4], [1, VR]])
                src = vt[:].rearrange("p (s c) -> p s c", c=VR)
                nc.sync.dma_start(dst, src)

    def fire_a2a1(h):
        if collectives:
            nc.gpsimd.collective_compute(
                "AllToAll", mybir.AluOpType.bypass, replica_groups=rg,
                ins=[a1i[h * HT : (h + 1) * HT, :]],
                outs=[a1o[h * HT : (h + 1) * HT, :]],
            )

    with tc.tile_pool(name=f"wqkA{rep}", bufs=8) as wqk_a, tc.tile_pool(
        name=f"wqkB{rep}", bufs=8
    ) as wqk_b, tc.tile_pool(name=f"qk{rep}", bufs=4, space="PSUM") as qk_ps, tc.tile_pool(
        name=f"vst{rep}", bufs=2
    ) as vst_p, tc.tile_pool(name=f"v_ps{rep}", bufs=2, space="PSUM") as v_ps:
        env["qkps"] = qk_ps
        wA, wB = [], []
        for dk in range(8):
            wt = wqk_a.tile([P, 1536], bf16, name="wA", tag="wA")
            nc.sync.dma_start(wt[:], wT[dk * P : (dk + 1) * P, 0:1536])
            wA.append(wt)
        for dk in range(8):
            wt = wqk_b.tile([P, 1536], bf16, name="wB", tag="wB")
            nc.sync.dma_start(wt[:], wT[dk * P : (dk + 1) * P, 1536:3072])
            wB.append(wt)
        for h in range(2):
            for blk in range(3):
                qk_block(h, list(range(blk * 4, blk * 4 + 4)), wA, 0)
            qk_block(h, [12, 13, 14, 15], wB, 1536)
            emit_v(h, wB, vst_p, v_ps)
            fire_a2a1(h)

    # ---------------- attention (batch-outer; overlaps collectives) --------
    # Wide per-core tiles; batch0 chunks at ascending slot order, batch1
    # chunks stored slot-reversed so both batches index by global chunk id.
    a2_all = pools["a2stg"].tile([P, 8 * T], bf16, name="a2all", tag="a2all")
    HT2 = NCORES * QR  # rows per half in a2i/a2o

    ap_stack = ExitStack()
    wp_p = ap_stack.enter_context(tc.tile_pool(name=f"wp{rep}", bufs=8))
    env["wp_pool"] = wp_p
    with tc.tile_pool(name=f"qkv{rep}", bufs=1) as qkv_p, tc.tile_pool(
        name=f"qe{rep}", bufs=4
    ) as qe_p, tc.tile_pool(name=f"s_ps{rep}", bufs=2, space="PSUM") as s_ps, tc.tile_pool(
        name=f"o_ps{rep}", bufs=2, space="PSUM"
    ) as o_ps:
        q_all = qkv_p.tile([P, 4096], f8, name="q_all", tag="q_all")
        k_all = qkv_p.tile([P, 4096], f8, name="k_all", tag="k_all")
        v_all = qkv_p.tile([P, 32 * VR], f8, name="v_all", tag="v_all")

        def load_attn(b):
            for s in range(8):
                pos = s if b == 0 else 7 - s
                for (tile_, rowoff) in ((q_all, 0), (k_all, QR)):
                    src = bass.AP(
                        a1o, (b * HT + s * SLOT + rowoff) * CH, [[CH, P], [1, CH]]
                    )
                    nc.sync.dma_start(
                        tile_[:, b * 2048 + pos * CH : b * 2048 + (pos + 1) * CH], src
                    )
                vbase = (b * HT + s * SLOT + QR + KR) * CH
                src = bass.AP(a1o, vbase, [[VR, P], [P * VR, 2], [1, VR]])
                blk0 = (b * 16 + pos * 2) * VR
                nc.sync.dma_start(v_all[:, blk0 : blk0 + 2 * VR], src)

        load_attn(0)
        load_attn(1)  # waits on A2A1(1) via deps; overlaps b0 compute
        # prefetch proj weights during attention
        wpt = []
        for dk in range(8):
            wt = env["wp_pool"].tile([P, D], bf16, name="wp", tag="wp")
            nc.sync.dma_start(wt[:], wpT[dk * P : (dk + 1) * P, :])
            wpt.append(wt)
        env["wpt"] = wpt

        for b in range(B):
            for lh in range(HL):
                for pr in range(4):  # query-chunk pairs (2pr, 2pr+1)
                    q0, q1 = 2 * pr, 2 * pr + 1
                    s0 = q0 if b == 0 else 7 - q0
                    s1 = q1 if b == 0 else 7 - q1
                    qcol0 = b * CH
                    qmv = q_all[lh * DH : (lh + 1) * DH,
                                b * 2048 + q0 * CH : b * 2048 + (q1 + 1) * CH]
                    po = o_ps.tile([65, 2 * CH], f32, name="o_ps", tag="o_ps")
                    n_mm = 2 * (q1 + 1)
                    mi = 0
                    for kc in range(q1 + 1):
                        # kc == q1 blocks only contribute to the q1 (right)
                        # query half; the q0 half is fully above-diagonal.
                        narrow = kc == q1
                        qs = qmv[:, CH : 2 * CH] if narrow else qmv
                        W = CH if narrow else 2 * CH
                        # both k sub-chunks land in one 2-bank psum tile so a
                        # single Exp covers them
                        ps = s_ps.tile([P, 4 * CH], f32, name="s_ps", tag="s_ps")
                        E = qe_p.tile([P, 4 * CH], bf16, name="E", tag="E")
                        for sub in range(2):
                            nc.tensor.matmul(
                                ps[:, sub * W : (sub + 1) * W],
                                k_all[lh * DH : (lh + 1) * DH,
                                      b * 2048 + kc * CH + sub * P
                                      : b * 2048 + kc * CH + (sub + 1) * P],
                                qs,
                                start=True, stop=True,
                                skip_group_check=True,
                            )
                        nc.scalar.activation(
                            E[:, 0 : 2 * W], ps[:, 0 : 2 * W], AFT.Exp, scale=0.125
                        )
                        for sub in range(2):
                            Es = E[:, sub * W : (sub + 1) * W]
                            if kc == q0 and not narrow:  # diagonal for q0
                                nc.vector.tensor_tensor(
                                    Es, Es, c["tri_lo"][sub][:], Alu.mult
                                )
                            elif narrow:  # diagonal for q1
                                nc.vector.tensor_tensor(
                                    Es, Es, c["tri_hi"][sub][:, CH : 2 * CH],
                                    Alu.mult,
                                )
                            vblk = (b * 16 + kc * 2 + sub) * VR + lh * 65
                            nc.tensor.matmul(
                                po[:, 2 * CH - W : 2 * CH],
                                v_all[:, vblk : vblk + 65], Es,
                                start=(mi == 0), stop=(mi == n_mm - 1),
                                skip_group_check=True,
                            )
                            mi += 1
                    rec = recp.tile([1, 2 * CH], bf16, name="rec", tag="rec")
                    with nc.allow_low_precision(reason="softmax denom bcast"):
                        nc.vector.reciprocal(rec[:], po[64:65, :])
                    rec_ps = s_ps.tile(
                        [DH, 2 * CH], f32, name="rec_ps", tag="rec_ps", bufs=2
                    )
                    nc.tensor.matmul(
                        rec_ps[:], c["ones_row"][:, 0:DH], rec[:], start=True, stop=True
                    )
                    rec_b = recp.tile([DH, 2 * CH], f32, name="rec_b", tag="rec_b")
                    nc.vector.tensor_copy(rec_b[:], rec_ps[:])
                    for half, sq in ((0, s0), (1, s1)):
                        nc.vector.tensor_tensor(
                            a2_all[lh * DH : (lh + 1) * DH,
                                   sq * T + qcol0 : sq * T + qcol0 + CH],
                            po[0:DH, half * CH : (half + 1) * CH],
                            rec_b[:, half * CH : (half + 1) * CH],
                            Alu.mult,
                        )
            # half-b attention done: ship its outputs + fire its AllToAll
            dst = bass.AP(a2i, b * HT2 * CH, [[CH, P], [P * CH, 8], [1, CH]])
            src = bass.AP(
                a2_all[:].tensor, a2_all[:].offset + b * CH,
                [[a2_all[:].ap[0][0], P], [T, 8], [1, CH]],
            )
            nc.sync.dma_start(dst, src)
            if collectives:
                nc.gpsimd.collective_compute(
                    "AllToAll", mybir.AluOpType.bypass, replica_groups=rg,
                    ins=[a2i[b * HT2 : (b + 1) * HT2, :]],
                    outs=[a2o[b * HT2 : (b + 1) * HT2, :]],
                )

    # ---------------- proj + residual1 (split by half) ----------------
    # LN2 stats for each half are emitted right after that half's residual,
    # so they overlap the other half's AllToAll/proj.
    ln2_stack = ExitStack()
    psp2 = ln2_stack.enter_context(
        tc.tile_pool(name=f"ln_psb{rep}", bufs=1, space="PSUM"))
    tmp2 = ln2_stack.enter_context(tc.tile_pool(name=f"ln_tmpb{rep}", bufs=3))
    ps_sum2 = psp2.tile([1, T], f32, name="ps_sum", tag="ps_sum")
    ps_sq2 = psp2.tile([1, T], f32, name="ps_sq", tag="ps_sq")

    x1_tiles = []
    with tc.tile_pool(
        name=f"p_ps{rep}", bufs=4, space="PSUM"
    ) as p_ps, tc.tile_pool(name=f"otp{rep}", bufs=1) as ot_p:
        wpt = env["wpt"]
        ot_all = ot_p.tile([P, 8 * T], bf16, name="ot_all", tag="ot_all")
        for h in range(2):
            dst = bass.AP(
                ot_all[:].tensor, ot_all[:].offset + h * CH,
                [[ot_all[:].ap[0][0], P], [T, 8], [1, CH]],
            )
            nc.sync.dma_start(
                dst, bass.AP(a2o, h * HT2 * CH, [[CH, P], [P * CH, 8], [1, CH]])
            )
            for do in range(8):
                ps = p_ps.tile([P, CH], f32, name="p_ps", tag="p_ps")
                for dk in range(8):
                    nc.tensor.matmul(
                        ps[:],
                        wpt[dk][:, do * P : (do + 1) * P],
                        ot_all[:, dk * T + h * CH : dk * T + (h + 1) * CH],
                        start=(dk == 0), stop=(dk == 7),
                    )
                if h == 0:
                    x1 = pools["x1"].tile([P, T], bf16, name="x1", tag="x1")
                    x1_tiles.append(x1)
                x1 = x1_tiles[do]
                nc.vector.scalar_tensor_tensor(
                    x1[:, h * CH : (h + 1) * CH], ps[:], c["bp"][:, do : do + 1],
                    x_tiles[do][:, h * CH : (h + 1) * CH], Alu.add, Alu.add,
                )
            ln_stats_half(x1_tiles, psp2, tmp2, h, ps_sum2, ps_sq2)

    # ---------------- LN2 finish ----------------
    h2_tiles = ln_finish(x1_tiles, c["g2"], c["be2"], pools["h2"], psp2, tmp2,
                         ps_sum2, ps_sq2)
    ln2_stack.close()
    ap_stack.close()

    # prefetch MLP-up half-0 weights; overlaps the LN2 apply tail
    wu_stack = ExitStack()
    wu_p = wu_stack.enter_context(tc.tile_pool(name=f"wu{rep}", bufs=9))
    wu_pre = []
    for dk in range(8):
        wt = wu_p.tile([P, 2048], bf16, name="wu", tag="wu")
        nc.sync.dma_start(wt[:], wuT[dk * P : (dk + 1) * P, 0:2048])
        wu_pre.append(wt)

    # ---------------- MLP up + gelu ----------------
    gu_tiles = []
    with tc.tile_pool(name=f"u_ps{rep}", bufs=3, space="PSUM") as u_ps:
        for half in range(2):
            if half == 0:
                wut = wu_pre
            else:
                wut = []
                for dk in range(8):
                    wt = wu_p.tile([P, 2048], bf16, name="wu", tag="wu")
                    nc.sync.dma_start(
                        wt[:],
                        wuT[dk * P : (dk + 1) * P, half * 2048 : (half + 1) * 2048],
                    )
                    wut.append(wt)
            for jl in range(16):
                j = half * 16 + jl
                ps = u_ps.tile([P, T], f32, name="u_ps", tag="u_ps")
                for dk in range(8):
                    nc.tensor.matmul(
                        ps[:], wut[dk][:, jl * P : (jl + 1) * P], h2_tiles[dk][:],
                        start=(dk == 0), stop=(dk == 7),
                    )
                gu = pools["gu"].tile([P, T], bf16, name="gu", tag="gu")
                nc.scalar.activation(
                    gu[:], ps[:], AFT.Gelu_apprx_tanh, bias=c["bu"][:, j : j + 1]
                )
                gu_tiles.append(gu)
    wu_stack.close()

    # ---------------- MLP down + residual2 ----------------
    out_tiles = []
    with tc.tile_pool(name=f"wd{rep}", bufs=6) as wd_p, tc.tile_pool(
        name=f"d_ps{rep}", bufs=1, space="PSUM"
    ) as d_ps:
        pss = [d_ps.tile([P, T], f32, name=f"d_ps{do}", tag=f"d_ps{do}")
               for do in range(8)]
        for j in range(32):
            wt = wd_p.tile([P, D], bf16, name="wd", tag="wd")
            nc.sync.dma_start(wt[:], wdT[j * P : (j + 1) * P, :])
            for do in range(8):
                nc.tensor.matmul(
                    pss[do][:], wt[:, do * P : (do + 1) * P], gu_tiles[j][:],
                    start=(j == 0), stop=(j == 31),
                )
        for do in range(8):
            o = pools["outp"].tile([P, T], f32, name="out_t", tag="out_t")
            nc.vector.scalar_tensor_tensor(
                o[:], pss[do][:], c["bd"][:, do : do + 1], x1_tiles[do][:],
                Alu.add, Alu.add,
            )
            if write_out:
                nc.sync.dma_start(out[do * P : (do + 1) * P, :], o[:])
            out_tiles.append(o)
    return out_tiles


def _build(nreps=1, collectives=True, hw_loop=0):
    from contextlib import ExitStack
    from concourse import bass, mybir, tile, bacc

    f32 = mybir.dt.float32
    bf16 = mybir.dt.bfloat16

    nc = bacc.Bacc("TRN2", target_bir_lowering=False, num_devices=NCORES)

    xT = nc.declare_dram_parameter("xT", [D, T], bf16, isOutput=False)
    wT = nc.declare_dram_parameter("wT", [D, 3 * D], bf16, isOutput=False)
    wpT = nc.declare_dram_parameter("wpT", [D, D], bf16, isOutput=False)
    wuT = nc.declare_dram_parameter("wuT", [D, DFF], bf16, isOutput=False)
    wdT = nc.declare_dram_parameter("wdT", [DFF, D], bf16, isOutput=False)
    bqk = nc.declare_dram_parameter("bqk", [P, 16], f32, isOutput=False)
    bv = nc.declare_dram_parameter("bv", [P, D], f32, isOutput=False)
    bp = nc.declare_dram_parameter("bp", [P, 8], f32, isOutput=False)
    bu = nc.declare_dram_parameter("bu", [P, 32], f32, isOutput=False)
    bd = nc.declare_dram_parameter("bd", [P, 8], f32, isOutput=False)
    g1 = nc.declare_dram_parameter("g1", [P, 8], f32, isOutput=False)
    be1 = nc.declare_dram_parameter("be1", [P, 8], f32, isOutput=False)
    g2 = nc.declare_dram_parameter("g2", [P, 8], f32, isOutput=False)
    be2 = nc.declare_dram_parameter("be2", [P, 8], f32, isOutput=False)
    tri = nc.declare_dram_parameter("tri", [CH, CH], bf16, isOutput=False)
    tri2 = nc.declare_dram_parameter("tri2", [CH, 4 * CH], bf16, isOutput=False)
    out = nc.declare_dram_parameter("out", [D, T], f32, isOutput=True)

    # Half-split bounce buffers: rows [h*8*SLOT, (h+1)*8*SLOT) hold batch-half
    # h (256 token cols) so each half's AllToAll is a contiguous slab.
    f8 = mybir.dt.float8e4
    a1i = nc.dram_tensor("a2a1_in", [2 * NCORES * SLOT, CH], f8)
    a1o = nc.dram_tensor("a2a1_out", [2 * NCORES * SLOT, CH], f8)
    a2i = nc.dram_tensor("a2a2_in", [2 * NCORES * QR, CH], bf16)
    a2o = nc.dram_tensor("a2a2_out", [2 * NCORES * QR, CH], bf16)

    with tile.TileContext(nc) as tc, ExitStack() as top:
        xt_pool = top.enter_context(tc.tile_pool(name="xt", bufs=8))
        x_tiles = []
        for dk in range(8):
            xt = xt_pool.tile([P, T], bf16, name="xt", tag="xt")
            nc.sync.dma_start(xt[:], xT[dk * P : (dk + 1) * P, :])
            x_tiles.append(xt)
        const = top.enter_context(tc.tile_pool(name="const", bufs=1))
        ones = const.tile([P, 1], bf16)
        nc.vector.memset(ones[:], 1.0)
        ones_f = const.tile([P, 1], f32)
        nc.vector.memset(ones_f[:], 1.0)
        ones_row = const.tile([1, P], bf16)
        nc.vector.memset(ones_row[:], 1.0)
        tri_t = [const.tile([P, CH], bf16, name=f"tri{s}", tag=f"tri{s}") for s in range(2)]
        for s in range(2):
            nc.sync.dma_start(tri_t[s][:], tri[s * P : (s + 1) * P, :])
        tri2_t = [const.tile([P, 4 * CH], bf16, name=f"tri2{s}", tag=f"tri2{s}") for s in range(2)]
        for s in range(2):
            nc.sync.dma_start(tri2_t[s][:], tri2[s * P : (s + 1) * P, :])

        def ctile(name, param, shape):
            t = const.tile(shape, f32, name=name, tag=name)
            nc.sync.dma_start(t[:], param[:, :])
            return t

        consts = {
            "ones": ones, "ones_f": ones_f, "ones_row": ones_row, "tri": tri_t,
            "tri_lo": [tri2_t[s][:, 0 : 2 * CH] for s in range(2)],
            "tri_hi": [tri2_t[s][:, 2 * CH : 4 * CH] for s in range(2)],
            "bqk": ctile("bqk_t", bqk, [P, 16]),
            "bv": ctile("bv_t", bv, [P, D]),
            "bp": ctile("bp_t", bp, [P, 8]),
            "bu": ctile("bu_t", bu, [P, 32]),
            "bd": ctile("bd_t", bd, [P, 8]),
            "g1": ctile("g1_t", g1, [P, 8]),
            "be1": ctile("be1_t", be1, [P, 8]),
            "g2": ctile("g2_t", g2, [P, 8]),
            "be2": ctile("be2_t", be2, [P, 8]),
        }

        pools = {
            "vec": top.enter_context(tc.tile_pool(name="vec", bufs=1)),
            "recp": top.enter_context(tc.tile_pool(name="recp", bufs=2)),
            "ht": top.enter_context(tc.tile_pool(name="ht", bufs=8)),
            "stg": top.enter_context(tc.tile_pool(name="stg", bufs=2)),
            "a2stg": top.enter_context(tc.tile_pool(name="a2stg", bufs=1)),
            "ot": top.enter_context(tc.tile_pool(name="ot", bufs=1)),
            "x1": top.enter_context(tc.tile_pool(name="x1", bufs=8)),
            "h2": top.enter_context(tc.tile_pool(name="h2", bufs=8)),
            "gu": top.enter_context(tc.tile_pool(name="gu", bufs=32)),
            "outp": top.enter_context(tc.tile_pool(name="outp", bufs=8)),
        }

        env = {
            "params": (xT, wT, wpT, wuT, wdT, out),
            "bounce": (a1i, a1o, a2i, a2o),
            "consts": consts,
            "pools": pools,
        }

        if hw_loop:
            with tc.For_i(0, hw_loop):
                _emit_block(nc, tc, env, 0, x_tiles, None, collectives, write_out=True)
        else:
            cur = x_tiles
            for rep in range(nreps):
                cur = _emit_block(
                    nc, tc, env, rep, cur, None, collectives,
                    write_out=(rep == nreps - 1),
                )

    nc.finalize()
    return nc


def _get_nc():
    if "nc" not in _CACHE:
        _CACHE["nc"] = _build()
    return _CACHE["nc"]


def _make_in_maps(inputs):
    x = np.asarray(inputs["x"], np.float32)
    ln1_g = np.asarray(inputs["ln1_g"], np.float32)
    ln1_b = np.asarray(inputs["ln1_b"], np.float32)
    W_attn = np.asarray(inputs["W_attn"], np.float32)
    b_attn = np.asarray(inputs["b_attn"], np.float32)
    W_proj = np.asarray(inputs["W_proj"], np.float32)
    b_proj = np.asarray(inputs["b_proj"], np.float32)
    ln2_g = np.asarray(inputs["ln2_g"], np.float32)
    ln2_b = np.asarray(inputs["ln2_b"], np.float32)
    W_up = np.asarray(inputs["W_up"], np.float32)
    b_up = np.asarray(inputs["b_up"], np.float32)
    W_down = np.asarray(inputs["W_down"], np.float32)
    b_down = np.asarray(inputs["b_down"], np.float32)

    bf = ml_dtypes.bfloat16
    wT = np.ascontiguousarray(W_attn.T).astype(bf)
    wpT = np.ascontiguousarray(W_proj.T).astype(bf)
    wuT = np.ascontiguousarray(W_up.T).astype(bf)
    wdT = np.ascontiguousarray(W_down.T).astype(bf)

    def cols(v):  # [N] -> [128, N//128]: col j = v[j*128:(j+1)*128]
        return np.ascontiguousarray(v.reshape(-1, P).T).astype(np.float32)

    tri = np.tril(np.ones((CH, CH), np.float32)).T.astype(bf)  # tri[a,b] = a<=b
    tri = np.ascontiguousarray(tri)

    ones_m = np.ones((CH, CH), np.float32)
    zeros_m = np.zeros((CH, CH), np.float32)
    tri_f = np.tril(np.ones((CH, CH), np.float32)).T
    tri2 = np.ascontiguousarray(
        np.concatenate([tri_f, ones_m, zeros_m, tri_f], axis=1)
    ).astype(bf)

    common = dict(
        wT=wT, wpT=wpT, wuT=wuT, wdT=wdT, tri2=tri2,
        bqk=cols(b_attn[: 2 * D]),
        bv=np.ascontiguousarray(np.broadcast_to(b_attn[2 * D :].reshape(1, D), (P, D))),
        bp=cols(b_proj), bu=cols(b_up), bd=cols(b_down),
        g1=cols(ln1_g), be1=cols(ln1_b), g2=cols(ln2_g), be2=cols(ln2_b),
        tri=tri,
    )

    in_maps = []
    for i in range(NCORES):
        c0 = x[0, i * CH : (i + 1) * CH]  # [256, 1024]
        c1 = x[1, (7 - i) * CH : (8 - i) * CH]
        xTi = np.ascontiguousarray(np.concatenate([c0, c1], 0).T).astype(bf)
        in_maps.append(dict(common, xT=xTi))
    return in_maps


def make_in_maps(inputs):
    return _make_in_maps(inputs)


def kernel(**inputs):
    in_maps = _make_in_maps(inputs)

    from concourse import bass_utils

    nc = _get_nc()
    res = bass_utils.run_bass_kernel_spmd(
        nc, in_maps, core_ids=list(range(NCORES)), trace=TRACE
    )
    _CACHE["last_res"] = res
    y = np.empty((B, S, D), np.float32)
    for i in range(NCORES):
        o = np.asarray(res.results[i]["out"], np.float32)  # [1024, 512]
        y[0, i * CH : (i + 1) * CH] = o[:, :CH].T
        y[1, (7 - i) * CH : (8 - i) * CH] = o[:, CH:].T
    return y


# revision 37
# speedup vs baseline: 183.1347x; 1.0287x over previous
"""Trainium2 distributed kernel for a dense transformer block (8 NeuronCores).

Sharding: tokens are data-parallel for LN/QKV/proj/MLP (512 tokens/core,
causal-balanced pairing: core i owns batch0 chunk i and batch1 chunk 7-i),
attention is head-parallel (2 heads/core) via an AllToAll exchange of
Q/K/V, plus a second AllToAll to bring attention outputs back to token
sharding.  All matmuls run in bf16 (f32 accumulation in PSUM); LayerNorm
statistics are computed with ones-vector matmuls so every activation
stays in transposed [d, token] layout on chip.

DMA strategy: weight/activation transfers are batched into wide tiles
(multi-block access patterns) to minimize HWDGE descriptor-queue
serialization; attention Q/K/V live in single wide SBUF tiles sliced
per-head/chunk, with batch1 slot order reversed so both batches index
chunks in ascending global order.
"""

import sys

sys.path.insert(0, "/opt/trn_rl_repo")

import numpy as np
import ml_dtypes

NCORES = 8
D = 1024
H = 16
DH = 64
HL = H // NCORES  # heads per core = 2
B = 2
S = 2048
T = 512  # tokens per core
CH = 256  # token chunk (half of T)
DFF = 4096
P = 128
QR, KR, VR = 128, 128, 130  # slot row counts: qT, kT, packed-v regions
SLOT = QR + KR + VR  # 386
EPS = 1e-5

_CACHE = {}
TRACE = False


def _emit_block(nc, tc, env, rep, x_tiles, x_all, collectives, write_out):
    """Emit one transformer block; returns the 8 output [128,T] f32 tiles."""
    from contextlib import ExitStack
    from concourse import bass, mybir

    f32 = mybir.dt.float32
    bf16 = mybir.dt.bfloat16
    f8 = mybir.dt.float8e4
    Alu = mybir.AluOpType
    AFT = mybir.ActivationFunctionType

    (xT, wT, wpT, wuT, wdT, out) = env["params"]
    (a1i, a1o, a2i, a2o) = env["bounce"]
    if not collectives:
        a1o, a2o = a1i, a2i
    c = env["consts"]
    pools = env["pools"]
    vec, recp = pools["vec"], pools["recp"]
    rg = [list(range(NCORES))]

    def ap3(t, part0, nprt, off, dims):
        """Custom free-dim AP on tile t at partition slice [part0, part0+nprt)."""
        base = t[:]
        pstride = base.ap[0][0]
        return bass.AP(base.tensor, base.offset + part0 * pstride + off,
                      [[pstride, nprt]] + dims)

    def ln_stats_half(x_tiles, psp, tmp_p, h, ps_sum, ps_sq):
        c0, c1 = h * CH, (h + 1) * CH
        for dk in range(8):
            nc.tensor.matmul(
                ps_sum[:, c0:c1], c["ones"][:], x_tiles[dk][:, c0:c1],
                start=(dk == 0), stop=(dk == 7),
            )
            sq = tmp_p.tile([P, CH], bf16, name="sq", tag="sq")
            nc.scalar.activation(sq[:], x_tiles[dk][:, c0:c1], AFT.Square)
            nc.tensor.matmul(
                ps_sq[:, c0:c1], c["ones"][:], sq[:],
                start=(dk == 0), stop=(dk == 7),
            )

    def ln_chain_half(st, h):
        """Per-half stats -> rstd/mur broadcast tiles (sbuf bf16)."""
        c0, c1 = h * CH, (h + 1) * CH
        ch, chb, psp, tmp_p, ps_sum, ps_sq = (
            st["ch"], st["chb"], st["psp"], st["tmp"], st["sum"], st["sq"])
        mu, msq, mu2, var, std, rstd, mur = (
            ch[0:1, i * T + c0 : i * T + c1] for i in range(7))
        rstd_c, mur_c = chb[0:1, c0:c1], chb[0:1, T + c0 : T + c1]
        nc.vector.tensor_scalar(mu, ps_sum[:, c0:c1], 1.0 / D, None, Alu.mult)
        nc.vector.tensor_scalar(msq, ps_sq[:, c0:c1], 1.0 / D, None, Alu.mult)
        nc.vector.tensor_tensor(mu2, mu, mu, Alu.mult)
        nc.vector.tensor_tensor(var, msq, mu2, Alu.subtract)
        nc.vector.tensor_scalar(var, var, EPS, None, Alu.add)
        nc.scalar.activation(std, var, AFT.Sqrt)
        nc.vector.reciprocal(rstd, std)
        nc.vector.tensor_tensor(mur, mu, rstd, Alu.mult)
        nc.vector.tensor_copy(rstd_c, rstd)
        nc.vector.tensor_copy(mur_c, mur)
        rstd_b = psp.tile([P, CH], f32, name="rstd_b", tag="rstd_b", bufs=1)
        nc.tensor.matmul(rstd_b[:], c["ones_row"][:], rstd_c, start=True, stop=True)
        mur_b = psp.tile([P, CH], f32, name="mur_b", tag="mur_b", bufs=1)
        nc.tensor.matmul(mur_b[:], c["ones_row"][:], mur_c, start=True, stop=True)
        rstd_bb = tmp_p.tile([P, CH], bf16, name="rstd_bb", tag="rstd_bb", bufs=2)
        nc.scalar.activation(rstd_bb[:], rstd_b[:], AFT.Copy)
        mur_bb = tmp_p.tile([P, CH], bf16, name="mur_bb", tag="mur_bb", bufs=2)
        nc.scalar.activation(mur_bb[:], mur_b[:], AFT.Copy)
        st[("bc", h)] = (rstd_bb, mur_bb)

    def ln_apply_half(st, h, x_tiles, g_tile, b_tile, out_pool, outs):
        c0, c1 = h * CH, (h + 1) * CH
        rstd_bb, mur_bb = st[("bc", h)]
        for dk in range(8):
            t1 = st["tmp"].tile([P, CH], bf16, name="lnt1", tag="lnt1")
            nc.vector.tensor_tensor(
                t1[:], x_tiles[dk][:, c0:c1], rstd_bb[:], Alu.mult)
            nc.vector.tensor_tensor(t1[:], t1[:], mur_bb[:], Alu.subtract)
            if h == 0:
                outs.append(out_pool.tile([P, T], bf16, name="ln_out",
                                          tag="ln_out"))
            nc.scalar.activation(
                outs[dk][:, c0:c1], t1[:], AFT.Identity,
                bias=b_tile[:, dk : dk + 1], scale=g_tile[:, dk : dk + 1],
            )
        return outs

    if x_tiles[0].dtype != bf16:
        conv = []
        for dk in range(8):
            xc = pools["ht"].tile([P, T], bf16, name="xc", tag="xc")
            nc.scalar.activation(xc[:], x_tiles[dk][:], AFT.Copy)
            conv.append(xc)
        x_tiles = conv

    # ---------------- LN1 (per-half so QKV(h0) starts after apply(h0)) ----
    ln1_stack = ExitStack()
    psp1 = ln1_stack.enter_context(
        tc.tile_pool(name=f"ln_psa{rep}", bufs=1, space="PSUM"))
    tmp1 = ln1_stack.enter_context(tc.tile_pool(name=f"ln_tmpa{rep}", bufs=3))
    st1 = {
        "psp": psp1, "tmp": tmp1,
        "sum": psp1.tile([1, T], f32, name="ps_sum", tag="ps_sum"),
        "sq": psp1.tile([1, T], f32, name="ps_sq", tag="ps_sq"),
        "ch": vec.tile([1, 8 * T], f32, name="lnchain", tag="lnchain"),
        "chb": vec.tile([1, 2 * T], bf16, name="lnchainb", tag="lnchainb"),
    }
    for h in range(2):
        ln_stats_half(x_tiles, psp1, tmp1, h, st1["sum"], st1["sq"])
        ln_chain_half(st1, h)
    h_tiles = []
    for h in range(2):
        ln_apply_half(st1, h, x_tiles, c["g1"], c["be1"], pools["ht"], h_tiles)
    ln1_stack.close()

    # ---------------- QKV (split by batch half for collective overlap) ------
    # Round A covers wT cols 0:1536 (q slots 0-7, k slots 0-3); round B covers
    # 1536:3072 (k slots 4-7 and all of v).  For each half h (b0 tokens =
    # cols 0:256, b1 = 256:512) all staging lands in a1i rows [h*8*SLOT, ...),
    # then the half's AllToAll fires while the other half computes.
    HT = NCORES * SLOT  # rows per half in a1i/a1o

    def qk_block(h, jts, wtiles, col0):
        """Emit psums + staged write for 4 consecutive q/k outputs, half h."""
        stg = pools["stg"].tile([P, 4 * CH], f8, name="stg", tag="stg")
        for i, jt in enumerate(jts):
            ps = env["qkps"].tile([P, CH], f32, name="qk_ps", tag="qk_ps")
            for dk in range(8):
                nc.tensor.matmul(
                    ps[:],
                    wtiles[dk][:, jt * P - col0 : (jt + 1) * P - col0],
                    h_tiles[dk][:, h * CH : (h + 1) * CH],
                    start=(dk == 0), stop=(dk == 7),
                )
            nc.scalar.activation(
                stg[:, i * CH : (i + 1) * CH], ps[:], AFT.Identity,
                bias=c["bqk"][:, jt : jt + 1],
            )
        jt0 = jts[0]
        base = (h * HT + (jt0 * SLOT if jt0 < 8 else (jt0 - 8) * SLOT + QR)) * CH
        dst = bass.AP(a1i, base, [[CH, P], [SLOT * CH, 4], [1, CH]])
        src = stg[:].rearrange("p (j t) -> p j t", t=CH)
        nc.sync.dma_start(dst, src)

    def emit_v(h, wB, vst_p, v_ps):
        # v: psum [tokens, vdims]; stage 4 slots (x2 heads+ones) per DMA
        for jc in range(2):
            for tt in range(2):
                ps = v_ps.tile([P, 512], f32, name="v_ps", tag="v_ps")
                for dk in range(8):
                    nc.tensor.matmul(
                        ps[:],
                        h_tiles[dk][:, h * CH + tt * P : h * CH + (tt + 1) * P],
                        wB[dk][:, 512 + jc * 512 : 1024 + jc * 512],
                        start=(dk == 0), stop=(dk == 7),
                    )
                vt = vst_p.tile([P, 4 * VR], f8, name="vst", tag="vst")
                for sl in range(4):
                    slot = jc * 4 + sl
                    for lh in range(HL):
                        nc.vector.tensor_tensor(
                            vt[:, sl * VR + lh * 65 : sl * VR + lh * 65 + DH],
                            ps[:, sl * P + lh * DH : sl * P + lh * DH + DH],
                            c["bv"][:, slot * P + lh * DH : slot * P + lh * DH + DH],
                            Alu.add,
                        )
                        nc.vector.memset(
                            vt[:, sl * VR + lh * 65 + DH : sl * VR + lh * 65 + DH + 1],
                            1.0,
                        )
                base = (h * HT + (jc * 4) * SLOT + QR + KR) * CH + tt * P * VR
                dst = bass.AP(a1i, base, [[VR, P], [SLOT * CH, 4], [1, VR]])
                src = vt[:].rearrange("p (s c) -> p s c", c=VR)
                nc.sync.dma_start(dst, src)

    def fire_a2a1(h):
        if collectives:
            nc.gpsimd.collective_compute(
                "AllToAll", mybir.AluOpType.bypass, replica_groups=rg,
                ins=[a1i[h * HT : (h + 1) * HT, :]],
                outs=[a1o[h * HT : (h + 1) * HT, :]],
            )

    with tc.tile_pool(name=f"wqkA{rep}", bufs=8) as wqk_a, tc.tile_pool(
        name=f"wqkB{rep}", bufs=8
    ) as wqk_b, tc.tile_pool(name=f"qk{rep}", bufs=4, space="PSUM") as qk_ps, tc.tile_pool(
        name=f"vst{rep}", bufs=2
    ) as vst_p, tc.tile_pool(name=f"v_ps{rep}", bufs=2, space="PSUM") as v_ps:
        env["qkps"] = qk_ps
        wA, wB = [], []
        for dk in range(8):
            wt = wqk_a.tile([P, 1536], bf16, name="wA", tag="wA")
            nc.sync.dma_start(wt[:], wT[dk * P : (dk + 1) * P, 0:1536])
            wA.append(wt)
        for dk in range(8):
            wt = wqk_b.tile([P, 1536], bf16, name="wB", tag="wB")
            nc.sync.dma_start(wt[:], wT[dk * P : (dk + 1) * P, 1536:3072])
            wB.append(wt)
        for h in range(2):
            for blk in range(3):
                qk_block(h, list(range(blk * 4, blk * 4 + 4)), wA, 0)
            qk_block(h, [12, 13, 14, 15], wB, 1536)
            emit_v(h, wB, vst_p, v_ps)
            fire_a2a1(h)

    # ---------------- attention (batch-outer; overlaps collectives) --------
    # Wide per-core tiles; batch0 chunks at ascending slot order, batch1
    # chunks stored slot-reversed so both batches index by global chunk id.
    a2_all = pools["a2stg"].tile([P, 8 * T], bf16, name="a2all", tag="a2all")
    HT2 = NCORES * QR  # rows per half in a2i/a2o

    ap_stack = ExitStack()
    wp_p = ap_stack.enter_context(tc.tile_pool(name=f"wp{rep}", bufs=8))
    env["wp_pool"] = wp_p
    with tc.tile_pool(name=f"qkv{rep}", bufs=1) as qkv_p, tc.tile_pool(
        name=f"qe{rep}", bufs=4
    ) as qe_p, tc.tile_pool(name=f"s_ps{rep}", bufs=2, space="PSUM") as s_ps, tc.tile_pool(
        name=f"o_ps{rep}", bufs=2, space="PSUM"
    ) as o_ps:
        q_all = qkv_p.tile([P, 4096], f8, name="q_all", tag="q_all")
        k_all = qkv_p.tile([P, 4096], f8, name="k_all", tag="k_all")
        v_all = qkv_p.tile([P, 32 * VR], f8, name="v_all", tag="v_all")

        def load_attn(b):
            # load in the order attention consumes chunks: for b1 the chunk
            # index is 7-s, so iterate slots descending
            order = range(8) if b == 0 else reversed(range(8))
            for s in order:
                pos = s if b == 0 else 7 - s
                for (tile_, rowoff) in ((q_all, 0), (k_all, QR)):
                    src = bass.AP(
                        a1o, (b * HT + s * SLOT + rowoff) * CH, [[CH, P], [1, CH]]
                    )
                    nc.sync.dma_start(
                        tile_[:, b * 2048 + pos * CH : b * 2048 + (pos + 1) * CH], src
                    )
                vbase = (b * HT + s * SLOT + QR + KR) * CH
                src = bass.AP(a1o, vbase, [[VR, P], [P * VR, 2], [1, VR]])
                blk0 = (b * 16 + pos * 2) * VR
                nc.sync.dma_start(v_all[:, blk0 : blk0 + 2 * VR], src)

        load_attn(0)
        load_attn(1)  # waits on A2A1(1) via deps; overlaps b0 compute
        # prefetch proj weights during attention
        wpt = []
        for dk in range(8):
            wt = env["wp_pool"].tile([P, D], bf16, name="wp", tag="wp")
            nc.sync.dma_start(wt[:], wpT[dk * P : (dk + 1) * P, :])
            wpt.append(wt)
        env["wpt"] = wpt

        for b in range(B):
            for lh in range(HL):
                for pr in range(4):  # query-chunk pairs (2pr, 2pr+1)
                    q0, q1 = 2 * pr, 2 * pr + 1
                    s0 = q0 if b == 0 else 7 - q0
                    s1 = q1 if b == 0 else 7 - q1
                    qcol0 = b * CH
                    qmv = q_all[lh * DH : (lh + 1) * DH,
                                b * 2048 + q0 * CH : b * 2048 + (q1 + 1) * CH]
                    po = o_ps.tile([65, 2 * CH], f32, name="o_ps", tag="o_ps")
                    n_mm = 2 * (q1 + 1)
                    mi = 0
                    for kc in range(q1 + 1):
                        # kc == q1 blocks only contribute to the q1 (right)
                        # query half; the q0 half is fully above-diagonal.
                        narrow = kc == q1
                        qs = qmv[:, CH : 2 * CH] if narrow else qmv
                        W = CH if narrow else 2 * CH
                        # both k sub-chunks land in one 2-bank psum tile so a
                        # single Exp covers them
                        ps = s_ps.tile([P, 4 * CH], f32, name="s_ps", tag="s_ps")
                        E = qe_p.tile([P, 4 * CH], bf16, name="E", tag="E")
                        for sub in range(2):
                            nc.tensor.matmul(
                                ps[:, sub * W : (sub + 1) * W],
                                k_all[lh * DH : (lh + 1) * DH,
                                      b * 2048 + kc * CH + sub * P
                                      : b * 2048 + kc * CH + (sub + 1) * P],
                                qs,
                                start=True, stop=True,
                                skip_group_check=True,
                            )
                        nc.scalar.activation(
                            E[:, 0 : 2 * W], ps[:, 0 : 2 * W], AFT.Exp, scale=0.125
                        )
                        for sub in range(2):
                            Es = E[:, sub * W : (sub + 1) * W]
                            if kc == q0 and not narrow:  # diagonal for q0
                                nc.vector.tensor_tensor(
                                    Es, Es, c["tri_lo"][sub][:], Alu.mult
                                )
                            elif narrow:  # diagonal for q1
                                nc.vector.tensor_tensor(
                                    Es, Es, c["tri_hi"][sub][:, CH : 2 * CH],
                                    Alu.mult,
                                )
                            vblk = (b * 16 + kc * 2 + sub) * VR + lh * 65
                            nc.tensor.matmul(
                                po[:, 2 * CH - W : 2 * CH],
                                v_all[:, vblk : vblk + 65], Es,
                                start=(mi == 0), stop=(mi == n_mm - 1),
                                skip_group_check=True,
                            )
                            mi += 1
                    rec = recp.tile([1, 2 * CH], bf16, name="rec", tag="rec")
                    with nc.allow_low_precision(reason="softmax denom bcast"):
                        nc.vector.reciprocal(rec[:], po[64:65, :])
                    rec_ps = s_ps.tile(
                        [DH, 2 * CH], f32, name="rec_ps", tag="rec_ps", bufs=2
                    )
                    nc.tensor.matmul(
                        rec_ps[:], c["ones_row"][:, 0:DH], rec[:], start=True, stop=True
                    )
                    rec_b = recp.tile([DH, 2 * CH], f32, name="rec_b", tag="rec_b")
                    nc.vector.tensor_copy(rec_b[:], rec_ps[:])
                    for half, sq in ((0, s0), (1, s1)):
                        nc.vector.tensor_tensor(
                            a2_all[lh * DH : (lh + 1) * DH,
                                   sq * T + qcol0 : sq * T + qcol0 + CH],
                            po[0:DH, half * CH : (half + 1) * CH],
                            rec_b[:, half * CH : (half + 1) * CH],
                            Alu.mult,
                        )
            # half-b attention done: ship its outputs + fire its AllToAll
            dst = bass.AP(a2i, b * HT2 * CH, [[CH, P], [P * CH, 8], [1, CH]])
            src = bass.AP(
                a2_all[:].tensor, a2_all[:].offset + b * CH,
                [[a2_all[:].ap[0][0], P], [T, 8], [1, CH]],
            )
            nc.sync.dma_start(dst, src)
            if collectives:
                nc.gpsimd.collective_compute(
                    "AllToAll", mybir.AluOpType.bypass, replica_groups=rg,
                    ins=[a2i[b * HT2 : (b + 1) * HT2, :]],
                    outs=[a2o[b * HT2 : (b + 1) * HT2, :]],
                )

    # ---------------- proj + residual1 (split by half) ----------------
    # LN2 stats for each half are emitted right after that half's residual,
    # so they overlap the other half's AllToAll/proj.
    ln2_stack = ExitStack()
    psp2 = ln2_stack.enter_context(
        tc.tile_pool(name=f"ln_psb{rep}", bufs=1, space="PSUM"))
    tmp2 = ln2_stack.enter_context(tc.tile_pool(name=f"ln_tmpb{rep}", bufs=3))
    st2 = {
        "psp": psp2, "tmp": tmp2,
        "sum": psp2.tile([1, T], f32, name="ps_sum", tag="ps_sum"),
        "sq": psp2.tile([1, T], f32, name="ps_sq", tag="ps_sq"),
        "ch": vec.tile([1, 8 * T], f32, name="lnchain", tag="lnchain"),
        "chb": vec.tile([1, 2 * T], bf16, name="lnchainb", tag="lnchainb"),
    }
    ps_sum2, ps_sq2 = st2["sum"], st2["sq"]

    x1_tiles = []
    with tc.tile_pool(
        name=f"p_ps{rep}", bufs=4, space="PSUM"
    ) as p_ps, tc.tile_pool(name=f"otp{rep}", bufs=1) as ot_p:
        wpt = env["wpt"]
        ot_all = ot_p.tile([P, 8 * T], bf16, name="ot_all", tag="ot_all")
        for h in range(2):
            dst = bass.AP(
                ot_all[:].tensor, ot_all[:].offset + h * CH,
                [[ot_all[:].ap[0][0], P], [T, 8], [1, CH]],
            )
            nc.sync.dma_start(
                dst, bass.AP(a2o, h * HT2 * CH, [[CH, P], [P * CH, 8], [1, CH]])
            )
            for do in range(8):
                ps = p_ps.tile([P, CH], f32, name="p_ps", tag="p_ps")
                for dk in range(8):
                    nc.tensor.matmul(
                        ps[:],
                        wpt[dk][:, do * P : (do + 1) * P],
                        ot_all[:, dk * T + h * CH : dk * T + (h + 1) * CH],
                        start=(dk == 0), stop=(dk == 7),
                    )
                if h == 0:
                    x1 = pools["x1"].tile([P, T], bf16, name="x1", tag="x1")
                    x1_tiles.append(x1)
                x1 = x1_tiles[do]
                nc.vector.scalar_tensor_tensor(
                    x1[:, h * CH : (h + 1) * CH], ps[:], c["bp"][:, do : do + 1],
                    x_tiles[do][:, h * CH : (h + 1) * CH], Alu.add, Alu.add,
                )
            ln_stats_half(x1_tiles, psp2, tmp2, h, ps_sum2, ps_sq2)
            ln_chain_half(st2, h)

    # ---------------- LN2 apply ----------------
    h2_tiles = []
    for h in range(2):
        ln_apply_half(st2, h, x1_tiles, c["g2"], c["be2"], pools["h2"],
                      h2_tiles)
    ln2_stack.close()
    ap_stack.close()

    # prefetch MLP-up half-0 weights; overlaps the LN2 apply tail
    wu_stack = ExitStack()
    wu_p = wu_stack.enter_context(tc.tile_pool(name=f"wu{rep}", bufs=9))
    wu_pre = []
    for dk in range(8):
        wt = wu_p.tile([P, 2048], bf16, name="wu", tag="wu")
        nc.sync.dma_start(wt[:], wuT[dk * P : (dk + 1) * P, 0:2048])
        wu_pre.append(wt)

    # ---------------- MLP up + gelu ----------------
    gu_tiles = []
    with tc.tile_pool(name=f"u_ps{rep}", bufs=3, space="PSUM") as u_ps:
        for half in range(2):
            if half == 0:
                wut = wu_pre
            else:
                wut = []
                for dk in range(8):
                    wt = wu_p.tile([P, 2048], bf16, name="wu", tag="wu")
                    nc.sync.dma_start(
                        wt[:],
                        wuT[dk * P : (dk + 1) * P, half * 2048 : (half + 1) * 2048],
                    )
                    wut.append(wt)
            for jl in range(16):
                j = half * 16 + jl
                ps = u_ps.tile([P, T], f32, name="u_ps", tag="u_ps")
                for dk in range(8):
                    nc.tensor.matmul(
                        ps[:], wut[dk][:, jl * P : (jl + 1) * P], h2_tiles[dk][:],
                        start=(dk == 0), stop=(dk == 7),
                    )
                gu = pools["gu"].tile([P, T], bf16, name="gu", tag="gu")
                nc.scalar.activation(
                    gu[:], ps[:], AFT.Gelu_apprx_tanh, bias=c["bu"][:, j : j + 1]
                )
                gu_tiles.append(gu)
    wu_stack.close()

    # ---------------- MLP down + residual2 ----------------
    out_tiles = []
    with tc.tile_pool(name=f"wd{rep}", bufs=6) as wd_p, tc.tile_pool(
        name=f"d_ps{rep}", bufs=1, space="PSUM"
    ) as d_ps:
        pss = [d_ps.tile([P, T], f32, name=f"d_ps{do}", tag=f"d_ps{do}")
               for do in range(8)]
        for j in range(32):
            wt = wd_p.tile([P, D], bf16, name="wd", tag="wd")
            nc.sync.dma_start(wt[:], wdT[j * P : (j + 1) * P, :])
            for do in range(8):
                nc.tensor.matmul(
                    pss[do][:], wt[:, do * P : (do + 1) * P], gu_tiles[j][:],
                    start=(j == 0), stop=(j == 31),
                )
        for do in range(8):
            o = pools["outp"].tile([P, T], f32, name="out_t", tag="out_t")
            nc.vector.scalar_tensor_tensor(
                o[:], pss[do][:], c["bd"][:, do : do + 1], x1_tiles[do][:],
                Alu.add, Alu.add,
            )
            if write_out:
                nc.sync.dma_start(out[do * P : (do + 1) * P, :], o[:])
            out_tiles.append(o)
    return out_tiles


def _build(nreps=1, collectives=True, hw_loop=0):
    from contextlib import ExitStack
    from concourse import bass, mybir, tile, bacc

    f32 = mybir.dt.float32
    bf16 = mybir.dt.bfloat16

    nc = bacc.Bacc("TRN2", target_bir_lowering=False, num_devices=NCORES)

    xT = nc.declare_dram_parameter("xT", [D, T], bf16, isOutput=False)
    wT = nc.declare_dram_parameter("wT", [D, 3 * D], bf16, isOutput=False)
    wpT = nc.declare_dram_parameter("wpT", [D, D], bf16, isOutput=False)
    wuT = nc.declare_dram_parameter("wuT", [D, DFF], bf16, isOutput=False)
    wdT = nc.declare_dram_parameter("wdT", [DFF, D], bf16, isOutput=False)
    bqk = nc.declare_dram_parameter("bqk", [P, 16], f32, isOutput=False)
    bv = nc.declare_dram_parameter("bv", [P, D], f32, isOutput=False)
    bp = nc.declare_dram_parameter("bp", [P, 8], f32, isOutput=False)
    bu = nc.declare_dram_parameter("bu", [P, 32], f32, isOutput=False)
    bd = nc.declare_dram_parameter("bd", [P, 8], f32, isOutput=False)
    g1 = nc.declare_dram_parameter("g1", [P, 8], f32, isOutput=False)
    be1 = nc.declare_dram_parameter("be1", [P, 8], f32, isOutput=False)
    g2 = nc.declare_dram_parameter("g2", [P, 8], f32, isOutput=False)
    be2 = nc.declare_dram_parameter("be2", [P, 8], f32, isOutput=False)
    tri = nc.declare_dram_parameter("tri", [CH, CH], bf16, isOutput=False)
    tri2 = nc.declare_dram_parameter("tri2", [CH, 4 * CH], bf16, isOutput=False)
    out = nc.declare_dram_parameter("out", [D, T], f32, isOutput=True)

    # Half-split bounce buffers: rows [h*8*SLOT, (h+1)*8*SLOT) hold batch-half
    # h (256 token cols) so each half's AllToAll is a contiguous slab.
    f8 = mybir.dt.float8e4
    a1i = nc.dram_tensor("a2a1_in", [2 * NCORES * SLOT, CH], f8)
    a1o = nc.dram_tensor("a2a1_out", [2 * NCORES * SLOT, CH], f8)
    a2i = nc.dram_tensor("a2a2_in", [2 * NCORES * QR, CH], bf16)
    a2o = nc.dram_tensor("a2a2_out", [2 * NCORES * QR, CH], bf16)

    with tile.TileContext(nc) as tc, ExitStack() as top:
        xt_pool = top.enter_context(tc.tile_pool(name="xt", bufs=8))
        x_tiles = []
        for dk in range(8):
            xt = xt_pool.tile([P, T], bf16, name="xt", tag="xt")
            nc.sync.dma_start(xt[:], xT[dk * P : (dk + 1) * P, :])
            x_tiles.append(xt)
        const = top.enter_context(tc.tile_pool(name="const", bufs=1))
        ones = const.tile([P, 1], bf16)
        nc.vector.memset(ones[:], 1.0)
        ones_f = const.tile([P, 1], f32)
        nc.vector.memset(ones_f[:], 1.0)
        ones_row = const.tile([1, P], bf16)
        nc.vector.memset(ones_row[:], 1.0)
        tri_t = [const.tile([P, CH], bf16, name=f"tri{s}", tag=f"tri{s}") for s in range(2)]
        for s in range(2):
            nc.sync.dma_start(tri_t[s][:], tri[s * P : (s + 1) * P, :])
        tri2_t = [const.tile([P, 4 * CH], bf16, name=f"tri2{s}", tag=f"tri2{s}") for s in range(2)]
        for s in range(2):
            nc.sync.dma_start(tri2_t[s][:], tri2[s * P : (s + 1) * P, :])

        def ctile(name, param, shape):
            t = const.tile(shape, f32, name=name, tag=name)
            nc.sync.dma_start(t[:], param[:, :])
            return t

        consts = {
            "ones": ones, "ones_f": ones_f, "ones_row": ones_row, "tri": tri_t,
            "tri_lo": [tri2_t[s][:, 0 : 2 * CH] for s in range(2)],
            "tri_hi": [tri2_t[s][:, 2 * CH : 4 * CH] for s in range(2)],
            "bqk": ctile("bqk_t", bqk, [P, 16]),
            "bv": ctile("bv_t", bv, [P, D]),
            "bp": ctile("bp_t", bp, [P, 8]),
            "bu": ctile("bu_t", bu, [P, 32]),
            "bd": ctile("bd_t", bd, [P, 8]),
            "g1": ctile("g1_t", g1, [P, 8]),
            "be1": ctile("be1_t", be1, [P, 8]),
            "g2": ctile("g2_t", g2, [P, 8]),
            "be2": ctile("be2_t", be2, [P, 8]),
        }

        pools = {
            "vec": top.enter_context(tc.tile_pool(name="vec", bufs=1)),
            "recp": top.enter_context(tc.tile_pool(name="recp", bufs=2)),
            "ht": top.enter_context(tc.tile_pool(name="ht", bufs=8)),
            "stg": top.enter_context(tc.tile_pool(name="stg", bufs=2)),
            "a2stg": top.enter_context(tc.tile_pool(name="a2stg", bufs=1)),
            "ot": top.enter_context(tc.tile_pool(name="ot", bufs=1)),
            "x1": top.enter_context(tc.tile_pool(name="x1", bufs=8)),
            "h2": top.enter_context(tc.tile_pool(name="h2", bufs=8)),
            "gu": top.enter_context(tc.tile_pool(name="gu", bufs=32)),
            "outp": top.enter_context(tc.tile_pool(name="outp", bufs=8)),
        }

        env = {
            "params": (xT, wT, wpT, wuT, wdT, out),
            "bounce": (a1i, a1o, a2i, a2o),
            "consts": consts,
            "pools": pools,
        }

        if hw_loop:
            with tc.For_i(0, hw_loop):
                _emit_block(nc, tc, env, 0, x_tiles, None, collectives, write_out=True)
        else:
            cur = x_tiles
            for rep in range(nreps):
                cur = _emit_block(
                    nc, tc, env, rep, cur, None, collectives,
                    write_out=(rep == nreps - 1),
                )

    nc.finalize()
    return nc


def _get_nc():
    if "nc" not in _CACHE:
        _CACHE["nc"] = _build()
    return _CACHE["nc"]


def _make_in_maps(inputs):
    x = np.asarray(inputs["x"], np.float32)
    ln1_g = np.asarray(inputs["ln1_g"], np.float32)
    ln1_b = np.asarray(inputs["ln1_b"], np.float32)
    W_attn = np.asarray(inputs["W_attn"], np.float32)
    b_attn = np.asarray(inputs["b_attn"], np.float32)
    W_proj = np.asarray(inputs["W_proj"], np.float32)
    b_proj = np.asarray(inputs["b_proj"], np.float32)
    ln2_g = np.asarray(inputs["ln2_g"], np.float32)
    ln2_b = np.asarray(inputs["ln2_b"], np.float32)
    W_up = np.asarray(inputs["W_up"], np.float32)
    b_up = np.asarray(inputs["b_up"], np.float32)
    W_down = np.asarray(inputs["W_down"], np.float32)
    b_down = np.asarray(inputs["b_down"], np.float32)

    bf = ml_dtypes.bfloat16
    wT = np.ascontiguousarray(W_attn.T).astype(bf)
    wpT = np.ascontiguousarray(W_proj.T).astype(bf)
    wuT = np.ascontiguousarray(W_up.T).astype(bf)
    wdT = np.ascontiguousarray(W_down.T).astype(bf)

    def cols(v):  # [N] -> [128, N//128]: col j = v[j*128:(j+1)*128]
        return np.ascontiguousarray(v.reshape(-1, P).T).astype(np.float32)

    tri = np.tril(np.ones((CH, CH), np.float32)).T.astype(bf)  # tri[a,b] = a<=b
    tri = np.ascontiguousarray(tri)

    ones_m = np.ones((CH, CH), np.float32)
    zeros_m = np.zeros((CH, CH), np.float32)
    tri_f = np.tril(np.ones((CH, CH), np.float32)).T
    tri2 = np.ascontiguousarray(
        np.concatenate([tri_f, ones_m, zeros_m, tri_f], axis=1)
    ).astype(bf)

    common = dict(
        wT=wT, wpT=wpT, wuT=wuT, wdT=wdT, tri2=tri2,
        bqk=cols(b_attn[: 2 * D]),
        bv=np.ascontiguousarray(np.broadcast_to(b_attn[2 * D :].reshape(1, D), (P, D))),
        bp=cols(b_proj), bu=cols(b_up), bd=cols(b_down),
        g1=cols(ln1_g), be1=cols(ln1_b), g2=cols(ln2_g), be2=cols(ln2_b),
        tri=tri,
    )

    in_maps = []
    for i in range(NCORES):
        c0 = x[0, i * CH : (i + 1) * CH]  # [256, 1024]
        c1 = x[1, (7 - i) * CH : (8 - i) * CH]
        xTi = np.ascontiguousarray(np.concatenate([c0, c1], 0).T).astype(bf)
        in_maps.append(dict(common, xT=xTi))
    return in_maps


def make_in_maps(inputs):
    return _make_in_maps(inputs)


def kernel(**inputs):
    in_maps = _make_in_maps(inputs)

    from concourse import bass_utils

    nc = _get_nc()
    res = bass_utils.run_bass_kernel_spmd(
        nc, in_maps, core_ids=list(range(NCORES)), trace=TRACE
    )
    _CACHE["last_res"] = res
    y = np.empty((B, S, D), np.float32)
    for i in range(NCORES):
        o = np.asarray(res.results[i]["out"], np.float32)  # [1024, 512]
        y[0, i * CH : (i + 1) * CH] = o[:, :CH].T
        y[1, (7 - i) * CH : (8 - i) * CH] = o[:, CH:].T
    return y


# revision 39
# speedup vs baseline: 194.0422x; 1.0596x over previous
"""Trainium2 distributed kernel for a dense transformer block (8 NeuronCores).

Sharding: tokens are data-parallel for LN/QKV/proj/MLP (512 tokens/core,
causal-balanced pairing: core i owns batch0 chunk i and batch1 chunk 7-i),
attention is head-parallel (2 heads/core) via an AllToAll exchange of
Q/K/V, plus a second AllToAll to bring attention outputs back to token
sharding.  All matmuls run in bf16 (f32 accumulation in PSUM); LayerNorm
statistics are computed with ones-vector matmuls so every activation
stays in transposed [d, token] layout on chip.

DMA strategy: weight/activation transfers are batched into wide tiles
(multi-block access patterns) to minimize HWDGE descriptor-queue
serialization; attention Q/K/V live in single wide SBUF tiles sliced
per-head/chunk, with batch1 slot order reversed so both batches index
chunks in ascending global order.
"""

import sys

sys.path.insert(0, "/opt/trn_rl_repo")

import numpy as np
import ml_dtypes

NCORES = 8
D = 1024
H = 16
DH = 64
HL = H // NCORES  # heads per core = 2
B = 2
S = 2048
T = 512  # tokens per core
CH = 256  # token chunk (half of T)
DFF = 4096
P = 128
QR, KR, VR = 128, 128, 130  # slot row counts: qT, kT, packed-v regions
SLOT = QR + KR + VR  # 386
EPS = 1e-5

_CACHE = {}
TRACE = False


def _emit_block(nc, tc, env, rep, x_tiles, x_all, collectives, write_out):
    """Emit one transformer block; returns the 8 output [128,T] f32 tiles."""
    from contextlib import ExitStack
    from concourse import bass, mybir

    f32 = mybir.dt.float32
    bf16 = mybir.dt.bfloat16
    f8 = mybir.dt.float8e4
    Alu = mybir.AluOpType
    AFT = mybir.ActivationFunctionType

    (xT, wT, wpT, wuT, wdT, out) = env["params"]
    (a1i, a1o, a2i, a2o) = env["bounce"]
    if not collectives:
        a1o, a2o = a1i, a2i
    c = env["consts"]
    pools = env["pools"]
    vec, recp = pools["vec"], pools["recp"]
    rg = [list(range(NCORES))]

    def ap3(t, part0, nprt, off, dims):
        """Custom free-dim AP on tile t at partition slice [part0, part0+nprt)."""
        base = t[:]
        pstride = base.ap[0][0]
        return bass.AP(base.tensor, base.offset + part0 * pstride + off,
                      [[pstride, nprt]] + dims)

    def ln_stats_half(x_tiles, psp, tmp_p, h, ps_sum, ps_sq):
        c0, c1 = h * CH, (h + 1) * CH
        for dk in range(8):
            nc.tensor.matmul(
                ps_sum[:, c0:c1], c["ones"][:], x_tiles[dk][:, c0:c1],
                start=(dk == 0), stop=(dk == 7),
            )
            sq = tmp_p.tile([P, CH], bf16, name="sq", tag="sq")
            nc.scalar.activation(sq[:], x_tiles[dk][:, c0:c1], AFT.Square)
            nc.tensor.matmul(
                ps_sq[:, c0:c1], c["ones"][:], sq[:],
                start=(dk == 0), stop=(dk == 7),
            )

    def ln_chain_half(st, h):
        """Per-half stats -> rstd/mur broadcast tiles (sbuf bf16)."""
        c0, c1 = h * CH, (h + 1) * CH
        ch, chb, psp, tmp_p, ps_sum, ps_sq = (
            st["ch"], st["chb"], st["psp"], st["tmp"], st["sum"], st["sq"])
        mu, msq, mu2, var, std, rstd, mur = (
            ch[0:1, i * T + c0 : i * T + c1] for i in range(7))
        rstd_c, mur_c = chb[0:1, c0:c1], chb[0:1, T + c0 : T + c1]
        nc.vector.tensor_scalar(mu, ps_sum[:, c0:c1], 1.0 / D, None, Alu.mult)
        nc.vector.tensor_scalar(msq, ps_sq[:, c0:c1], 1.0 / D, None, Alu.mult)
        nc.vector.tensor_tensor(mu2, mu, mu, Alu.mult)
        nc.vector.tensor_tensor(var, msq, mu2, Alu.subtract)
        nc.vector.tensor_scalar(var, var, EPS, None, Alu.add)
        nc.scalar.activation(std, var, AFT.Sqrt)
        nc.vector.reciprocal(rstd, std)
        nc.vector.tensor_tensor(mur, mu, rstd, Alu.mult)
        nc.vector.tensor_copy(rstd_c, rstd)
        nc.vector.tensor_copy(mur_c, mur)
        rstd_b = psp.tile([P, CH], f32, name="rstd_b", tag="rstd_b", bufs=1)
        nc.tensor.matmul(rstd_b[:], c["ones_row"][:], rstd_c, start=True, stop=True)
        mur_b = psp.tile([P, CH], f32, name="mur_b", tag="mur_b", bufs=1)
        nc.tensor.matmul(mur_b[:], c["ones_row"][:], mur_c, start=True, stop=True)
        rstd_bb = tmp_p.tile([P, CH], bf16, name="rstd_bb", tag="rstd_bb", bufs=2)
        nc.scalar.activation(rstd_bb[:], rstd_b[:], AFT.Copy)
        mur_bb = tmp_p.tile([P, CH], bf16, name="mur_bb", tag="mur_bb", bufs=2)
        nc.scalar.activation(mur_bb[:], mur_b[:], AFT.Copy)
        st[("bc", h)] = (rstd_bb, mur_bb)

    def ln_apply_half(st, h, x_tiles, g_tile, b_tile, out_pool, outs):
        c0, c1 = h * CH, (h + 1) * CH
        rstd_bb, mur_bb = st[("bc", h)]
        for dk in range(8):
            t1 = st["tmp"].tile([P, CH], bf16, name="lnt1", tag="lnt1")
            nc.vector.tensor_tensor(
                t1[:], x_tiles[dk][:, c0:c1], rstd_bb[:], Alu.mult)
            nc.vector.tensor_tensor(t1[:], t1[:], mur_bb[:], Alu.subtract)
            if h == 0:
                outs.append(out_pool.tile([P, T], bf16, name="ln_out",
                                          tag="ln_out"))
            nc.scalar.activation(
                outs[dk][:, c0:c1], t1[:], AFT.Identity,
                bias=b_tile[:, dk : dk + 1], scale=g_tile[:, dk : dk + 1],
            )
        return outs

    if x_tiles[0].dtype != bf16:
        conv = []
        for dk in range(8):
            xc = pools["ht"].tile([P, T], bf16, name="xc", tag="xc")
            nc.scalar.activation(xc[:], x_tiles[dk][:], AFT.Copy)
            conv.append(xc)
        x_tiles = conv

    # ---------------- LN1 (per-half so QKV(h0) starts after apply(h0)) ----
    ln1_stack = ExitStack()
    psp1 = ln1_stack.enter_context(
        tc.tile_pool(name=f"ln_psa{rep}", bufs=1, space="PSUM"))
    tmp1 = ln1_stack.enter_context(tc.tile_pool(name=f"ln_tmpa{rep}", bufs=3))
    st1 = {
        "psp": psp1, "tmp": tmp1,
        "sum": psp1.tile([1, T], f32, name="ps_sum", tag="ps_sum"),
        "sq": psp1.tile([1, T], f32, name="ps_sq", tag="ps_sq"),
        "ch": vec.tile([1, 8 * T], f32, name="lnchain", tag="lnchain"),
        "chb": vec.tile([1, 2 * T], bf16, name="lnchainb", tag="lnchainb"),
    }
    for h in range(2):
        ln_stats_half(x_tiles, psp1, tmp1, h, st1["sum"], st1["sq"])
        ln_chain_half(st1, h)
    h_tiles = []
    for h in range(2):
        ln_apply_half(st1, h, x_tiles, c["g1"], c["be1"], pools["ht"], h_tiles)
    ln1_stack.close()

    # ---------------- QKV (split by batch half for collective overlap) ------
    # Round A covers wT cols 0:1536 (q slots 0-7, k slots 0-3); round B covers
    # 1536:3072 (k slots 4-7 and all of v).  For each half h (b0 tokens =
    # cols 0:256, b1 = 256:512) all staging lands in a1i rows [h*8*SLOT, ...),
    # then the half's AllToAll fires while the other half computes.
    HT = NCORES * SLOT  # rows per half in a1i/a1o

    def qk_block(h, jts, wtiles, col0):
        """Emit psums + staged write for 4 consecutive q/k outputs, half h."""
        stg = pools["stg"].tile([P, 4 * CH], f8, name="stg", tag="stg")
        for i, jt in enumerate(jts):
            ps = env["qkps"].tile([P, CH], f32, name="qk_ps", tag="qk_ps")
            for dk in range(8):
                nc.tensor.matmul(
                    ps[:],
                    wtiles[dk][:, jt * P - col0 : (jt + 1) * P - col0],
                    h_tiles[dk][:, h * CH : (h + 1) * CH],
                    start=(dk == 0), stop=(dk == 7),
                )
            nc.scalar.activation(
                stg[:, i * CH : (i + 1) * CH], ps[:], AFT.Identity,
                bias=c["bqk"][:, jt : jt + 1],
            )
        jt0 = jts[0]
        base = (h * HT + (jt0 * SLOT if jt0 < 8 else (jt0 - 8) * SLOT + QR)) * CH
        dst = bass.AP(a1i, base, [[CH, P], [SLOT * CH, 4], [1, CH]])
        src = stg[:].rearrange("p (j t) -> p j t", t=CH)
        nc.sync.dma_start(dst, src)

    def emit_v(h, wB, vst_p, v_ps):
        # v: psum [tokens, vdims]; stage 4 slots (x2 heads+ones) per DMA
        for jc in range(2):
            for tt in range(2):
                ps = v_ps.tile([P, 512], f32, name="v_ps", tag="v_ps")
                for dk in range(8):
                    nc.tensor.matmul(
                        ps[:],
                        h_tiles[dk][:, h * CH + tt * P : h * CH + (tt + 1) * P],
                        wB[dk][:, 512 + jc * 512 : 1024 + jc * 512],
                        start=(dk == 0), stop=(dk == 7),
                    )
                vt = vst_p.tile([P, 4 * VR], f8, name="vst", tag="vst")
                for sl in range(4):
                    slot = jc * 4 + sl
                    for lh in range(HL):
                        nc.vector.tensor_tensor(
                            vt[:, sl * VR + lh * 65 : sl * VR + lh * 65 + DH],
                            ps[:, sl * P + lh * DH : sl * P + lh * DH + DH],
                            c["bv"][:, slot * P + lh * DH : slot * P + lh * DH + DH],
                            Alu.add,
                        )
                        nc.vector.memset(
                            vt[:, sl * VR + lh * 65 + DH : sl * VR + lh * 65 + DH + 1],
                            1.0,
                        )
                base = (h * HT + (jc * 4) * SLOT + QR + KR) * CH + tt * P * VR
                dst = bass.AP(a1i, base, [[VR, P], [SLOT * CH, 4], [1, VR]])
                src = vt[:].rearrange("p (s c) -> p s c", c=VR)
                nc.sync.dma_start(dst, src)

    def fire_a2a1(h):
        if collectives:
            nc.gpsimd.collective_compute(
                "AllToAll", mybir.AluOpType.bypass, replica_groups=rg,
                ins=[a1i[h * HT : (h + 1) * HT, :]],
                outs=[a1o[h * HT : (h + 1) * HT, :]],
            )

    with tc.tile_pool(name=f"wqkA{rep}", bufs=8) as wqk_a, tc.tile_pool(
        name=f"wqkB{rep}", bufs=8
    ) as wqk_b, tc.tile_pool(name=f"qk{rep}", bufs=4, space="PSUM") as qk_ps, tc.tile_pool(
        name=f"vst{rep}", bufs=2
    ) as vst_p, tc.tile_pool(name=f"v_ps{rep}", bufs=2, space="PSUM") as v_ps:
        env["qkps"] = qk_ps
        wA, wB = [], []
        for dk in range(8):
            wt = wqk_a.tile([P, 1536], bf16, name="wA", tag="wA")
            nc.sync.dma_start(wt[:], wT[dk * P : (dk + 1) * P, 0:1536])
            wA.append(wt)
        for dk in range(8):
            wt = wqk_b.tile([P, 1536], bf16, name="wB", tag="wB")
            nc.sync.dma_start(wt[:], wT[dk * P : (dk + 1) * P, 1536:3072])
            wB.append(wt)
        for h in range(2):
            for blk in range(3):
                qk_block(h, list(range(blk * 4, blk * 4 + 4)), wA, 0)
            qk_block(h, [12, 13, 14, 15], wB, 1536)
            emit_v(h, wB, vst_p, v_ps)
            fire_a2a1(h)

    # ---------------- attention (batch-outer; overlaps collectives) --------
    # Wide per-core tiles; batch0 chunks at ascending slot order, batch1
    # chunks stored slot-reversed so both batches index by global chunk id.
    a2_all = pools["a2stg"].tile([P, 8 * T], bf16, name="a2all", tag="a2all")
    HT2 = NCORES * QR  # rows per half in a2i/a2o

    wu_stack = ExitStack()
    wu_p = wu_stack.enter_context(tc.tile_pool(name=f"wu{rep}", bufs=9))
    wuA = []
    for dk in range(8):
        wt = wu_p.tile([P, 1024], bf16, name="wuA", tag="wuA", bufs=8)
        nc.sync.dma_start(wt[:], wuT[dk * P : (dk + 1) * P, 0:1024])
        wuA.append(wt)
    ap_stack = ExitStack()
    wp_p = ap_stack.enter_context(tc.tile_pool(name=f"wp{rep}", bufs=8))
    env["wp_pool"] = wp_p
    with tc.tile_pool(name=f"qkv{rep}", bufs=1) as qkv_p, tc.tile_pool(
        name=f"qe{rep}", bufs=4
    ) as qe_p, tc.tile_pool(name=f"s_ps{rep}", bufs=2, space="PSUM") as s_ps, tc.tile_pool(
        name=f"o_ps{rep}", bufs=2, space="PSUM"
    ) as o_ps:
        q_all = qkv_p.tile([P, 4096], f8, name="q_all", tag="q_all")
        k_all = qkv_p.tile([P, 4096], f8, name="k_all", tag="k_all")
        v_all = qkv_p.tile([P, 32 * VR], f8, name="v_all", tag="v_all")

        def load_attn(b):
            # load in the order attention consumes chunks: for b1 the chunk
            # index is 7-s, so iterate slots descending
            order = range(8) if b == 0 else reversed(range(8))
            for s in order:
                pos = s if b == 0 else 7 - s
                for (tile_, rowoff) in ((q_all, 0), (k_all, QR)):
                    src = bass.AP(
                        a1o, (b * HT + s * SLOT + rowoff) * CH, [[CH, P], [1, CH]]
                    )
                    nc.sync.dma_start(
                        tile_[:, b * 2048 + pos * CH : b * 2048 + (pos + 1) * CH], src
                    )
                vbase = (b * HT + s * SLOT + QR + KR) * CH
                src = bass.AP(a1o, vbase, [[VR, P], [P * VR, 2], [1, VR]])
                blk0 = (b * 16 + pos * 2) * VR
                nc.sync.dma_start(v_all[:, blk0 : blk0 + 2 * VR], src)

        load_attn(0)
        load_attn(1)  # waits on A2A1(1) via deps; overlaps b0 compute
        # prefetch proj weights during attention
        wpt = []
        for dk in range(8):
            wt = env["wp_pool"].tile([P, D], bf16, name="wp", tag="wp")
            nc.sync.dma_start(wt[:], wpT[dk * P : (dk + 1) * P, :])
            wpt.append(wt)
        env["wpt"] = wpt

        for b in range(B):
            for lh in range(HL):
                for pr in range(4):  # query-chunk pairs (2pr, 2pr+1)
                    q0, q1 = 2 * pr, 2 * pr + 1
                    s0 = q0 if b == 0 else 7 - q0
                    s1 = q1 if b == 0 else 7 - q1
                    qcol0 = b * CH
                    qmv = q_all[lh * DH : (lh + 1) * DH,
                                b * 2048 + q0 * CH : b * 2048 + (q1 + 1) * CH]
                    po = o_ps.tile([65, 2 * CH], f32, name="o_ps", tag="o_ps")
                    n_mm = 2 * (q1 + 1)
                    mi = 0
                    for kc in range(q1 + 1):
                        # kc == q1 blocks only contribute to the q1 (right)
                        # query half; the q0 half is fully above-diagonal.
                        narrow = kc == q1
                        qs = qmv[:, CH : 2 * CH] if narrow else qmv
                        W = CH if narrow else 2 * CH
                        # both k sub-chunks land in one 2-bank psum tile so a
                        # single Exp covers them
                        ps = s_ps.tile([P, 4 * CH], f32, name="s_ps", tag="s_ps")
                        E = qe_p.tile([P, 4 * CH], bf16, name="E", tag="E")
                        for sub in range(2):
                            nc.tensor.matmul(
                                ps[:, sub * W : (sub + 1) * W],
                                k_all[lh * DH : (lh + 1) * DH,
                                      b * 2048 + kc * CH + sub * P
                                      : b * 2048 + kc * CH + (sub + 1) * P],
                                qs,
                                start=True, stop=True,
                                skip_group_check=True,
                            )
                        nc.scalar.activation(
                            E[:, 0 : 2 * W], ps[:, 0 : 2 * W], AFT.Exp, scale=0.125
                        )
                        for sub in range(2):
                            Es = E[:, sub * W : (sub + 1) * W]
                            if kc == q0 and not narrow:  # diagonal for q0
                                nc.vector.tensor_tensor(
                                    Es, Es, c["tri_lo"][sub][:], Alu.mult
                                )
                            elif narrow:  # diagonal for q1
                                nc.vector.tensor_tensor(
                                    Es, Es, c["tri_hi"][sub][:, CH : 2 * CH],
                                    Alu.mult,
                                )
                            vblk = (b * 16 + kc * 2 + sub) * VR + lh * 65
                            nc.tensor.matmul(
                                po[:, 2 * CH - W : 2 * CH],
                                v_all[:, vblk : vblk + 65], Es,
                                start=(mi == 0), stop=(mi == n_mm - 1),
                                skip_group_check=True,
                            )
                            mi += 1
                    rec = recp.tile([1, 2 * CH], bf16, name="rec", tag="rec")
                    with nc.allow_low_precision(reason="softmax denom bcast"):
                        nc.vector.reciprocal(rec[:], po[64:65, :])
                    rec_ps = s_ps.tile(
                        [DH, 2 * CH], f32, name="rec_ps", tag="rec_ps", bufs=2
                    )
                    nc.tensor.matmul(
                        rec_ps[:], c["ones_row"][:, 0:DH], rec[:], start=True, stop=True
                    )
                    rec_b = recp.tile([DH, 2 * CH], f32, name="rec_b", tag="rec_b")
                    nc.vector.tensor_copy(rec_b[:], rec_ps[:])
                    for half, sq in ((0, s0), (1, s1)):
                        nc.vector.tensor_tensor(
                            a2_all[lh * DH : (lh + 1) * DH,
                                   sq * T + qcol0 : sq * T + qcol0 + CH],
                            po[0:DH, half * CH : (half + 1) * CH],
                            rec_b[:, half * CH : (half + 1) * CH],
                            Alu.mult,
                        )
            # half-b attention done: ship its outputs + fire its AllToAll
            dst = bass.AP(a2i, b * HT2 * CH, [[CH, P], [P * CH, 8], [1, CH]])
            src = bass.AP(
                a2_all[:].tensor, a2_all[:].offset + b * CH,
                [[a2_all[:].ap[0][0], P], [T, 8], [1, CH]],
            )
            nc.sync.dma_start(dst, src)
            if collectives:
                nc.gpsimd.collective_compute(
                    "AllToAll", mybir.AluOpType.bypass, replica_groups=rg,
                    ins=[a2i[b * HT2 : (b + 1) * HT2, :]],
                    outs=[a2o[b * HT2 : (b + 1) * HT2, :]],
                )

    # ---------------- proj + residual1 (split by half) ----------------
    # LN2 stats for each half are emitted right after that half's residual,
    # so they overlap the other half's AllToAll/proj.
    ln2_stack = ExitStack()
    psp2 = ln2_stack.enter_context(
        tc.tile_pool(name=f"ln_psb{rep}", bufs=1, space="PSUM"))
    tmp2 = ln2_stack.enter_context(tc.tile_pool(name=f"ln_tmpb{rep}", bufs=3))
    st2 = {
        "psp": psp2, "tmp": tmp2,
        "sum": psp2.tile([1, T], f32, name="ps_sum", tag="ps_sum"),
        "sq": psp2.tile([1, T], f32, name="ps_sq", tag="ps_sq"),
        "ch": vec.tile([1, 8 * T], f32, name="lnchain", tag="lnchain"),
        "chb": vec.tile([1, 2 * T], bf16, name="lnchainb", tag="lnchainb"),
    }
    ps_sum2, ps_sq2 = st2["sum"], st2["sq"]

    x1_tiles = []
    with tc.tile_pool(
        name=f"p_ps{rep}", bufs=2, space="PSUM"
    ) as p_ps, tc.tile_pool(name=f"otp{rep}", bufs=1) as ot_p:
        wpt = env["wpt"]
        ot_all = ot_p.tile([P, 8 * T], bf16, name="ot_all", tag="ot_all")
        for h in range(2):
            dst = bass.AP(
                ot_all[:].tensor, ot_all[:].offset + h * CH,
                [[ot_all[:].ap[0][0], P], [T, 8], [1, CH]],
            )
            nc.sync.dma_start(
                dst, bass.AP(a2o, h * HT2 * CH, [[CH, P], [P * CH, 8], [1, CH]])
            )
            for do in range(8):
                ps = p_ps.tile([P, CH], f32, name="p_ps", tag="p_ps")
                for dk in range(8):
                    nc.tensor.matmul(
                        ps[:],
                        wpt[dk][:, do * P : (do + 1) * P],
                        ot_all[:, dk * T + h * CH : dk * T + (h + 1) * CH],
                        start=(dk == 0), stop=(dk == 7),
                    )
                if h == 0:
                    x1 = pools["x1"].tile([P, T], bf16, name="x1", tag="x1")
                    x1_tiles.append(x1)
                x1 = x1_tiles[do]
                nc.vector.scalar_tensor_tensor(
                    x1[:, h * CH : (h + 1) * CH], ps[:], c["bp"][:, do : do + 1],
                    x_tiles[do][:, h * CH : (h + 1) * CH], Alu.add, Alu.add,
                )
            ln_stats_half(x1_tiles, psp2, tmp2, h, ps_sum2, ps_sq2)
            ln_chain_half(st2, h)
            if h == 0:
                # fill the A2A2(1) window: apply LN2 on half 0 and run a
                # half-width up-proj pass for the first 8 j blocks
                h2_tiles = []
                ln_apply_half(st2, 0, x1_tiles, c["g2"], c["be2"],
                              pools["h2"], h2_tiles)
                gu_part = []
                with tc.tile_pool(name=f"u_pre{rep}", bufs=2,
                                  space="PSUM") as upre_ps:
                    for j in range(8):
                        ps = upre_ps.tile([P, CH], f32, name="upre",
                                          tag="upre")
                        for dk in range(8):
                            nc.tensor.matmul(
                                ps[:], wuA[dk][:, j * P : (j + 1) * P],
                                h2_tiles[dk][:, 0:CH],
                                start=(dk == 0), stop=(dk == 7),
                            )
                        gu = pools["gu"].tile([P, T], bf16, name="gu",
                                              tag="gu")
                        nc.scalar.activation(
                            gu[:, 0:CH], ps[:], AFT.Gelu_apprx_tanh,
                            bias=c["bu"][:, j : j + 1],
                        )
                        gu_part.append(gu)

    # ---------------- LN2 apply (half 1) ----------------
    ln_apply_half(st2, 1, x1_tiles, c["g2"], c["be2"], pools["h2"], h2_tiles)
    ln2_stack.close()
    ap_stack.close()

    # ---------------- MLP up + gelu ----------------
    gu_tiles = list(gu_part)
    with tc.tile_pool(name=f"u_ps{rep}", bufs=3, space="PSUM") as u_ps:
        # finish j0..7: token half 1
        for j in range(8):
            ps = u_ps.tile([P, CH], f32, name="u_ps1", tag="u_ps1", bufs=3)
            for dk in range(8):
                nc.tensor.matmul(
                    ps[:], wuA[dk][:, j * P : (j + 1) * P],
                    h2_tiles[dk][:, CH:T],
                    start=(dk == 0), stop=(dk == 7),
                )
            nc.scalar.activation(
                gu_part[j][:, CH:T], ps[:], AFT.Gelu_apprx_tanh,
                bias=c["bu"][:, j : j + 1],
            )
        # j8..31 full width, weights loaded in [128,1024] groups
        for grp in range(1, 4):
            wut = []
            for dk in range(8):
                wt = wu_p.tile([P, 1024], bf16, name="wu", tag="wu")
                nc.sync.dma_start(
                    wt[:],
                    wuT[dk * P : (dk + 1) * P, grp * 1024 : (grp + 1) * 1024],
                )
                wut.append(wt)
            for jl in range(8):
                j = grp * 8 + jl
                ps = u_ps.tile([P, T], f32, name="u_ps", tag="u_ps")
                for dk in range(8):
                    nc.tensor.matmul(
                        ps[:], wut[dk][:, jl * P : (jl + 1) * P], h2_tiles[dk][:],
                        start=(dk == 0), stop=(dk == 7),
                    )
                gu = pools["gu"].tile([P, T], bf16, name="gu", tag="gu")
                nc.scalar.activation(
                    gu[:], ps[:], AFT.Gelu_apprx_tanh, bias=c["bu"][:, j : j + 1]
                )
                gu_tiles.append(gu)
    wu_stack.close()

    # ---------------- MLP down + residual2 ----------------
    out_tiles = []
    with tc.tile_pool(name=f"wd{rep}", bufs=6) as wd_p, tc.tile_pool(
        name=f"d_ps{rep}", bufs=1, space="PSUM"
    ) as d_ps:
        pss = [d_ps.tile([P, T], f32, name=f"d_ps{do}", tag=f"d_ps{do}")
               for do in range(8)]
        for j in range(32):
            wt = wd_p.tile([P, D], bf16, name="wd", tag="wd")
            nc.sync.dma_start(wt[:], wdT[j * P : (j + 1) * P, :])
            for do in range(8):
                nc.tensor.matmul(
                    pss[do][:], wt[:, do * P : (do + 1) * P], gu_tiles[j][:],
                    start=(j == 0), stop=(j == 31),
                )
        for do in range(8):
            o = pools["outp"].tile([P, T], f32, name="out_t", tag="out_t")
            nc.vector.scalar_tensor_tensor(
                o[:], pss[do][:], c["bd"][:, do : do + 1], x1_tiles[do][:],
                Alu.add, Alu.add,
            )
            if write_out:
                nc.sync.dma_start(out[do * P : (do + 1) * P, :], o[:])
            out_tiles.append(o)
    return out_tiles


def _build(nreps=1, collectives=True, hw_loop=0):
    from contextlib import ExitStack
    from concourse import bass, mybir, tile, bacc

    f32 = mybir.dt.float32
    bf16 = mybir.dt.bfloat16

    nc = bacc.Bacc("TRN2", target_bir_lowering=False, num_devices=NCORES)

    xT = nc.declare_dram_parameter("xT", [D, T], bf16, isOutput=False)
    wT = nc.declare_dram_parameter("wT", [D, 3 * D], bf16, isOutput=False)
    wpT = nc.declare_dram_parameter("wpT", [D, D], bf16, isOutput=False)
    wuT = nc.declare_dram_parameter("wuT", [D, DFF], bf16, isOutput=False)
    wdT = nc.declare_dram_parameter("wdT", [DFF, D], bf16, isOutput=False)
    bqk = nc.declare_dram_parameter("bqk", [P, 16], f32, isOutput=False)
    bv = nc.declare_dram_parameter("bv", [P, D], f32, isOutput=False)
    bp = nc.declare_dram_parameter("bp", [P, 8], f32, isOutput=False)
    bu = nc.declare_dram_parameter("bu", [P, 32], f32, isOutput=False)
    bd = nc.declare_dram_parameter("bd", [P, 8], f32, isOutput=False)
    g1 = nc.declare_dram_parameter("g1", [P, 8], f32, isOutput=False)
    be1 = nc.declare_dram_parameter("be1", [P, 8], f32, isOutput=False)
    g2 = nc.declare_dram_parameter("g2", [P, 8], f32, isOutput=False)
    be2 = nc.declare_dram_parameter("be2", [P, 8], f32, isOutput=False)
    tri = nc.declare_dram_parameter("tri", [CH, CH], bf16, isOutput=False)
    tri2 = nc.declare_dram_parameter("tri2", [CH, 4 * CH], bf16, isOutput=False)
    out = nc.declare_dram_parameter("out", [D, T], f32, isOutput=True)

    # Half-split bounce buffers: rows [h*8*SLOT, (h+1)*8*SLOT) hold batch-half
    # h (256 token cols) so each half's AllToAll is a contiguous slab.
    f8 = mybir.dt.float8e4
    a1i = nc.dram_tensor("a2a1_in", [2 * NCORES * SLOT, CH], f8)
    a1o = nc.dram_tensor("a2a1_out", [2 * NCORES * SLOT, CH], f8)
    a2i = nc.dram_tensor("a2a2_in", [2 * NCORES * QR, CH], bf16)
    a2o = nc.dram_tensor("a2a2_out", [2 * NCORES * QR, CH], bf16)

    with tile.TileContext(nc) as tc, ExitStack() as top:
        xt_pool = top.enter_context(tc.tile_pool(name="xt", bufs=8))
        x_tiles = []
        for dk in range(8):
            xt = xt_pool.tile([P, T], bf16, name="xt", tag="xt")
            nc.sync.dma_start(xt[:], xT[dk * P : (dk + 1) * P, :])
            x_tiles.append(xt)
        const = top.enter_context(tc.tile_pool(name="const", bufs=1))
        ones = const.tile([P, 1], bf16)
        nc.vector.memset(ones[:], 1.0)
        ones_f = const.tile([P, 1], f32)
        nc.vector.memset(ones_f[:], 1.0)
        ones_row = const.tile([1, P], bf16)
        nc.vector.memset(ones_row[:], 1.0)
        tri_t = [const.tile([P, CH], bf16, name=f"tri{s}", tag=f"tri{s}") for s in range(2)]
        for s in range(2):
            nc.sync.dma_start(tri_t[s][:], tri[s * P : (s + 1) * P, :])
        tri2_t = [const.tile([P, 4 * CH], bf16, name=f"tri2{s}", tag=f"tri2{s}") for s in range(2)]
        for s in range(2):
            nc.sync.dma_start(tri2_t[s][:], tri2[s * P : (s + 1) * P, :])

        def ctile(name, param, shape):
            t = const.tile(shape, f32, name=name, tag=name)
            nc.sync.dma_start(t[:], param[:, :])
            return t

        consts = {
            "ones": ones, "ones_f": ones_f, "ones_row": ones_row, "tri": tri_t,
            "tri_lo": [tri2_t[s][:, 0 : 2 * CH] for s in range(2)],
            "tri_hi": [tri2_t[s][:, 2 * CH : 4 * CH] for s in range(2)],
            "bqk": ctile("bqk_t", bqk, [P, 16]),
            "bv": ctile("bv_t", bv, [P, D]),
            "bp": ctile("bp_t", bp, [P, 8]),
            "bu": ctile("bu_t", bu, [P, 32]),
            "bd": ctile("bd_t", bd, [P, 8]),
            "g1": ctile("g1_t", g1, [P, 8]),
            "be1": ctile("be1_t", be1, [P, 8]),
            "g2": ctile("g2_t", g2, [P, 8]),
            "be2": ctile("be2_t", be2, [P, 8]),
        }

        pools = {
            "vec": top.enter_context(tc.tile_pool(name="vec", bufs=1)),
            "recp": top.enter_context(tc.tile_pool(name="recp", bufs=2)),
            "ht": top.enter_context(tc.tile_pool(name="ht", bufs=8)),
            "stg": top.enter_context(tc.tile_pool(name="stg", bufs=2)),
            "a2stg": top.enter_context(tc.tile_pool(name="a2stg", bufs=1)),
            "ot": top.enter_context(tc.tile_pool(name="ot", bufs=1)),
            "x1": top.enter_context(tc.tile_pool(name="x1", bufs=8)),
            "h2": top.enter_context(tc.tile_pool(name="h2", bufs=8)),
            "gu": top.enter_context(tc.tile_pool(name="gu", bufs=32)),
            "outp": top.enter_context(tc.tile_pool(name="outp", bufs=8)),
        }

        env = {
            "params": (xT, wT, wpT, wuT, wdT, out),
            "bounce": (a1i, a1o, a2i, a2o),
            "consts": consts,
            "pools": pools,
        }

        if hw_loop:
            with tc.For_i(0, hw_loop):
                _emit_block(nc, tc, env, 0, x_tiles, None, collectives, write_out=True)
        else:
            cur = x_tiles
            for rep in range(nreps):
                cur = _emit_block(
                    nc, tc, env, rep, cur, None, collectives,
                    write_out=(rep == nreps - 1),
                )

    nc.finalize()
    return nc


def _get_nc():
    if "nc" not in _CACHE:
        _CACHE["nc"] = _build()
    return _CACHE["nc"]


def _make_in_maps(inputs):
    x = np.asarray(inputs["x"], np.float32)
    ln1_g = np.asarray(inputs["ln1_g"], np.float32)
    ln1_b = np.asarray(inputs["ln1_b"], np.float32)
    W_attn = np.asarray(inputs["W_attn"], np.float32)
    b_attn = np.asarray(inputs["b_attn"], np.float32)
    W_proj = np.asarray(inputs["W_proj"], np.float32)
    b_proj = np.asarray(inputs["b_proj"], np.float32)
    ln2_g = np.asarray(inputs["ln2_g"], np.float32)
    ln2_b = np.asarray(inputs["ln2_b"], np.float32)
    W_up = np.asarray(inputs["W_up"], np.float32)
    b_up = np.asarray(inputs["b_up"], np.float32)
    W_down = np.asarray(inputs["W_down"], np.float32)
    b_down = np.asarray(inputs["b_down"], np.float32)

    bf = ml_dtypes.bfloat16
    wT = np.ascontiguousarray(W_attn.T).astype(bf)
    wpT = np.ascontiguousarray(W_proj.T).astype(bf)
    wuT = np.ascontiguousarray(W_up.T).astype(bf)
    wdT = np.ascontiguousarray(W_down.T).astype(bf)

    def cols(v):  # [N] -> [128, N//128]: col j = v[j*128:(j+1)*128]
        return np.ascontiguousarray(v.reshape(-1, P).T).astype(np.float32)

    tri = np.tril(np.ones((CH, CH), np.float32)).T.astype(bf)  # tri[a,b] = a<=b
    tri = np.ascontiguousarray(tri)

    ones_m = np.ones((CH, CH), np.float32)
    zeros_m = np.zeros((CH, CH), np.float32)
    tri_f = np.tril(np.ones((CH, CH), np.float32)).T
    tri2 = np.ascontiguousarray(
        np.concatenate([tri_f, ones_m, zeros_m, tri_f], axis=1)
    ).astype(bf)

    common = dict(
        wT=wT, wpT=wpT, wuT=wuT, wdT=wdT, tri2=tri2,
        bqk=cols(b_attn[: 2 * D]),
        bv=np.ascontiguousarray(np.broadcast_to(b_attn[2 * D :].reshape(1, D), (P, D))),
        bp=cols(b_proj), bu=cols(b_up), bd=cols(b_down),
        g1=cols(ln1_g), be1=cols(ln1_b), g2=cols(ln2_g), be2=cols(ln2_b),
        tri=tri,
    )

    in_maps = []
    for i in range(NCORES):
        c0 = x[0, i * CH : (i + 1) * CH]  # [256, 1024]
        c1 = x[1, (7 - i) * CH : (8 - i) * CH]
        xTi = np.ascontiguousarray(np.concatenate([c0, c1], 0).T).astype(bf)
        in_maps.append(dict(common, xT=xTi))
    return in_maps


def make_in_maps(inputs):
    return _make_in_maps(inputs)


def kernel(**inputs):
    in_maps = _make_in_maps(inputs)

    from concourse import bass_utils

    nc = _get_nc()
    res = bass_utils.run_bass_kernel_spmd(
        nc, in_maps, core_ids=list(range(NCORES)), trace=TRACE
    )
    _CACHE["last_res"] = res
    y = np.empty((B, S, D), np.float32)
    for i in range(NCORES):
        o = np.asarray(res.results[i]["out"], np.float32)  # [1024, 512]
        y[0, i * CH : (i + 1) * CH] = o[:, :CH].T
        y[1, (7 - i) * CH : (8 - i) * CH] = o[:, CH:].T
    return y


# revision 40
# speedup vs baseline: 195.2828x; 1.0064x over previous
"""Trainium2 distributed kernel for a dense transformer block (8 NeuronCores).

Sharding: tokens are data-parallel for LN/QKV/proj/MLP (512 tokens/core,
causal-balanced pairing: core i owns batch0 chunk i and batch1 chunk 7-i),
attention is head-parallel (2 heads/core) via an AllToAll exchange of
Q/K/V, plus a second AllToAll to bring attention outputs back to token
sharding.  All matmuls run in bf16 (f32 accumulation in PSUM); LayerNorm
statistics are computed with ones-vector matmuls so every activation
stays in transposed [d, token] layout on chip.

DMA strategy: weight/activation transfers are batched into wide tiles
(multi-block access patterns) to minimize HWDGE descriptor-queue
serialization; attention Q/K/V live in single wide SBUF tiles sliced
per-head/chunk, with batch1 slot order reversed so both batches index
chunks in ascending global order.
"""

import sys

sys.path.insert(0, "/opt/trn_rl_repo")

import numpy as np
import ml_dtypes

NCORES = 8
D = 1024
H = 16
DH = 64
HL = H // NCORES  # heads per core = 2
B = 2
S = 2048
T = 512  # tokens per core
CH = 256  # token chunk (half of T)
DFF = 4096
P = 128
QR, KR, VR = 128, 128, 130  # slot row counts: qT, kT, packed-v regions
SLOT = QR + KR + VR  # 386
EPS = 1e-5

_CACHE = {}
TRACE = False


def _emit_block(nc, tc, env, rep, x_tiles, x_all, collectives, write_out):
    """Emit one transformer block; returns the 8 output [128,T] f32 tiles."""
    from contextlib import ExitStack
    from concourse import bass, mybir

    f32 = mybir.dt.float32
    bf16 = mybir.dt.bfloat16
    f8 = mybir.dt.float8e4
    Alu = mybir.AluOpType
    AFT = mybir.ActivationFunctionType

    (xT, wT, wpT, wuT, wdT, out) = env["params"]
    (a1i, a1o, a2i, a2o) = env["bounce"]
    if not collectives:
        a1o, a2o = a1i, a2i
    c = env["consts"]
    pools = env["pools"]
    vec, recp = pools["vec"], pools["recp"]
    rg = [list(range(NCORES))]

    def ap3(t, part0, nprt, off, dims):
        """Custom free-dim AP on tile t at partition slice [part0, part0+nprt)."""
        base = t[:]
        pstride = base.ap[0][0]
        return bass.AP(base.tensor, base.offset + part0 * pstride + off,
                      [[pstride, nprt]] + dims)

    def ln_stats_half(x_tiles, psp, tmp_p, h, ps_sum, ps_sq):
        c0, c1 = h * CH, (h + 1) * CH
        for dk in range(8):
            nc.tensor.matmul(
                ps_sum[:, c0:c1], c["ones"][:], x_tiles[dk][:, c0:c1],
                start=(dk == 0), stop=(dk == 7),
            )
            sq = tmp_p.tile([P, CH], bf16, name="sq", tag="sq")
            nc.scalar.activation(sq[:], x_tiles[dk][:, c0:c1], AFT.Square)
            nc.tensor.matmul(
                ps_sq[:, c0:c1], c["ones"][:], sq[:],
                start=(dk == 0), stop=(dk == 7),
            )

    def ln_chain_half(st, h):
        """Per-half stats -> rstd/mur broadcast tiles (sbuf bf16)."""
        c0, c1 = h * CH, (h + 1) * CH
        ch, chb, psp, tmp_p, ps_sum, ps_sq = (
            st["ch"], st["chb"], st["psp"], st["tmp"], st["sum"], st["sq"])
        mu, msq, mu2, var, std, rstd, mur = (
            ch[0:1, i * T + c0 : i * T + c1] for i in range(7))
        rstd_c, mur_c = chb[0:1, c0:c1], chb[0:1, T + c0 : T + c1]
        nc.vector.tensor_scalar(mu, ps_sum[:, c0:c1], 1.0 / D, None, Alu.mult)
        nc.vector.tensor_scalar(msq, ps_sq[:, c0:c1], 1.0 / D, None, Alu.mult)
        nc.vector.tensor_tensor(mu2, mu, mu, Alu.mult)
        nc.vector.tensor_tensor(var, msq, mu2, Alu.subtract)
        nc.vector.tensor_scalar(var, var, EPS, None, Alu.add)
        nc.scalar.activation(std, var, AFT.Sqrt)
        nc.vector.reciprocal(rstd, std)
        nc.vector.tensor_tensor(mur, mu, rstd, Alu.mult)
        nc.vector.tensor_copy(rstd_c, rstd)
        nc.vector.tensor_copy(mur_c, mur)
        rstd_b = psp.tile([P, CH], f32, name="rstd_b", tag="rstd_b", bufs=1)
        nc.tensor.matmul(rstd_b[:], c["ones_row"][:], rstd_c, start=True, stop=True)
        mur_b = psp.tile([P, CH], f32, name="mur_b", tag="mur_b", bufs=1)
        nc.tensor.matmul(mur_b[:], c["ones_row"][:], mur_c, start=True, stop=True)
        rstd_bb = tmp_p.tile([P, CH], bf16, name="rstd_bb", tag="rstd_bb", bufs=2)
        nc.scalar.activation(rstd_bb[:], rstd_b[:], AFT.Copy)
        mur_bb = tmp_p.tile([P, CH], bf16, name="mur_bb", tag="mur_bb", bufs=2)
        nc.scalar.activation(mur_bb[:], mur_b[:], AFT.Copy)
        st[("bc", h)] = (rstd_bb, mur_bb)

    def ln_apply_half(st, h, x_tiles, g_tile, b_tile, out_pool, outs):
        c0, c1 = h * CH, (h + 1) * CH
        rstd_bb, mur_bb = st[("bc", h)]
        for dk in range(8):
            t1 = st["tmp"].tile([P, CH], bf16, name="lnt1", tag="lnt1")
            nc.vector.tensor_tensor(
                t1[:], x_tiles[dk][:, c0:c1], rstd_bb[:], Alu.mult)
            nc.vector.tensor_tensor(t1[:], t1[:], mur_bb[:], Alu.subtract)
            if h == 0:
                outs.append(out_pool.tile([P, T], bf16, name="ln_out",
                                          tag="ln_out"))
            nc.scalar.activation(
                outs[dk][:, c0:c1], t1[:], AFT.Identity,
                bias=b_tile[:, dk : dk + 1], scale=g_tile[:, dk : dk + 1],
            )
        return outs

    if x_tiles[0].dtype != bf16:
        conv = []
        for dk in range(8):
            xc = pools["ht"].tile([P, T], bf16, name="xc", tag="xc")
            nc.scalar.activation(xc[:], x_tiles[dk][:], AFT.Copy)
            conv.append(xc)
        x_tiles = conv

    # ---------------- LN1 (per-half so QKV(h0) starts after apply(h0)) ----
    ln1_stack = ExitStack()
    psp1 = ln1_stack.enter_context(
        tc.tile_pool(name=f"ln_psa{rep}", bufs=1, space="PSUM"))
    tmp1 = ln1_stack.enter_context(tc.tile_pool(name=f"ln_tmpa{rep}", bufs=3))
    st1 = {
        "psp": psp1, "tmp": tmp1,
        "sum": psp1.tile([1, T], f32, name="ps_sum", tag="ps_sum"),
        "sq": psp1.tile([1, T], f32, name="ps_sq", tag="ps_sq"),
        "ch": vec.tile([1, 8 * T], f32, name="lnchain", tag="lnchain"),
        "chb": vec.tile([1, 2 * T], bf16, name="lnchainb", tag="lnchainb"),
    }
    for h in range(2):
        ln_stats_half(x_tiles, psp1, tmp1, h, st1["sum"], st1["sq"])
        ln_chain_half(st1, h)
    h_tiles = []
    for h in range(2):
        ln_apply_half(st1, h, x_tiles, c["g1"], c["be1"], pools["ht"], h_tiles)
    ln1_stack.close()

    # ---------------- QKV (split by batch half for collective overlap) ------
    # Round A covers wT cols 0:1536 (q slots 0-7, k slots 0-3); round B covers
    # 1536:3072 (k slots 4-7 and all of v).  For each half h (b0 tokens =
    # cols 0:256, b1 = 256:512) all staging lands in a1i rows [h*8*SLOT, ...),
    # then the half's AllToAll fires while the other half computes.
    HT = NCORES * SLOT  # rows per half in a1i/a1o

    def qk_block(h, jts, wtiles, col0):
        """Emit psums + staged write for 4 consecutive q/k outputs, half h."""
        stg = pools["stg"].tile([P, 4 * CH], f8, name="stg", tag="stg")
        for i, jt in enumerate(jts):
            ps = env["qkps"].tile([P, CH], f32, name="qk_ps", tag="qk_ps")
            for dk in range(8):
                nc.tensor.matmul(
                    ps[:],
                    wtiles[dk][:, jt * P - col0 : (jt + 1) * P - col0],
                    h_tiles[dk][:, h * CH : (h + 1) * CH],
                    start=(dk == 0), stop=(dk == 7),
                )
            nc.scalar.activation(
                stg[:, i * CH : (i + 1) * CH], ps[:], AFT.Identity,
                bias=c["bqk"][:, jt : jt + 1],
            )
        jt0 = jts[0]
        base = (h * HT + (jt0 * SLOT if jt0 < 8 else (jt0 - 8) * SLOT + QR)) * CH
        dst = bass.AP(a1i, base, [[CH, P], [SLOT * CH, 4], [1, CH]])
        src = stg[:].rearrange("p (j t) -> p j t", t=CH)
        nc.sync.dma_start(dst, src)

    def emit_v(h, wB, vst_p, v_ps):
        # v: psum [tokens, vdims]; stage 4 slots (x2 heads+ones) per DMA
        for jc in range(2):
            for tt in range(2):
                ps = v_ps.tile([P, 512], f32, name="v_ps", tag="v_ps")
                for dk in range(8):
                    nc.tensor.matmul(
                        ps[:],
                        h_tiles[dk][:, h * CH + tt * P : h * CH + (tt + 1) * P],
                        wB[dk][:, 512 + jc * 512 : 1024 + jc * 512],
                        start=(dk == 0), stop=(dk == 7),
                    )
                vt = vst_p.tile([P, 4 * VR], f8, name="vst", tag="vst")
                for sl in range(4):
                    slot = jc * 4 + sl
                    for lh in range(HL):
                        nc.vector.tensor_tensor(
                            vt[:, sl * VR + lh * 65 : sl * VR + lh * 65 + DH],
                            ps[:, sl * P + lh * DH : sl * P + lh * DH + DH],
                            c["bv"][:, slot * P + lh * DH : slot * P + lh * DH + DH],
                            Alu.add,
                        )
                        nc.vector.memset(
                            vt[:, sl * VR + lh * 65 + DH : sl * VR + lh * 65 + DH + 1],
                            1.0,
                        )
                base = (h * HT + (jc * 4) * SLOT + QR + KR) * CH + tt * P * VR
                dst = bass.AP(a1i, base, [[VR, P], [SLOT * CH, 4], [1, VR]])
                src = vt[:].rearrange("p (s c) -> p s c", c=VR)
                nc.sync.dma_start(dst, src)

    def fire_a2a1(h):
        if collectives:
            nc.gpsimd.collective_compute(
                "AllToAll", mybir.AluOpType.bypass, replica_groups=rg,
                ins=[a1i[h * HT : (h + 1) * HT, :]],
                outs=[a1o[h * HT : (h + 1) * HT, :]],
            )

    with tc.tile_pool(name=f"wqkA{rep}", bufs=8) as wqk_a, tc.tile_pool(
        name=f"wqkB{rep}", bufs=8
    ) as wqk_b, tc.tile_pool(name=f"qk{rep}", bufs=4, space="PSUM") as qk_ps, tc.tile_pool(
        name=f"vst{rep}", bufs=2
    ) as vst_p, tc.tile_pool(name=f"v_ps{rep}", bufs=2, space="PSUM") as v_ps:
        env["qkps"] = qk_ps
        wA, wB = [], []
        for dk in range(8):
            wt = wqk_a.tile([P, 1536], bf16, name="wA", tag="wA")
            nc.sync.dma_start(wt[:], wT[dk * P : (dk + 1) * P, 0:1536])
            wA.append(wt)
        for dk in range(8):
            wt = wqk_b.tile([P, 1536], bf16, name="wB", tag="wB")
            nc.sync.dma_start(wt[:], wT[dk * P : (dk + 1) * P, 1536:3072])
            wB.append(wt)
        for h in range(2):
            for blk in range(3):
                qk_block(h, list(range(blk * 4, blk * 4 + 4)), wA, 0)
            qk_block(h, [12, 13, 14, 15], wB, 1536)
            emit_v(h, wB, vst_p, v_ps)
            fire_a2a1(h)

    # ---------------- attention (batch-outer; overlaps collectives) --------
    # Wide per-core tiles; batch0 chunks at ascending slot order, batch1
    # chunks stored slot-reversed so both batches index by global chunk id.
    a2_all = pools["a2stg"].tile([P, 8 * T], bf16, name="a2all", tag="a2all")
    HT2 = NCORES * QR  # rows per half in a2i/a2o

    wu_stack = ExitStack()
    wu_p = wu_stack.enter_context(tc.tile_pool(name=f"wu{rep}", bufs=9))
    wuA = []
    for dk in range(8):
        wt = wu_p.tile([P, 1024], bf16, name="wuA", tag="wuA", bufs=8)
        nc.sync.dma_start(wt[:], wuT[dk * P : (dk + 1) * P, 0:1024])
        wuA.append(wt)
    ap_stack = ExitStack()
    wp_p = ap_stack.enter_context(tc.tile_pool(name=f"wp{rep}", bufs=8))
    env["wp_pool"] = wp_p
    with tc.tile_pool(name=f"qkv{rep}", bufs=1) as qkv_p, tc.tile_pool(
        name=f"qe{rep}", bufs=4
    ) as qe_p, tc.tile_pool(name=f"s_ps{rep}", bufs=2, space="PSUM") as s_ps, tc.tile_pool(
        name=f"o_ps{rep}", bufs=2, space="PSUM"
    ) as o_ps:
        q_all = qkv_p.tile([P, 4096], f8, name="q_all", tag="q_all")
        k_all = qkv_p.tile([P, 4096], f8, name="k_all", tag="k_all")
        v_all = qkv_p.tile([P, 32 * VR], f8, name="v_all", tag="v_all")

        def load_attn(b):
            # load in the order attention consumes chunks: for b1 the chunk
            # index is 7-s, so iterate slots descending
            order = range(8) if b == 0 else reversed(range(8))
            for s in order:
                pos = s if b == 0 else 7 - s
                for (tile_, rowoff) in ((q_all, 0), (k_all, QR)):
                    src = bass.AP(
                        a1o, (b * HT + s * SLOT + rowoff) * CH, [[CH, P], [1, CH]]
                    )
                    nc.sync.dma_start(
                        tile_[:, b * 2048 + pos * CH : b * 2048 + (pos + 1) * CH], src
                    )
                vbase = (b * HT + s * SLOT + QR + KR) * CH
                src = bass.AP(a1o, vbase, [[VR, P], [P * VR, 2], [1, VR]])
                blk0 = (b * 16 + pos * 2) * VR
                nc.sync.dma_start(v_all[:, blk0 : blk0 + 2 * VR], src)

        load_attn(0)
        load_attn(1)  # waits on A2A1(1) via deps; overlaps b0 compute
        # prefetch proj weights during attention
        wpt = []
        for dk in range(8):
            wt = env["wp_pool"].tile([P, D], bf16, name="wp", tag="wp")
            nc.sync.dma_start(wt[:], wpT[dk * P : (dk + 1) * P, :])
            wpt.append(wt)
        env["wpt"] = wpt

        for b in range(B):
            for lh in range(HL):
                for pr in range(4):  # query-chunk pairs (2pr, 2pr+1)
                    q0, q1 = 2 * pr, 2 * pr + 1
                    s0 = q0 if b == 0 else 7 - q0
                    s1 = q1 if b == 0 else 7 - q1
                    qcol0 = b * CH
                    qmv = q_all[lh * DH : (lh + 1) * DH,
                                b * 2048 + q0 * CH : b * 2048 + (q1 + 1) * CH]
                    po = o_ps.tile([65, 2 * CH], f32, name="o_ps", tag="o_ps")
                    n_mm = 2 * (q1 + 1)
                    mi = 0
                    for kc in range(q1 + 1):
                        # kc == q1 blocks only contribute to the q1 (right)
                        # query half; the q0 half is fully above-diagonal.
                        narrow = kc == q1
                        qs = qmv[:, CH : 2 * CH] if narrow else qmv
                        W = CH if narrow else 2 * CH
                        # both k sub-chunks land in one 2-bank psum tile so a
                        # single Exp covers them
                        ps = s_ps.tile([P, 4 * CH], f32, name="s_ps", tag="s_ps")
                        E = qe_p.tile([P, 4 * CH], bf16, name="E", tag="E")
                        for sub in range(2):
                            nc.tensor.matmul(
                                ps[:, sub * W : (sub + 1) * W],
                                k_all[lh * DH : (lh + 1) * DH,
                                      b * 2048 + kc * CH + sub * P
                                      : b * 2048 + kc * CH + (sub + 1) * P],
                                qs,
                                start=True, stop=True,
                                skip_group_check=True,
                            )
                        nc.scalar.activation(
                            E[:, 0 : 2 * W], ps[:, 0 : 2 * W], AFT.Exp, scale=0.125
                        )
                        for sub in range(2):
                            Es = E[:, sub * W : (sub + 1) * W]
                            if kc == q0 and not narrow:  # diagonal for q0
                                nc.vector.tensor_tensor(
                                    Es, Es, c["tri_lo"][sub][:], Alu.mult
                                )
                            elif narrow:  # diagonal for q1
                                nc.vector.tensor_tensor(
                                    Es, Es, c["tri_hi"][sub][:, CH : 2 * CH],
                                    Alu.mult,
                                )
                            vblk = (b * 16 + kc * 2 + sub) * VR + lh * 65
                            nc.tensor.matmul(
                                po[:, 2 * CH - W : 2 * CH],
                                v_all[:, vblk : vblk + 65], Es,
                                start=(mi == 0), stop=(mi == n_mm - 1),
                                skip_group_check=True,
                            )
                            mi += 1
                    rec = recp.tile([1, 2 * CH], bf16, name="rec", tag="rec")
                    with nc.allow_low_precision(reason="softmax denom bcast"):
                        nc.vector.reciprocal(rec[:], po[64:65, :])
                    rec_ps = s_ps.tile(
                        [DH, 2 * CH], f32, name="rec_ps", tag="rec_ps", bufs=2
                    )
                    nc.tensor.matmul(
                        rec_ps[:], c["ones_row"][:, 0:DH], rec[:], start=True, stop=True
                    )
                    rec_b = recp.tile([DH, 2 * CH], f32, name="rec_b", tag="rec_b")
                    nc.vector.tensor_copy(rec_b[:], rec_ps[:])
                    for half, sq in ((0, s0), (1, s1)):
                        nc.vector.tensor_tensor(
                            a2_all[lh * DH : (lh + 1) * DH,
                                   sq * T + qcol0 : sq * T + qcol0 + CH],
                            po[0:DH, half * CH : (half + 1) * CH],
                            rec_b[:, half * CH : (half + 1) * CH],
                            Alu.mult,
                        )
            # half-b attention done: ship its outputs + fire its AllToAll
            dst = bass.AP(a2i, b * HT2 * CH, [[CH, P], [P * CH, 8], [1, CH]])
            src = bass.AP(
                a2_all[:].tensor, a2_all[:].offset + b * CH,
                [[a2_all[:].ap[0][0], P], [T, 8], [1, CH]],
            )
            nc.sync.dma_start(dst, src)
            if collectives:
                nc.gpsimd.collective_compute(
                    "AllToAll", mybir.AluOpType.bypass, replica_groups=rg,
                    ins=[a2i[b * HT2 : (b + 1) * HT2, :]],
                    outs=[a2o[b * HT2 : (b + 1) * HT2, :]],
                )

    # ---------------- proj + residual1 (split by half) ----------------
    # LN2 stats for each half are emitted right after that half's residual,
    # so they overlap the other half's AllToAll/proj.
    ln2_stack = ExitStack()
    psp2 = ln2_stack.enter_context(
        tc.tile_pool(name=f"ln_psb{rep}", bufs=1, space="PSUM"))
    tmp2 = ln2_stack.enter_context(tc.tile_pool(name=f"ln_tmpb{rep}", bufs=3))
    st2 = {
        "psp": psp2, "tmp": tmp2,
        "sum": psp2.tile([1, T], f32, name="ps_sum", tag="ps_sum"),
        "sq": psp2.tile([1, T], f32, name="ps_sq", tag="ps_sq"),
        "ch": vec.tile([1, 8 * T], f32, name="lnchain", tag="lnchain"),
        "chb": vec.tile([1, 2 * T], bf16, name="lnchainb", tag="lnchainb"),
    }
    ps_sum2, ps_sq2 = st2["sum"], st2["sq"]

    x1_tiles = []
    with tc.tile_pool(
        name=f"p_ps{rep}", bufs=2, space="PSUM"
    ) as p_ps, tc.tile_pool(name=f"otp{rep}", bufs=1) as ot_p:
        wpt = env["wpt"]
        ot_all = ot_p.tile([P, 8 * T], bf16, name="ot_all", tag="ot_all")
        for h in range(2):
            dst = bass.AP(
                ot_all[:].tensor, ot_all[:].offset + h * CH,
                [[ot_all[:].ap[0][0], P], [T, 8], [1, CH]],
            )
            nc.sync.dma_start(
                dst, bass.AP(a2o, h * HT2 * CH, [[CH, P], [P * CH, 8], [1, CH]])
            )
            for do in range(8):
                ps = p_ps.tile([P, CH], f32, name="p_ps", tag="p_ps")
                for dk in range(8):
                    nc.tensor.matmul(
                        ps[:],
                        wpt[dk][:, do * P : (do + 1) * P],
                        ot_all[:, dk * T + h * CH : dk * T + (h + 1) * CH],
                        start=(dk == 0), stop=False,
                    )
                # residual folded into the accumulation: psum += I @ x
                nc.tensor.matmul(
                    ps[:], c["ident"][:],
                    x_tiles[do][:, h * CH : (h + 1) * CH],
                    start=False, stop=True,
                )
                if h == 0:
                    x1 = pools["x1"].tile([P, T], bf16, name="x1", tag="x1")
                    x1_tiles.append(x1)
                x1 = x1_tiles[do]
                nc.scalar.activation(
                    x1[:, h * CH : (h + 1) * CH], ps[:], AFT.Identity,
                    bias=c["bp"][:, do : do + 1],
                )
            ln_stats_half(x1_tiles, psp2, tmp2, h, ps_sum2, ps_sq2)
            ln_chain_half(st2, h)
            if h == 0:
                # fill the A2A2(1) window: apply LN2 on half 0 and run a
                # half-width up-proj pass for the first 8 j blocks
                h2_tiles = []
                ln_apply_half(st2, 0, x1_tiles, c["g2"], c["be2"],
                              pools["h2"], h2_tiles)
                gu_part = []
                with tc.tile_pool(name=f"u_pre{rep}", bufs=2,
                                  space="PSUM") as upre_ps:
                    for j in range(8):
                        ps = upre_ps.tile([P, CH], f32, name="upre",
                                          tag="upre")
                        for dk in range(8):
                            nc.tensor.matmul(
                                ps[:], wuA[dk][:, j * P : (j + 1) * P],
                                h2_tiles[dk][:, 0:CH],
                                start=(dk == 0), stop=(dk == 7),
                            )
                        gu = pools["gu"].tile([P, T], bf16, name="gu",
                                              tag="gu")
                        nc.scalar.activation(
                            gu[:, 0:CH], ps[:], AFT.Gelu_apprx_tanh,
                            bias=c["bu"][:, j : j + 1],
                        )
                        gu_part.append(gu)

    # ---------------- LN2 apply (half 1) ----------------
    ln_apply_half(st2, 1, x1_tiles, c["g2"], c["be2"], pools["h2"], h2_tiles)
    ln2_stack.close()
    ap_stack.close()

    # ---------------- MLP up + gelu ----------------
    gu_tiles = list(gu_part)
    with tc.tile_pool(name=f"u_ps{rep}", bufs=3, space="PSUM") as u_ps:
        # finish j0..7: token half 1
        for j in range(8):
            ps = u_ps.tile([P, CH], f32, name="u_ps1", tag="u_ps1", bufs=3)
            for dk in range(8):
                nc.tensor.matmul(
                    ps[:], wuA[dk][:, j * P : (j + 1) * P],
                    h2_tiles[dk][:, CH:T],
                    start=(dk == 0), stop=(dk == 7),
                )
            nc.scalar.activation(
                gu_part[j][:, CH:T], ps[:], AFT.Gelu_apprx_tanh,
                bias=c["bu"][:, j : j + 1],
            )
        # j8..31 full width, weights loaded in [128,1024] groups
        for grp in range(1, 4):
            wut = []
            for dk in range(8):
                wt = wu_p.tile([P, 1024], bf16, name="wu", tag="wu")
                nc.sync.dma_start(
                    wt[:],
                    wuT[dk * P : (dk + 1) * P, grp * 1024 : (grp + 1) * 1024],
                )
                wut.append(wt)
            for jl in range(8):
                j = grp * 8 + jl
                ps = u_ps.tile([P, T], f32, name="u_ps", tag="u_ps")
                for dk in range(8):
                    nc.tensor.matmul(
                        ps[:], wut[dk][:, jl * P : (jl + 1) * P], h2_tiles[dk][:],
                        start=(dk == 0), stop=(dk == 7),
                    )
                gu = pools["gu"].tile([P, T], bf16, name="gu", tag="gu")
                nc.scalar.activation(
                    gu[:], ps[:], AFT.Gelu_apprx_tanh, bias=c["bu"][:, j : j + 1]
                )
                gu_tiles.append(gu)
    wu_stack.close()

    # ---------------- MLP down + residual2 ----------------
    out_tiles = []
    with tc.tile_pool(name=f"wd{rep}", bufs=6) as wd_p, tc.tile_pool(
        name=f"d_ps{rep}", bufs=1, space="PSUM"
    ) as d_ps:
        pss = [d_ps.tile([P, T], f32, name=f"d_ps{do}", tag=f"d_ps{do}")
               for do in range(8)]
        for j in range(32):
            wt = wd_p.tile([P, D], bf16, name="wd", tag="wd")
            nc.sync.dma_start(wt[:], wdT[j * P : (j + 1) * P, :])
            for do in range(8):
                nc.tensor.matmul(
                    pss[do][:], wt[:, do * P : (do + 1) * P], gu_tiles[j][:],
                    start=(j == 0), stop=False,
                )
        for do in range(8):
            nc.tensor.matmul(
                pss[do][:], c["ident"][:], x1_tiles[do][:],
                start=False, stop=True,
            )
            o = pools["outp"].tile([P, T], f32, name="out_t", tag="out_t")
            nc.scalar.activation(
                o[:], pss[do][:], AFT.Identity, bias=c["bd"][:, do : do + 1]
            )
            if write_out:
                nc.sync.dma_start(out[do * P : (do + 1) * P, :], o[:])
            out_tiles.append(o)
    return out_tiles


def _build(nreps=1, collectives=True, hw_loop=0):
    from contextlib import ExitStack
    from concourse import bass, mybir, tile, bacc

    f32 = mybir.dt.float32
    bf16 = mybir.dt.bfloat16

    nc = bacc.Bacc("TRN2", target_bir_lowering=False, num_devices=NCORES)

    xT = nc.declare_dram_parameter("xT", [D, T], bf16, isOutput=False)
    wT = nc.declare_dram_parameter("wT", [D, 3 * D], bf16, isOutput=False)
    wpT = nc.declare_dram_parameter("wpT", [D, D], bf16, isOutput=False)
    wuT = nc.declare_dram_parameter("wuT", [D, DFF], bf16, isOutput=False)
    wdT = nc.declare_dram_parameter("wdT", [DFF, D], bf16, isOutput=False)
    bqk = nc.declare_dram_parameter("bqk", [P, 16], f32, isOutput=False)
    bv = nc.declare_dram_parameter("bv", [P, D], f32, isOutput=False)
    bp = nc.declare_dram_parameter("bp", [P, 8], f32, isOutput=False)
    bu = nc.declare_dram_parameter("bu", [P, 32], f32, isOutput=False)
    bd = nc.declare_dram_parameter("bd", [P, 8], f32, isOutput=False)
    g1 = nc.declare_dram_parameter("g1", [P, 8], f32, isOutput=False)
    be1 = nc.declare_dram_parameter("be1", [P, 8], f32, isOutput=False)
    g2 = nc.declare_dram_parameter("g2", [P, 8], f32, isOutput=False)
    be2 = nc.declare_dram_parameter("be2", [P, 8], f32, isOutput=False)
    ident = nc.declare_dram_parameter("ident", [P, P], bf16, isOutput=False)
    tri = nc.declare_dram_parameter("tri", [CH, CH], bf16, isOutput=False)
    tri2 = nc.declare_dram_parameter("tri2", [CH, 4 * CH], bf16, isOutput=False)
    out = nc.declare_dram_parameter("out", [D, T], f32, isOutput=True)

    # Half-split bounce buffers: rows [h*8*SLOT, (h+1)*8*SLOT) hold batch-half
    # h (256 token cols) so each half's AllToAll is a contiguous slab.
    f8 = mybir.dt.float8e4
    a1i = nc.dram_tensor("a2a1_in", [2 * NCORES * SLOT, CH], f8)
    a1o = nc.dram_tensor("a2a1_out", [2 * NCORES * SLOT, CH], f8)
    a2i = nc.dram_tensor("a2a2_in", [2 * NCORES * QR, CH], bf16)
    a2o = nc.dram_tensor("a2a2_out", [2 * NCORES * QR, CH], bf16)

    with tile.TileContext(nc) as tc, ExitStack() as top:
        xt_pool = top.enter_context(tc.tile_pool(name="xt", bufs=8))
        x_tiles = []
        for dk in range(8):
            xt = xt_pool.tile([P, T], bf16, name="xt", tag="xt")
            nc.sync.dma_start(xt[:], xT[dk * P : (dk + 1) * P, :])
            x_tiles.append(xt)
        const = top.enter_context(tc.tile_pool(name="const", bufs=1))
        ones = const.tile([P, 1], bf16)
        nc.vector.memset(ones[:], 1.0)
        ones_f = const.tile([P, 1], f32)
        nc.vector.memset(ones_f[:], 1.0)
        ones_row = const.tile([1, P], bf16)
        nc.vector.memset(ones_row[:], 1.0)
        ident_t = const.tile([P, P], bf16, name="ident_t", tag="ident_t")
        nc.sync.dma_start(ident_t[:], ident[:, :])
        tri_t = [const.tile([P, CH], bf16, name=f"tri{s}", tag=f"tri{s}") for s in range(2)]
        for s in range(2):
            nc.sync.dma_start(tri_t[s][:], tri[s * P : (s + 1) * P, :])
        tri2_t = [const.tile([P, 4 * CH], bf16, name=f"tri2{s}", tag=f"tri2{s}") for s in range(2)]
        for s in range(2):
            nc.sync.dma_start(tri2_t[s][:], tri2[s * P : (s + 1) * P, :])

        def ctile(name, param, shape):
            t = const.tile(shape, f32, name=name, tag=name)
            nc.sync.dma_start(t[:], param[:, :])
            return t

        consts = {
            "ones": ones, "ones_f": ones_f, "ones_row": ones_row, "tri": tri_t,
            "ident": ident_t,
            "tri_lo": [tri2_t[s][:, 0 : 2 * CH] for s in range(2)],
            "tri_hi": [tri2_t[s][:, 2 * CH : 4 * CH] for s in range(2)],
            "bqk": ctile("bqk_t", bqk, [P, 16]),
            "bv": ctile("bv_t", bv, [P, D]),
            "bp": ctile("bp_t", bp, [P, 8]),
            "bu": ctile("bu_t", bu, [P, 32]),
            "bd": ctile("bd_t", bd, [P, 8]),
            "g1": ctile("g1_t", g1, [P, 8]),
            "be1": ctile("be1_t", be1, [P, 8]),
            "g2": ctile("g2_t", g2, [P, 8]),
            "be2": ctile("be2_t", be2, [P, 8]),
        }

        pools = {
            "vec": top.enter_context(tc.tile_pool(name="vec", bufs=1)),
            "recp": top.enter_context(tc.tile_pool(name="recp", bufs=2)),
            "ht": top.enter_context(tc.tile_pool(name="ht", bufs=8)),
            "stg": top.enter_context(tc.tile_pool(name="stg", bufs=2)),
            "a2stg": top.enter_context(tc.tile_pool(name="a2stg", bufs=1)),
            "ot": top.enter_context(tc.tile_pool(name="ot", bufs=1)),
            "x1": top.enter_context(tc.tile_pool(name="x1", bufs=8)),
            "h2": top.enter_context(tc.tile_pool(name="h2", bufs=8)),
            "gu": top.enter_context(tc.tile_pool(name="gu", bufs=32)),
            "outp": top.enter_context(tc.tile_pool(name="outp", bufs=8)),
        }

        env = {
            "params": (xT, wT, wpT, wuT, wdT, out),
            "bounce": (a1i, a1o, a2i, a2o),
            "consts": consts,
            "pools": pools,
        }

        if hw_loop:
            with tc.For_i(0, hw_loop):
                _emit_block(nc, tc, env, 0, x_tiles, None, collectives, write_out=True)
        else:
            cur = x_tiles
            for rep in range(nreps):
                cur = _emit_block(
                    nc, tc, env, rep, cur, None, collectives,
                    write_out=(rep == nreps - 1),
                )

    nc.finalize()
    return nc


def _get_nc():
    if "nc" not in _CACHE:
        _CACHE["nc"] = _build()
    return _CACHE["nc"]


def _make_in_maps(inputs):
    x = np.asarray(inputs["x"], np.float32)
    ln1_g = np.asarray(inputs["ln1_g"], np.float32)
    ln1_b = np.asarray(inputs["ln1_b"], np.float32)
    W_attn = np.asarray(inputs["W_attn"], np.float32)
    b_attn = np.asarray(inputs["b_attn"], np.float32)
    W_proj = np.asarray(inputs["W_proj"], np.float32)
    b_proj = np.asarray(inputs["b_proj"], np.float32)
    ln2_g = np.asarray(inputs["ln2_g"], np.float32)
    ln2_b = np.asarray(inputs["ln2_b"], np.float32)
    W_up = np.asarray(inputs["W_up"], np.float32)
    b_up = np.asarray(inputs["b_up"], np.float32)
    W_down = np.asarray(inputs["W_down"], np.float32)
    b_down = np.asarray(inputs["b_down"], np.float32)

    bf = ml_dtypes.bfloat16
    wT = np.ascontiguousarray(W_attn.T).astype(bf)
    wpT = np.ascontiguousarray(W_proj.T).astype(bf)
    wuT = np.ascontiguousarray(W_up.T).astype(bf)
    wdT = np.ascontiguousarray(W_down.T).astype(bf)

    def cols(v):  # [N] -> [128, N//128]: col j = v[j*128:(j+1)*128]
        return np.ascontiguousarray(v.reshape(-1, P).T).astype(np.float32)

    tri = np.tril(np.ones((CH, CH), np.float32)).T.astype(bf)  # tri[a,b] = a<=b
    tri = np.ascontiguousarray(tri)

    ones_m = np.ones((CH, CH), np.float32)
    zeros_m = np.zeros((CH, CH), np.float32)
    tri_f = np.tril(np.ones((CH, CH), np.float32)).T
    tri2 = np.ascontiguousarray(
        np.concatenate([tri_f, ones_m, zeros_m, tri_f], axis=1)
    ).astype(bf)

    common = dict(
        wT=wT, wpT=wpT, wuT=wuT, wdT=wdT, tri2=tri2,
        ident=np.eye(P, dtype=bf),
        bqk=cols(b_attn[: 2 * D]),
        bv=np.ascontiguousarray(np.broadcast_to(b_attn[2 * D :].reshape(1, D), (P, D))),
        bp=cols(b_proj), bu=cols(b_up), bd=cols(b_down),
        g1=cols(ln1_g), be1=cols(ln1_b), g2=cols(ln2_g), be2=cols(ln2_b),
        tri=tri,
    )

    in_maps = []
    for i in range(NCORES):
        c0 = x[0, i * CH : (i + 1) * CH]  # [256, 1024]
        c1 = x[1, (7 - i) * CH : (8 - i) * CH]
        xTi = np.ascontiguousarray(np.concatenate([c0, c1], 0).T).astype(bf)
        in_maps.append(dict(common, xT=xTi))
    return in_maps


def make_in_maps(inputs):
    return _make_in_maps(inputs)


def kernel(**inputs):
    in_maps = _make_in_maps(inputs)

    from concourse import bass_utils

    nc = _get_nc()
    res = bass_utils.run_bass_kernel_spmd(
        nc, in_maps, core_ids=list(range(NCORES)), trace=TRACE
    )
    _CACHE["last_res"] = res
    y = np.empty((B, S, D), np.float32)
    for i in range(NCORES):
        o = np.asarray(res.results[i]["out"], np.float32)  # [1024, 512]
        y[0, i * CH : (i + 1) * CH] = o[:, :CH].T
        y[1, (7 - i) * CH : (8 - i) * CH] = o[:, CH:].T
    return y


# revision 41
# speedup vs baseline: 195.7096x; 1.0022x over previous
"""Trainium2 distributed kernel for a dense transformer block (8 NeuronCores).

Sharding: tokens are data-parallel for LN/QKV/proj/MLP (512 tokens/core,
causal-balanced pairing: core i owns batch0 chunk i and batch1 chunk 7-i),
attention is head-parallel (2 heads/core) via an AllToAll exchange of
Q/K/V, plus a second AllToAll to bring attention outputs back to token
sharding.  All matmuls run in bf16 (f32 accumulation in PSUM); LayerNorm
statistics are computed with ones-vector matmuls so every activation
stays in transposed [d, token] layout on chip.

DMA strategy: weight/activation transfers are batched into wide tiles
(multi-block access patterns) to minimize HWDGE descriptor-queue
serialization; attention Q/K/V live in single wide SBUF tiles sliced
per-head/chunk, with batch1 slot order reversed so both batches index
chunks in ascending global order.
"""

import sys

sys.path.insert(0, "/opt/trn_rl_repo")

import numpy as np
import ml_dtypes

NCORES = 8
D = 1024
H = 16
DH = 64
HL = H // NCORES  # heads per core = 2
B = 2
S = 2048
T = 512  # tokens per core
CH = 256  # token chunk (half of T)
DFF = 4096
P = 128
QR, KR, VR = 128, 128, 130  # slot row counts: qT, kT, packed-v regions
SLOT = QR + KR + VR  # 386
EPS = 1e-5

_CACHE = {}
TRACE = False


def _emit_block(nc, tc, env, rep, x_tiles, x_all, collectives, write_out):
    """Emit one transformer block; returns the 8 output [128,T] f32 tiles."""
    from contextlib import ExitStack
    from concourse import bass, mybir

    f32 = mybir.dt.float32
    bf16 = mybir.dt.bfloat16
    f8 = mybir.dt.float8e4
    Alu = mybir.AluOpType
    AFT = mybir.ActivationFunctionType

    (xT, wT, wpT, wuT, wdT, out) = env["params"]
    (a1i, a1o, a2i, a2o) = env["bounce"]
    if not collectives:
        a1o, a2o = a1i, a2i
    c = env["consts"]
    pools = env["pools"]
    vec, recp = pools["vec"], pools["recp"]
    rg = [list(range(NCORES))]

    def ap3(t, part0, nprt, off, dims):
        """Custom free-dim AP on tile t at partition slice [part0, part0+nprt)."""
        base = t[:]
        pstride = base.ap[0][0]
        return bass.AP(base.tensor, base.offset + part0 * pstride + off,
                      [[pstride, nprt]] + dims)

    def ln_stats_half(x_tiles, psp, tmp_p, h, ps_sum, ps_sq):
        c0, c1 = h * CH, (h + 1) * CH
        for dk in range(8):
            nc.tensor.matmul(
                ps_sum[:, c0:c1], c["ones"][:], x_tiles[dk][:, c0:c1],
                start=(dk == 0), stop=(dk == 7),
            )
            sq = tmp_p.tile([P, CH], bf16, name="sq", tag="sq")
            nc.scalar.activation(sq[:], x_tiles[dk][:, c0:c1], AFT.Square)
            nc.tensor.matmul(
                ps_sq[:, c0:c1], c["ones"][:], sq[:],
                start=(dk == 0), stop=(dk == 7),
            )

    def ln_chain_half(st, h):
        """Per-half stats -> rstd/mur broadcast tiles (sbuf bf16)."""
        c0, c1 = h * CH, (h + 1) * CH
        ch, chb, psp, tmp_p, ps_sum, ps_sq = (
            st["ch"], st["chb"], st["psp"], st["tmp"], st["sum"], st["sq"])
        mu, msq, mu2, var, std, rstd, mur = (
            ch[0:1, i * T + c0 : i * T + c1] for i in range(7))
        rstd_c, mur_c = chb[0:1, c0:c1], chb[0:1, T + c0 : T + c1]
        nc.vector.tensor_scalar(mu, ps_sum[:, c0:c1], 1.0 / D, None, Alu.mult)
        nc.vector.tensor_scalar(msq, ps_sq[:, c0:c1], 1.0 / D, None, Alu.mult)
        nc.vector.tensor_tensor(mu2, mu, mu, Alu.mult)
        nc.vector.tensor_tensor(var, msq, mu2, Alu.subtract)
        nc.vector.tensor_scalar(var, var, EPS, None, Alu.add)
        nc.scalar.activation(std, var, AFT.Sqrt)
        nc.vector.reciprocal(rstd, std)
        nc.vector.tensor_tensor(mur, mu, rstd, Alu.mult)
        nc.vector.tensor_copy(rstd_c, rstd)
        nc.vector.tensor_copy(mur_c, mur)
        rstd_b = psp.tile([P, CH], f32, name="rstd_b", tag="rstd_b", bufs=1)
        nc.tensor.matmul(rstd_b[:], c["ones_row"][:], rstd_c, start=True, stop=True)
        mur_b = psp.tile([P, CH], f32, name="mur_b", tag="mur_b", bufs=1)
        nc.tensor.matmul(mur_b[:], c["ones_row"][:], mur_c, start=True, stop=True)
        rstd_bb = tmp_p.tile([P, CH], bf16, name="rstd_bb", tag="rstd_bb", bufs=2)
        nc.scalar.activation(rstd_bb[:], rstd_b[:], AFT.Copy)
        mur_bb = tmp_p.tile([P, CH], bf16, name="mur_bb", tag="mur_bb", bufs=2)
        nc.scalar.activation(mur_bb[:], mur_b[:], AFT.Copy)
        st[("bc", h)] = (rstd_bb, mur_bb)

    def ln_apply_half(st, h, x_tiles, g_tile, b_tile, out_pool, outs):
        c0, c1 = h * CH, (h + 1) * CH
        rstd_bb, mur_bb = st[("bc", h)]
        for dk in range(8):
            t1 = st["tmp"].tile([P, CH], bf16, name="lnt1", tag="lnt1")
            nc.vector.tensor_tensor(
                t1[:], x_tiles[dk][:, c0:c1], rstd_bb[:], Alu.mult)
            nc.vector.tensor_tensor(t1[:], t1[:], mur_bb[:], Alu.subtract)
            if h == 0:
                outs.append(out_pool.tile([P, T], bf16, name="ln_out",
                                          tag="ln_out"))
            nc.scalar.activation(
                outs[dk][:, c0:c1], t1[:], AFT.Identity,
                bias=b_tile[:, dk : dk + 1], scale=g_tile[:, dk : dk + 1],
            )
        return outs

    if x_tiles[0].dtype != bf16:
        conv = []
        for dk in range(8):
            xc = pools["ht"].tile([P, T], bf16, name="xc", tag="xc")
            nc.scalar.activation(xc[:], x_tiles[dk][:], AFT.Copy)
            conv.append(xc)
        x_tiles = conv

    # ---------------- LN1 (per-half so QKV(h0) starts after apply(h0)) ----
    ln1_stack = ExitStack()
    psp1 = ln1_stack.enter_context(
        tc.tile_pool(name=f"ln_psa{rep}", bufs=1, space="PSUM"))
    tmp1 = ln1_stack.enter_context(tc.tile_pool(name=f"ln_tmpa{rep}", bufs=3))
    st1 = {
        "psp": psp1, "tmp": tmp1,
        "sum": psp1.tile([1, T], f32, name="ps_sum", tag="ps_sum"),
        "sq": psp1.tile([1, T], f32, name="ps_sq", tag="ps_sq"),
        "ch": vec.tile([1, 8 * T], f32, name="lnchain", tag="lnchain"),
        "chb": vec.tile([1, 2 * T], bf16, name="lnchainb", tag="lnchainb"),
    }
    for h in range(2):
        ln_stats_half(x_tiles, psp1, tmp1, h, st1["sum"], st1["sq"])
        ln_chain_half(st1, h)
    h_tiles = []
    for h in range(2):
        ln_apply_half(st1, h, x_tiles, c["g1"], c["be1"], pools["ht"], h_tiles)
    ln1_stack.close()

    # ---------------- QKV (split by batch half for collective overlap) ------
    # Round A covers wT cols 0:1536 (q slots 0-7, k slots 0-3); round B covers
    # 1536:3072 (k slots 4-7 and all of v).  For each half h (b0 tokens =
    # cols 0:256, b1 = 256:512) all staging lands in a1i rows [h*8*SLOT, ...),
    # then the half's AllToAll fires while the other half computes.
    HT = NCORES * SLOT  # rows per half in a1i/a1o

    def qk_block(h, jts, wtiles, col0):
        """Emit psums + staged write for 4 consecutive q/k outputs, half h."""
        stg = pools["stg"].tile([P, 4 * CH], f8, name="stg", tag="stg")
        for i, jt in enumerate(jts):
            ps = env["qkps"].tile([P, CH], f32, name="qk_ps", tag="qk_ps")
            for dk in range(8):
                nc.tensor.matmul(
                    ps[:],
                    wtiles[dk][:, jt * P - col0 : (jt + 1) * P - col0],
                    h_tiles[dk][:, h * CH : (h + 1) * CH],
                    start=(dk == 0), stop=(dk == 7),
                )
            nc.scalar.activation(
                stg[:, i * CH : (i + 1) * CH], ps[:], AFT.Identity,
                bias=c["bqk"][:, jt : jt + 1],
            )
        jt0 = jts[0]
        base = (h * HT + (jt0 * SLOT if jt0 < 8 else (jt0 - 8) * SLOT + QR)) * CH
        dst = bass.AP(a1i, base, [[CH, P], [SLOT * CH, 4], [1, CH]])
        src = stg[:].rearrange("p (j t) -> p j t", t=CH)
        nc.sync.dma_start(dst, src)

    def emit_v(h, wB, vst_p, v_ps):
        # v: psum [tokens, vdims]; stage 4 slots (x2 heads+ones) per DMA
        for jc in range(2):
            for tt in range(2):
                ps = v_ps.tile([P, 512], f32, name="v_ps", tag="v_ps")
                for dk in range(8):
                    nc.tensor.matmul(
                        ps[:],
                        h_tiles[dk][:, h * CH + tt * P : h * CH + (tt + 1) * P],
                        wB[dk][:, 512 + jc * 512 : 1024 + jc * 512],
                        start=(dk == 0), stop=(dk == 7),
                    )
                vt = vst_p.tile([P, 4 * VR], f8, name="vst", tag="vst")
                for sl in range(4):
                    slot = jc * 4 + sl
                    for lh in range(HL):
                        nc.vector.tensor_tensor(
                            vt[:, sl * VR + lh * 65 : sl * VR + lh * 65 + DH],
                            ps[:, sl * P + lh * DH : sl * P + lh * DH + DH],
                            c["bv"][:, slot * P + lh * DH : slot * P + lh * DH + DH],
                            Alu.add,
                        )
                        nc.vector.memset(
                            vt[:, sl * VR + lh * 65 + DH : sl * VR + lh * 65 + DH + 1],
                            1.0,
                        )
                base = (h * HT + (jc * 4) * SLOT + QR + KR) * CH + tt * P * VR
                dst = bass.AP(a1i, base, [[VR, P], [SLOT * CH, 4], [1, VR]])
                src = vt[:].rearrange("p (s c) -> p s c", c=VR)
                nc.sync.dma_start(dst, src)

    def fire_a2a1(h):
        if collectives:
            nc.gpsimd.collective_compute(
                "AllToAll", mybir.AluOpType.bypass, replica_groups=rg,
                ins=[a1i[h * HT : (h + 1) * HT, :]],
                outs=[a1o[h * HT : (h + 1) * HT, :]],
            )

    with tc.tile_pool(name=f"wqkA{rep}", bufs=8) as wqk_a, tc.tile_pool(
        name=f"wqkB{rep}", bufs=8
    ) as wqk_b, tc.tile_pool(name=f"qk{rep}", bufs=4, space="PSUM") as qk_ps, tc.tile_pool(
        name=f"vst{rep}", bufs=2
    ) as vst_p, tc.tile_pool(name=f"v_ps{rep}", bufs=2, space="PSUM") as v_ps:
        env["qkps"] = qk_ps
        wA, wB = [], []
        for dk in range(8):
            wt = wqk_a.tile([P, 1536], bf16, name="wA", tag="wA")
            nc.sync.dma_start(wt[:], wT[dk * P : (dk + 1) * P, 0:1536])
            wA.append(wt)
        for dk in range(8):
            wt = wqk_b.tile([P, 1536], bf16, name="wB", tag="wB")
            nc.sync.dma_start(wt[:], wT[dk * P : (dk + 1) * P, 1536:3072])
            wB.append(wt)
        for h in range(2):
            for blk in range(3):
                qk_block(h, list(range(blk * 4, blk * 4 + 4)), wA, 0)
            qk_block(h, [12, 13, 14, 15], wB, 1536)
            emit_v(h, wB, vst_p, v_ps)
            fire_a2a1(h)
            if h == 0:
                # warm the Exp activation table off the critical path: the
                # half-1 staging it delays has slack behind AllToAll(0)
                warm = vec.tile([1, 2], bf16, name="expwarm", tag="expwarm")
                nc.vector.memset(warm[:], 0.0)
                nc.scalar.activation(warm[:], warm[:], AFT.Exp)

    # ---------------- attention (batch-outer; overlaps collectives) --------
    # Wide per-core tiles; batch0 chunks at ascending slot order, batch1
    # chunks stored slot-reversed so both batches index by global chunk id.
    a2_all = pools["a2stg"].tile([P, 8 * T], bf16, name="a2all", tag="a2all")
    HT2 = NCORES * QR  # rows per half in a2i/a2o

    wu_stack = ExitStack()
    wu_p = wu_stack.enter_context(tc.tile_pool(name=f"wu{rep}", bufs=9))
    wuA = []
    for dk in range(8):
        wt = wu_p.tile([P, 1024], bf16, name="wuA", tag="wuA", bufs=8)
        nc.sync.dma_start(wt[:], wuT[dk * P : (dk + 1) * P, 0:1024])
        wuA.append(wt)
    ap_stack = ExitStack()
    wp_p = ap_stack.enter_context(tc.tile_pool(name=f"wp{rep}", bufs=8))
    env["wp_pool"] = wp_p
    with tc.tile_pool(name=f"qkv{rep}", bufs=1) as qkv_p, tc.tile_pool(
        name=f"qe{rep}", bufs=4
    ) as qe_p, tc.tile_pool(name=f"s_ps{rep}", bufs=2, space="PSUM") as s_ps, tc.tile_pool(
        name=f"o_ps{rep}", bufs=2, space="PSUM"
    ) as o_ps:
        q_all = qkv_p.tile([P, 4096], f8, name="q_all", tag="q_all")
        k_all = qkv_p.tile([P, 4096], f8, name="k_all", tag="k_all")
        v_all = qkv_p.tile([P, 32 * VR], f8, name="v_all", tag="v_all")

        def load_attn(b):
            # load in the order attention consumes chunks: for b1 the chunk
            # index is 7-s, so iterate slots descending
            order = range(8) if b == 0 else reversed(range(8))
            for s in order:
                pos = s if b == 0 else 7 - s
                for (tile_, rowoff) in ((q_all, 0), (k_all, QR)):
                    src = bass.AP(
                        a1o, (b * HT + s * SLOT + rowoff) * CH, [[CH, P], [1, CH]]
                    )
                    nc.sync.dma_start(
                        tile_[:, b * 2048 + pos * CH : b * 2048 + (pos + 1) * CH], src
                    )
                vbase = (b * HT + s * SLOT + QR + KR) * CH
                src = bass.AP(a1o, vbase, [[VR, P], [P * VR, 2], [1, VR]])
                blk0 = (b * 16 + pos * 2) * VR
                nc.sync.dma_start(v_all[:, blk0 : blk0 + 2 * VR], src)

        load_attn(0)
        load_attn(1)  # waits on A2A1(1) via deps; overlaps b0 compute
        # prefetch proj weights during attention
        wpt = []
        for dk in range(8):
            wt = env["wp_pool"].tile([P, D], bf16, name="wp", tag="wp")
            nc.sync.dma_start(wt[:], wpT[dk * P : (dk + 1) * P, :])
            wpt.append(wt)
        env["wpt"] = wpt

        for b in range(B):
            for lh in range(HL):
                for pr in range(4):  # query-chunk pairs (2pr, 2pr+1)
                    q0, q1 = 2 * pr, 2 * pr + 1
                    s0 = q0 if b == 0 else 7 - q0
                    s1 = q1 if b == 0 else 7 - q1
                    qcol0 = b * CH
                    qmv = q_all[lh * DH : (lh + 1) * DH,
                                b * 2048 + q0 * CH : b * 2048 + (q1 + 1) * CH]
                    po = o_ps.tile([65, 2 * CH], f32, name="o_ps", tag="o_ps")
                    n_mm = 2 * (q1 + 1)
                    mi = 0
                    for kc in range(q1 + 1):
                        # kc == q1 blocks only contribute to the q1 (right)
                        # query half; the q0 half is fully above-diagonal.
                        narrow = kc == q1
                        qs = qmv[:, CH : 2 * CH] if narrow else qmv
                        W = CH if narrow else 2 * CH
                        # both k sub-chunks land in one 2-bank psum tile so a
                        # single Exp covers them
                        ps = s_ps.tile([P, 4 * CH], f32, name="s_ps", tag="s_ps")
                        E = qe_p.tile([P, 4 * CH], bf16, name="E", tag="E")
                        for sub in range(2):
                            nc.tensor.matmul(
                                ps[:, sub * W : (sub + 1) * W],
                                k_all[lh * DH : (lh + 1) * DH,
                                      b * 2048 + kc * CH + sub * P
                                      : b * 2048 + kc * CH + (sub + 1) * P],
                                qs,
                                start=True, stop=True,
                                skip_group_check=True,
                            )
                        nc.scalar.activation(
                            E[:, 0 : 2 * W], ps[:, 0 : 2 * W], AFT.Exp, scale=0.125
                        )
                        for sub in range(2):
                            Es = E[:, sub * W : (sub + 1) * W]
                            if kc == q0 and not narrow:  # diagonal for q0
                                nc.vector.tensor_tensor(
                                    Es, Es, c["tri_lo"][sub][:], Alu.mult
                                )
                            elif narrow:  # diagonal for q1
                                nc.vector.tensor_tensor(
                                    Es, Es, c["tri_hi"][sub][:, CH : 2 * CH],
                                    Alu.mult,
                                )
                            vblk = (b * 16 + kc * 2 + sub) * VR + lh * 65
                            nc.tensor.matmul(
                                po[:, 2 * CH - W : 2 * CH],
                                v_all[:, vblk : vblk + 65], Es,
                                start=(mi == 0), stop=(mi == n_mm - 1),
                                skip_group_check=True,
                            )
                            mi += 1
                    rec = recp.tile([1, 2 * CH], bf16, name="rec", tag="rec")
                    with nc.allow_low_precision(reason="softmax denom bcast"):
                        nc.vector.reciprocal(rec[:], po[64:65, :])
                    rec_ps = s_ps.tile(
                        [DH, 2 * CH], f32, name="rec_ps", tag="rec_ps", bufs=2
                    )
                    nc.tensor.matmul(
                        rec_ps[:], c["ones_row"][:, 0:DH], rec[:], start=True, stop=True
                    )
                    rec_b = recp.tile([DH, 2 * CH], f32, name="rec_b", tag="rec_b")
                    nc.vector.tensor_copy(rec_b[:], rec_ps[:])
                    for half, sq in ((0, s0), (1, s1)):
                        nc.vector.tensor_tensor(
                            a2_all[lh * DH : (lh + 1) * DH,
                                   sq * T + qcol0 : sq * T + qcol0 + CH],
                            po[0:DH, half * CH : (half + 1) * CH],
                            rec_b[:, half * CH : (half + 1) * CH],
                            Alu.mult,
                        )
            # half-b attention done: ship its outputs + fire its AllToAll
            dst = bass.AP(a2i, b * HT2 * CH, [[CH, P], [P * CH, 8], [1, CH]])
            src = bass.AP(
                a2_all[:].tensor, a2_all[:].offset + b * CH,
                [[a2_all[:].ap[0][0], P], [T, 8], [1, CH]],
            )
            nc.sync.dma_start(dst, src)
            if collectives:
                nc.gpsimd.collective_compute(
                    "AllToAll", mybir.AluOpType.bypass, replica_groups=rg,
                    ins=[a2i[b * HT2 : (b + 1) * HT2, :]],
                    outs=[a2o[b * HT2 : (b + 1) * HT2, :]],
                )

    # ---------------- proj + residual1 (split by half) ----------------
    # LN2 stats for each half are emitted right after that half's residual,
    # so they overlap the other half's AllToAll/proj.
    ln2_stack = ExitStack()
    psp2 = ln2_stack.enter_context(
        tc.tile_pool(name=f"ln_psb{rep}", bufs=1, space="PSUM"))
    tmp2 = ln2_stack.enter_context(tc.tile_pool(name=f"ln_tmpb{rep}", bufs=3))
    st2 = {
        "psp": psp2, "tmp": tmp2,
        "sum": psp2.tile([1, T], f32, name="ps_sum", tag="ps_sum"),
        "sq": psp2.tile([1, T], f32, name="ps_sq", tag="ps_sq"),
        "ch": vec.tile([1, 8 * T], f32, name="lnchain", tag="lnchain"),
        "chb": vec.tile([1, 2 * T], bf16, name="lnchainb", tag="lnchainb"),
    }
    ps_sum2, ps_sq2 = st2["sum"], st2["sq"]

    x1_tiles = []
    with tc.tile_pool(
        name=f"p_ps{rep}", bufs=2, space="PSUM"
    ) as p_ps, tc.tile_pool(name=f"otp{rep}", bufs=1) as ot_p:
        wpt = env["wpt"]
        ot_all = ot_p.tile([P, 8 * T], bf16, name="ot_all", tag="ot_all")
        for h in range(2):
            dst = bass.AP(
                ot_all[:].tensor, ot_all[:].offset + h * CH,
                [[ot_all[:].ap[0][0], P], [T, 8], [1, CH]],
            )
            nc.sync.dma_start(
                dst, bass.AP(a2o, h * HT2 * CH, [[CH, P], [P * CH, 8], [1, CH]])
            )
            for do in range(8):
                ps = p_ps.tile([P, CH], f32, name="p_ps", tag="p_ps")
                for dk in range(8):
                    nc.tensor.matmul(
                        ps[:],
                        wpt[dk][:, do * P : (do + 1) * P],
                        ot_all[:, dk * T + h * CH : dk * T + (h + 1) * CH],
                        start=(dk == 0), stop=False,
                    )
                # residual folded into the accumulation: psum += I @ x
                nc.tensor.matmul(
                    ps[:], c["ident"][:],
                    x_tiles[do][:, h * CH : (h + 1) * CH],
                    start=False, stop=True,
                )
                if h == 0:
                    x1 = pools["x1"].tile([P, T], bf16, name="x1", tag="x1")
                    x1_tiles.append(x1)
                x1 = x1_tiles[do]
                nc.scalar.activation(
                    x1[:, h * CH : (h + 1) * CH], ps[:], AFT.Identity,
                    bias=c["bp"][:, do : do + 1],
                )
            ln_stats_half(x1_tiles, psp2, tmp2, h, ps_sum2, ps_sq2)
            ln_chain_half(st2, h)
            if h == 0:
                # fill the A2A2(1) window: apply LN2 on half 0 and run a
                # half-width up-proj pass for the first 8 j blocks
                h2_tiles = []
                ln_apply_half(st2, 0, x1_tiles, c["g2"], c["be2"],
                              pools["h2"], h2_tiles)
                gu_part = []
                with tc.tile_pool(name=f"u_pre{rep}", bufs=2,
                                  space="PSUM") as upre_ps:
                    for j in range(8):
                        ps = upre_ps.tile([P, CH], f32, name="upre",
                                          tag="upre")
                        for dk in range(8):
                            nc.tensor.matmul(
                                ps[:], wuA[dk][:, j * P : (j + 1) * P],
                                h2_tiles[dk][:, 0:CH],
                                start=(dk == 0), stop=(dk == 7),
                            )
                        gu = pools["gu"].tile([P, T], bf16, name="gu",
                                              tag="gu")
                        nc.scalar.activation(
                            gu[:, 0:CH], ps[:], AFT.Gelu_apprx_tanh,
                            bias=c["bu"][:, j : j + 1],
                        )
                        gu_part.append(gu)

    # ---------------- LN2 apply (half 1) ----------------
    ln_apply_half(st2, 1, x1_tiles, c["g2"], c["be2"], pools["h2"], h2_tiles)
    ln2_stack.close()
    ap_stack.close()

    # ---------------- MLP up + gelu ----------------
    gu_tiles = list(gu_part)
    with tc.tile_pool(name=f"u_ps{rep}", bufs=3, space="PSUM") as u_ps:
        # finish j0..7: token half 1
        for j in range(8):
            ps = u_ps.tile([P, CH], f32, name="u_ps1", tag="u_ps1", bufs=3)
            for dk in range(8):
                nc.tensor.matmul(
                    ps[:], wuA[dk][:, j * P : (j + 1) * P],
                    h2_tiles[dk][:, CH:T],
                    start=(dk == 0), stop=(dk == 7),
                )
            nc.scalar.activation(
                gu_part[j][:, CH:T], ps[:], AFT.Gelu_apprx_tanh,
                bias=c["bu"][:, j : j + 1],
            )
        # j8..31 full width, weights loaded in [128,1024] groups
        for grp in range(1, 4):
            wut = []
            for dk in range(8):
                wt = wu_p.tile([P, 1024], bf16, name="wu", tag="wu")
                nc.sync.dma_start(
                    wt[:],
                    wuT[dk * P : (dk + 1) * P, grp * 1024 : (grp + 1) * 1024],
                )
                wut.append(wt)
            for jl in range(8):
                j = grp * 8 + jl
                ps = u_ps.tile([P, T], f32, name="u_ps", tag="u_ps")
                for dk in range(8):
                    nc.tensor.matmul(
                        ps[:], wut[dk][:, jl * P : (jl + 1) * P], h2_tiles[dk][:],
                        start=(dk == 0), stop=(dk == 7),
                    )
                gu = pools["gu"].tile([P, T], bf16, name="gu", tag="gu")
                nc.scalar.activation(
                    gu[:], ps[:], AFT.Gelu_apprx_tanh, bias=c["bu"][:, j : j + 1]
                )
                gu_tiles.append(gu)
    wu_stack.close()

    # ---------------- MLP down + residual2 ----------------
    out_tiles = []
    with tc.tile_pool(name=f"wd{rep}", bufs=6) as wd_p, tc.tile_pool(
        name=f"d_ps{rep}", bufs=1, space="PSUM"
    ) as d_ps:
        pss = [d_ps.tile([P, T], f32, name=f"d_ps{do}", tag=f"d_ps{do}")
               for do in range(8)]
        for j in range(32):
            wt = wd_p.tile([P, D], bf16, name="wd", tag="wd")
            nc.sync.dma_start(wt[:], wdT[j * P : (j + 1) * P, :])
            for do in range(8):
                nc.tensor.matmul(
                    pss[do][:], wt[:, do * P : (do + 1) * P], gu_tiles[j][:],
                    start=(j == 0), stop=False,
                )
        for do in range(8):
            nc.tensor.matmul(
                pss[do][:], c["ident"][:], x1_tiles[do][:],
                start=False, stop=True,
            )
            o = pools["outp"].tile([P, T], f32, name="out_t", tag="out_t")
            nc.scalar.activation(
                o[:], pss[do][:], AFT.Identity, bias=c["bd"][:, do : do + 1]
            )
            if write_out:
                nc.sync.dma_start(out[do * P : (do + 1) * P, :], o[:])
            out_tiles.append(o)
    return out_tiles


def _build(nreps=1, collectives=True, hw_loop=0):
    from contextlib import ExitStack
    from concourse import bass, mybir, tile, bacc

    f32 = mybir.dt.float32
    bf16 = mybir.dt.bfloat16

    nc = bacc.Bacc("TRN2", target_bir_lowering=False, num_devices=NCORES)

    xT = nc.declare_dram_parameter("xT", [D, T], bf16, isOutput=False)
    wT = nc.declare_dram_parameter("wT", [D, 3 * D], bf16, isOutput=False)
    wpT = nc.declare_dram_parameter("wpT", [D, D], bf16, isOutput=False)
    wuT = nc.declare_dram_parameter("wuT", [D, DFF], bf16, isOutput=False)
    wdT = nc.declare_dram_parameter("wdT", [DFF, D], bf16, isOutput=False)
    bqk = nc.declare_dram_parameter("bqk", [P, 16], f32, isOutput=False)
    bv = nc.declare_dram_parameter("bv", [P, D], f32, isOutput=False)
    bp = nc.declare_dram_parameter("bp", [P, 8], f32, isOutput=False)
    bu = nc.declare_dram_parameter("bu", [P, 32], f32, isOutput=False)
    bd = nc.declare_dram_parameter("bd", [P, 8], f32, isOutput=False)
    g1 = nc.declare_dram_parameter("g1", [P, 8], f32, isOutput=False)
    be1 = nc.declare_dram_parameter("be1", [P, 8], f32, isOutput=False)
    g2 = nc.declare_dram_parameter("g2", [P, 8], f32, isOutput=False)
    be2 = nc.declare_dram_parameter("be2", [P, 8], f32, isOutput=False)
    ident = nc.declare_dram_parameter("ident", [P, P], bf16, isOutput=False)
    tri = nc.declare_dram_parameter("tri", [CH, CH], bf16, isOutput=False)
    tri2 = nc.declare_dram_parameter("tri2", [CH, 4 * CH], bf16, isOutput=False)
    out = nc.declare_dram_parameter("out", [D, T], f32, isOutput=True)

    # Half-split bounce buffers: rows [h*8*SLOT, (h+1)*8*SLOT) hold batch-half
    # h (256 token cols) so each half's AllToAll is a contiguous slab.
    f8 = mybir.dt.float8e4
    a1i = nc.dram_tensor("a2a1_in", [2 * NCORES * SLOT, CH], f8)
    a1o = nc.dram_tensor("a2a1_out", [2 * NCORES * SLOT, CH], f8)
    a2i = nc.dram_tensor("a2a2_in", [2 * NCORES * QR, CH], bf16)
    a2o = nc.dram_tensor("a2a2_out", [2 * NCORES * QR, CH], bf16)

    with tile.TileContext(nc) as tc, ExitStack() as top:
        xt_pool = top.enter_context(tc.tile_pool(name="xt", bufs=8))
        x_tiles = []
        for dk in range(8):
            xt = xt_pool.tile([P, T], bf16, name="xt", tag="xt")
            nc.sync.dma_start(xt[:], xT[dk * P : (dk + 1) * P, :])
            x_tiles.append(xt)
        const = top.enter_context(tc.tile_pool(name="const", bufs=1))
        ones = const.tile([P, 1], bf16)
        nc.vector.memset(ones[:], 1.0)
        ones_f = const.tile([P, 1], f32)
        nc.vector.memset(ones_f[:], 1.0)
        ones_row = const.tile([1, P], bf16)
        nc.vector.memset(ones_row[:], 1.0)
        ident_t = const.tile([P, P], bf16, name="ident_t", tag="ident_t")
        nc.sync.dma_start(ident_t[:], ident[:, :])
        tri_t = [const.tile([P, CH], bf16, name=f"tri{s}", tag=f"tri{s}") for s in range(2)]
        for s in range(2):
            nc.sync.dma_start(tri_t[s][:], tri[s * P : (s + 1) * P, :])
        tri2_t = [const.tile([P, 4 * CH], bf16, name=f"tri2{s}", tag=f"tri2{s}") for s in range(2)]
        for s in range(2):
            nc.sync.dma_start(tri2_t[s][:], tri2[s * P : (s + 1) * P, :])

        def ctile(name, param, shape):
            t = const.tile(shape, f32, name=name, tag=name)
            nc.sync.dma_start(t[:], param[:, :])
            return t

        consts = {
            "ones": ones, "ones_f": ones_f, "ones_row": ones_row, "tri": tri_t,
            "ident": ident_t,
            "tri_lo": [tri2_t[s][:, 0 : 2 * CH] for s in range(2)],
            "tri_hi": [tri2_t[s][:, 2 * CH : 4 * CH] for s in range(2)],
            "bqk": ctile("bqk_t", bqk, [P, 16]),
            "bv": ctile("bv_t", bv, [P, D]),
            "bp": ctile("bp_t", bp, [P, 8]),
            "bu": ctile("bu_t", bu, [P, 32]),
            "bd": ctile("bd_t", bd, [P, 8]),
            "g1": ctile("g1_t", g1, [P, 8]),
            "be1": ctile("be1_t", be1, [P, 8]),
            "g2": ctile("g2_t", g2, [P, 8]),
            "be2": ctile("be2_t", be2, [P, 8]),
        }

        pools = {
            "vec": top.enter_context(tc.tile_pool(name="vec", bufs=1)),
            "recp": top.enter_context(tc.tile_pool(name="recp", bufs=2)),
            "ht": top.enter_context(tc.tile_pool(name="ht", bufs=8)),
            "stg": top.enter_context(tc.tile_pool(name="stg", bufs=2)),
            "a2stg": top.enter_context(tc.tile_pool(name="a2stg", bufs=1)),
            "ot": top.enter_context(tc.tile_pool(name="ot", bufs=1)),
            "x1": top.enter_context(tc.tile_pool(name="x1", bufs=8)),
            "h2": top.enter_context(tc.tile_pool(name="h2", bufs=8)),
            "gu": top.enter_context(tc.tile_pool(name="gu", bufs=32)),
            "outp": top.enter_context(tc.tile_pool(name="outp", bufs=8)),
        }

        env = {
            "params": (xT, wT, wpT, wuT, wdT, out),
            "bounce": (a1i, a1o, a2i, a2o),
            "consts": consts,
            "pools": pools,
        }

        if hw_loop:
            with tc.For_i(0, hw_loop):
                _emit_block(nc, tc, env, 0, x_tiles, None, collectives, write_out=True)
        else:
            cur = x_tiles
            for rep in range(nreps):
                cur = _emit_block(
                    nc, tc, env, rep, cur, None, collectives,
                    write_out=(rep == nreps - 1),
                )

    nc.finalize()
    return nc


def _get_nc():
    if "nc" not in _CACHE:
        _CACHE["nc"] = _build()
    return _CACHE["nc"]


def _make_in_maps(inputs):
    x = np.asarray(inputs["x"], np.float32)
    ln1_g = np.asarray(inputs["ln1_g"], np.float32)
    ln1_b = np.asarray(inputs["ln1_b"], np.float32)
    W_attn = np.asarray(inputs["W_attn"], np.float32)
    b_attn = np.asarray(inputs["b_attn"], np.float32)
    W_proj = np.asarray(inputs["W_proj"], np.float32)
    b_proj = np.asarray(inputs["b_proj"], np.float32)
    ln2_g = np.asarray(inputs["ln2_g"], np.float32)
    ln2_b = np.asarray(inputs["ln2_b"], np.float32)
    W_up = np.asarray(inputs["W_up"], np.float32)
    b_up = np.asarray(inputs["b_up"], np.float32)
    W_down = np.asarray(inputs["W_down"], np.float32)
    b_down = np.asarray(inputs["b_down"], np.float32)

    bf = ml_dtypes.bfloat16
    wT = np.ascontiguousarray(W_attn.T).astype(bf)
    wpT = np.ascontiguousarray(W_proj.T).astype(bf)
    wuT = np.ascontiguousarray(W_up.T).astype(bf)
    wdT = np.ascontiguousarray(W_down.T).astype(bf)

    def cols(v):  # [N] -> [128, N//128]: col j = v[j*128:(j+1)*128]
        return np.ascontiguousarray(v.reshape(-1, P).T).astype(np.float32)

    tri = np.tril(np.ones((CH, CH), np.float32)).T.astype(bf)  # tri[a,b] = a<=b
    tri = np.ascontiguousarray(tri)

    ones_m = np.ones((CH, CH), np.float32)
    zeros_m = np.zeros((CH, CH), np.float32)
    tri_f = np.tril(np.ones((CH, CH), np.float32)).T
    tri2 = np.ascontiguousarray(
        np.concatenate([tri_f, ones_m, zeros_m, tri_f], axis=1)
    ).astype(bf)

    common = dict(
        wT=wT, wpT=wpT, wuT=wuT, wdT=wdT, tri2=tri2,
        ident=np.eye(P, dtype=bf),
        bqk=cols(b_attn[: 2 * D]),
        bv=np.ascontiguousarray(np.broadcast_to(b_attn[2 * D :].reshape(1, D), (P, D))),
        bp=cols(b_proj), bu=cols(b_up), bd=cols(b_down),
        g1=cols(ln1_g), be1=cols(ln1_b), g2=cols(ln2_g), be2=cols(ln2_b),
        tri=tri,
    )

    in_maps = []
    for i in range(NCORES):
        c0 = x[0, i * CH : (i + 1) * CH]  # [256, 1024]
        c1 = x[1, (7 - i) * CH : (8 - i) * CH]
        xTi = np.ascontiguousarray(np.concatenate([c0, c1], 0).T).astype(bf)
        in_maps.append(dict(common, xT=xTi))
    return in_maps


def make_in_maps(inputs):
    return _make_in_maps(inputs)


def kernel(**inputs):
    in_maps = _make_in_maps(inputs)

    from concourse import bass_utils

    nc = _get_nc()
    res = bass_utils.run_bass_kernel_spmd(
        nc, in_maps, core_ids=list(range(NCORES)), trace=TRACE
    )
    _CACHE["last_res"] = res
    y = np.empty((B, S, D), np.float32)
    for i in range(NCORES):
        o = np.asarray(res.results[i]["out"], np.float32)  # [1024, 512]
        y[0, i * CH : (i + 1) * CH] = o[:, :CH].T
        y[1, (7 - i) * CH : (8 - i) * CH] = o[:, CH:].T
    return y
